# revision 3
# baseline (speedup 1.0000x reference)
"""CPFStudent (GNN label propagation + MLP mix) on 8 TRN2 NeuronCores.

Strategy (dst-sharded SpMM with selector matmuls), v2:
  - Reference: 10 PLP steps of plp <- where(mask, hard, A_hat @ plp), with
    A_hat = D^-1/2 (A+I) D^-1/2 built from out-degrees of edge_index[0];
    final logits = sigmoid(alpha)*plp + (1-sigmoid(alpha))*relu(x@W1^T+b1)@W2^T+b2.
  - Only non-train (NT) rows of plp evolve; train (T) rows are constant after
    step 1.  State kept as table = dis * plp (dis = deg^-1/2), fp16:
        plp_new[d] = dis[d] * ( sum_{e: src NT} table[src] + c )
    where c is a per-dst constant: c1 (dis*label_init over T srcs, step 1) or
    c2 (dis*hard over T srcs, steps 2..10).  c1/c2 are precomputed HOST-side
    (they are iteration-invariant) — no T-side SpMM passes on device.
  - Nodes permuted host-side: NT nodes first, padded per-core stripes.  Each
    core owns a contiguous stripe of NT dst rows; edges bucketed by (dst tile
    of 128, src chunk of <=32768) host-side, padded to uniform capacities
    across cores (SPMD), gathered per iteration with gpsimd.dma_gather (256B
    elements) from an HBM fp16 table, spread round-robin over 4 SWDGE queues.
  - Scatter/segment-sum on the TensorEngine: per 128-edge slot a
    host-precomputed fp8 selector S (S[e,d]=1 iff dst_local(e)==d) multiplies
    the gathered messages, accumulating in PSUM per dst tile.
  - Halo exchange: AllGather of each core's full-width (padded 256B) rows
    directly produces the next iteration's gather table — no post-collective
    re-strided table write.
"""

import math
import os
import sys

import numpy as np

sys.path.insert(0, "/opt/trn_rl_repo")

import ml_dtypes  # noqa: E402

import concourse.bass as bass  # noqa: E402
import concourse.mybir as mybir  # noqa: E402
import concourse.tile as tile  # noqa: E402
from concourse import bacc  # noqa: E402
from concourse.bass_utils import run_bass_kernel_spmd  # noqa: E402

P = 128
NCORES = 8
TPAD = 128  # fp16 elements per table row (256B, dma_gather elem granularity)
GROUP = 7  # dst tiles per dma_gather call group
MAX_CALL = int(os.environ.get("KERNEL_MAX_CALL", "1024"))
NQ = int(os.environ.get("KERNEL_NQ", "1"))
EXCHANGE = os.environ.get("KERNEL_EXCHANGE", "pad")

F16 = mybir.dt.float16
F32 = mybir.dt.float32
F8 = mybir.dt.float8e4
I16 = mybir.dt.int16
NP_F8 = ml_dtypes.float8_e4m3


def _ceil(a, b):
    return -(-a // b)


class BuildOnly(Exception):
    pass


class EdgePlan:
    """Host-side bucketed edge plan for one SpMM pass, uniform across cores.

    src_row: int array, row index into the pass's gather table
    dst_pid: int array, padded NT id of the destination
    """

    def __init__(self, src_row, dst_pid, n_rows, s_pad, n_tiles):
        self.n_chunks = max(1, _ceil(n_rows, 32768))
        self.chunk = _ceil(n_rows, self.n_chunks)
        self.n_tiles = n_tiles
        nch = self.n_chunks

        core = dst_pid // s_pad
        dloc = dst_pid - core * s_pad
        tl = dloc // P
        dstloc = dloc % P
        ch = src_row // self.chunk

        key = (core * n_tiles + tl) * nch + ch
        counts = np.bincount(key, minlength=NCORES * n_tiles * nch).reshape(
            NCORES, n_tiles, nch
        )
        caps = counts.max(axis=0)  # [n_tiles, nch]
        caps = ((caps + P - 1) // P) * P
        self.caps = caps
        self.slots_per_tile = caps.sum(axis=1) // P  # [n_tiles]
        self.s_off = np.concatenate([[0], np.cumsum(self.slots_per_tile)])
        self.total_slots = int(self.s_off[-1])

        # per (chunk, group) call: num idxs and per-tile column offsets
        self.n_groups = _ceil(n_tiles, GROUP)
        self.call_num = np.zeros((nch, self.n_groups), dtype=np.int64)
        self.buck_col = np.zeros((nch, n_tiles), dtype=np.int64)  # col in its call buf
        for c in range(nch):
            for g in range(self.n_groups):
                off = 0
                for t in range(g * GROUP, min((g + 1) * GROUP, n_tiles)):
                    self.buck_col[c, t] = off
                    off += caps[t, c] // P
                self.call_num[c, g] = off * P
        # col offset of each call inside the flat idx stream (per chunk then group)
        self.call_off = np.zeros((nch, self.n_groups), dtype=np.int64)
        off = 0
        for c in range(nch):
            for g in range(self.n_groups):
                self.call_off[c, g] = off
                off += self.call_num[c, g]
        self.total_idx = off

        # sub-calls of <= MAX_CALL idxs: per (c, g) a list of (idx_off, num, col0)
        self.subcalls = {}
        for c in range(nch):
            for g in range(self.n_groups):
                num = int(self.call_num[c, g])
                base = int(self.call_off[c, g])
                subs = []
                p0 = 0
                while p0 < num:
                    n_ = min(MAX_CALL, num - p0)
                    subs.append((base + p0, n_, p0 // P))
                    p0 += n_
                self.subcalls[(c, g)] = subs

        # order edges by (core, chunk, tile); build padded per-core streams
        order = np.argsort((core * nch + ch) * n_tiles + tl, kind="stable")
        src_o = src_row[order]
        core_o = core[order]
        ch_o = ch[order]
        tl_o = tl[order]
        dst_o = dstloc[order]

        # destination position of each edge in the padded stream
        # padded stream order: for chunk c, group g, tile t in g: cap[t,c] entries
        base_tc = np.zeros((nch, n_tiles), dtype=np.int64)
        for c in range(nch):
            for g in range(self.n_groups):
                for t in range(g * GROUP, min((g + 1) * GROUP, n_tiles)):
                    base_tc[c, t] = self.call_off[c, g] + self.buck_col[c, t] * P

        self.idx16 = np.zeros((NCORES, self.total_idx), dtype=np.int16)
        self.dstloc = np.full((NCORES, self.total_idx), -1, dtype=np.int16)
        # rank of each edge within its (core, chunk, tile) bucket
        grp_key = (core_o * nch + ch_o) * n_tiles + tl_o
        # stable sort keeps original order; compute rank via cumcount
        uniq, inv, cnt = np.unique(grp_key, return_inverse=True, return_counts=True)
        starts = np.concatenate([[0], np.cumsum(cnt)])[:-1]
        rank = np.arange(len(grp_key)) - starts[inv]
        pos = base_tc[ch_o, tl_o] + rank
        self.idx16[core_o, pos] = (src_o - ch_o * self.chunk).astype(np.int16)
        self.dstloc[core_o, pos] = dst_o.astype(np.int16)

    def wrapped_idx(self, core):
        """[128, total_idx//16] int16, wrapped-16 and replicated to 8 groups."""
        v = self.idx16[core].reshape(-1, 16).T  # [16, total/16]
        return np.tile(v, (8, 1)).copy()

    def s_blob(self, core):
        """[128, total_slots*128] fp8: per slot S[e,d] = (dstloc[e]==d).

        Slot order: tile-major (tile t: its chunk-0 slots then chunk-1 slots),
        matching the matmul loop.  Column range of tile t: s_off[t]*128.
        """
        nch = self.n_chunks
        out = np.zeros((P, self.total_slots * P), dtype=NP_F8)
        iota = np.arange(P, dtype=np.int16)
        for t in range(self.n_tiles):
            si = self.s_off[t]
            for c in range(nch):
                nsl = self.caps[t, c] // P
                if nsl == 0:
                    continue
                g = t // GROUP
                base = self.call_off[c, g] + self.buck_col[c, t] * P
                d = self.dstloc[core, base : base + nsl * P].reshape(nsl, P)
                # S [slot, e, d]
                s = (d[:, :, None] == iota[None, None, :]).astype(NP_F8)
                # [P(e), nsl, P(d)] -> columns
                out[:, si * P : (si + nsl) * P] = (
                    s.transpose(1, 0, 2).reshape(P, nsl * P)
                )
                si += nsl
        return out


def _build_program(pm, s_pad, st_pad, tn, tt):
    """pm: main-pass EdgePlan (NT->NT)."""
    nt_pad = NCORES * s_pad
    nc = bacc.Bacc(
        None, target_bir_lowering=False, num_devices=NCORES, num_swdge_queues=NQ
    )

    def param(name, shape, dt, out=False):
        return nc.declare_dram_parameter(name, list(shape), dt, isOutput=out)

    tbl_init = param("tbl_init", (nt_pad, TPAD), F16)
    idx_nt = param("idx_nt", (P, pm.total_idx // 16), I16)
    s_nt = param("s_nt", (P, pm.total_slots * P), F8)
    xnt = param("xnt", (512, s_pad), F16)  # pre-transposed on host
    xt = param("xt", (512, st_pad), F16)
    w1t = param("w1t", (512, 256), F16)
    b1 = param("b1", (256, 1), F32)
    w2t = param("w2t", (256, 40), F16)
    b2b = param("b2b", (P, 40), F32)
    alpha_nt = param("alpha_nt", (s_pad, 1), F32)
    alpha_t = param("alpha_t", (st_pad, 1), F32)
    dis_nt = param("dis_nt", (s_pad, 1), F32)
    dissq_nt = param("dissq_nt", (s_pad, 1), F32)
    c1p = param("c1p", (s_pad, 40), F32)
    c2p = param("c2p", (s_pad, 40), F32)
    hard_t = param("hard_t", (st_pad, 40), F32)
    out_nt = param("out_nt", (s_pad, 40), F32, out=True)
    out_t = param("out_t", (st_pad, 40), F32, out=True)

    if EXCHANGE == "pad":
        cown = nc.dram_tensor("cown", [s_pad, TPAD], F16)
        callg = nc.dram_tensor("callg", [nt_pad, TPAD], F16, addr_space="Shared")
        table = None
    else:
        cown = nc.dram_tensor("cown", [s_pad, 40], F16)
        callg = nc.dram_tensor("callg", [nt_pad, 40], F16, addr_space="Shared")
        table = nc.dram_tensor("table", [nt_pad, TPAD], F16)

    RG = [list(range(NCORES))]

    with tile.TileContext(nc) as tc:
        with (
            tc.tile_pool(name="persist", bufs=1) as pp,
            tc.tile_pool(name="work", bufs=4) as wp,
            tc.tile_pool(name="gpool", bufs=4) as gp,
            tc.tile_pool(name="spool", bufs=3) as sp,
            tc.tile_pool(name="mpsum", bufs=2, space="PSUM") as mp,
            tc.tile_pool(name="apsum", bufs=4, space="PSUM") as ap_,
        ):
            if table is not None:
                nc.sync.dma_start(out=table[:, :], in_=tbl_init[:, :])

            # ---- persistent SBUF ----
            idxm_sb = pp.tile([P, pm.total_idx // 16], I16, tag="idxm")
            nc.sync.dma_start(out=idxm_sb[:], in_=idx_nt[:, :])

            ft_nt = pp.tile([P, tn, 40], F32, tag="ftnt")
            ft_t = pp.tile([P, tt, 40], F32, tag="ftt")
            cwid = TPAD if EXCHANGE == "pad" else 40
            compact = pp.tile([P, tn, cwid], F16, tag="compact")
            if EXCHANGE == "pad":
                nc.vector.memset(compact[:], 0.0)

            w1_sb = pp.tile([P, 4, 256], F16, tag="w1")
            nc.sync.dma_start(
                out=w1_sb[:], in_=w1t.ap().rearrange("(k p) h -> p k h", p=P)
            )
            w2_sb = pp.tile([P, 2, 40], F16, tag="w2")
            nc.sync.dma_start(
                out=w2_sb[:], in_=w2t.ap().rearrange("(h p) c -> p h c", p=P)
            )
            b1_sb = pp.tile([P, 2], F32, tag="b1")
            nc.sync.dma_start(
                out=b1_sb[:], in_=b1.ap().rearrange("(h p) o -> p (h o)", p=P)
            )
            b2_sb = pp.tile([P, 40], F32, tag="b2")
            nc.sync.dma_start(out=b2_sb[:], in_=b2b[:, :])

            c1 = pp.tile([P, tn, 40], F32, tag="c1")
            nc.sync.dma_start(
                out=c1[:], in_=c1p.ap().rearrange("(t p) c -> p t c", p=P)
            )
            c2 = pp.tile([P, tn, 40], F32, tag="c2")
            nc.sync.dma_start(
                out=c2[:], in_=c2p.ap().rearrange("(t p) c -> p t c", p=P)
            )

            def cols_load(prm, n_tiles, tag):
                t_ = pp.tile([P, n_tiles], F32, tag=tag)
                nc.sync.dma_start(
                    out=t_[:], in_=prm.ap().rearrange("(t p) o -> p (t o)", p=P)
                )
                return t_

            disn_sb = cols_load(dis_nt, tn, "disn")
            dsqn_sb = cols_load(dissq_nt, tn, "dsqn")
            aln_sb = cols_load(alpha_nt, tn, "aln")
            alt_sb = cols_load(alpha_t, tt, "alt")

            # sigmoid(alpha); a*dis; 1-a
            sign_sb = pp.tile([P, tn], F32, tag="sign")
            nc.scalar.activation(
                sign_sb[:], aln_sb[:], mybir.ActivationFunctionType.Sigmoid
            )
            sigt_sb = pp.tile([P, tt], F32, tag="sigt")
            nc.scalar.activation(
                sigt_sb[:], alt_sb[:], mybir.ActivationFunctionType.Sigmoid
            )
            disa_sb = pp.tile([P, tn], F32, tag="disa")
            nc.vector.tensor_tensor(
                out=disa_sb[:], in0=sign_sb[:], in1=disn_sb[:],
                op=mybir.AluOpType.mult,
            )
            oman_sb = pp.tile([P, tn], F32, tag="oman")
            nc.vector.tensor_scalar(
                out=oman_sb[:], in0=sign_sb[:], scalar1=-1.0, scalar2=1.0,
                op0=mybir.AluOpType.mult, op1=mybir.AluOpType.add,
            )
            omat_sb = pp.tile([P, tt], F32, tag="omat")
            nc.vector.tensor_scalar(
                out=omat_sb[:], in0=sigt_sb[:], scalar1=-1.0, scalar2=1.0,
                op0=mybir.AluOpType.mult, op1=mybir.AluOpType.add,
            )

            # ---- MLP (FT branch) ----
            def mlp(xsrc, n_tiles, ft_dst):
                for n in range(n_tiles):
                    xTs = []
                    for k in range(4):
                        xT = wp.tile([P, P], F16, tag="xT")
                        nc.sync.dma_start(
                            out=xT[:],
                            in_=xsrc[k * P : (k + 1) * P, n * P : (n + 1) * P],
                        )
                        xTs.append(xT)
                    ps2 = mp.tile([P, 40], F32, tag="ps2")
                    for h in range(2):
                        ps1 = mp.tile([P, P], F32, tag="ps1")
                        for k in range(4):
                            nc.tensor.matmul(
                                ps1[:],
                                lhsT=w1_sb[:, k, h * P : (h + 1) * P],
                                rhs=xTs[k][:],
                                start=(k == 0),
                                stop=(k == 3),
                            )
                        hT = wp.tile([P, P], F16, tag="hT")
                        nc.scalar.activation(
                            hT[:], ps1[:], mybir.ActivationFunctionType.Relu,
                            bias=b1_sb[:, h : h + 1],
                        )
                        nc.tensor.matmul(
                            ps2[:], lhsT=hT[:], rhs=w2_sb[:, h, :],
                            start=(h == 0), stop=(h == 1),
                        )
                    nc.vector.tensor_tensor(
                        out=ft_dst[:, n, :], in0=ps2[:], in1=b2_sb[:],
                        op=mybir.AluOpType.add,
                    )

            mlp(xnt, tn, ft_nt)
            mlp(xt, tt, ft_t)

            # ---- generic SpMM pass ----
            _regs = {}

            def num_reg(v):
                if v not in _regs:
                    _regs[v] = nc.gpsimd.to_reg(v)
                return _regs[v]

            _q = [0]

            def spmm_pass(plan, tsrc, idx_sb, s_param, evac):
                """tsrc: DRAM table. evac(t, psum_ap) -> emits eviction."""
                nch = plan.n_chunks
                for g in range(plan.n_groups):
                    gbufs = []
                    for c in range(nch):
                        num = int(plan.call_num[c, g])
                        if num == 0:
                            gbufs.append(None)
                            continue
                        gb = gp.tile([P, num // P, TPAD], F16, tag="gb")
                        r0 = c * plan.chunk
                        nrow = plan.chunk
                        if os.environ.get("KERNEL_NO_GATHER", "0") == "1":
                            # debug: sequential read instead of gather
                            nc.sync.dma_start(
                                out=gb[:],
                                in_=tsrc[r0 : r0 + num, :].rearrange(
                                    "(n p) e -> p n e", p=P
                                ),
                            )
                        else:
                            for off, n_, col0 in plan.subcalls[(c, g)]:
                                nc.gpsimd.dma_gather(
                                    out_ap=gb[:, col0 : col0 + n_ // P, :],
                                    in_ap=tsrc[r0 : r0 + nrow, :],
                                    idxs_ap=idx_sb[:, off // 16 : (off + n_) // 16],
                                    num_idxs=n_,
                                    num_idxs_reg=num_reg(n_),
                                    elem_size=TPAD,
                                    queue_num=_q[0] % NQ,
                                )
                                _q[0] += 1
                        gbufs.append(gb)
                    for t in range(g * GROUP, min((g + 1) * GROUP, plan.n_tiles)):
                        tot = int(plan.slots_per_tile[t])
                        if tot == 0:
                            continue
                        si = int(plan.s_off[t])
                        st_ = sp.tile([P, tot * P], F8, tag="sstr")
                        nc.sync.dma_start(
                            out=st_[:], in_=s_param[:, si * P : (si + tot) * P]
                        )
                        ps = ap_.tile([P, 40], F32, tag="acc")
                        k = 0
                        for c in range(nch):
                            nsl = int(plan.caps[t, c]) // P
                            bc = int(plan.buck_col[c, t])
                            for j in range(nsl):
                                nc.tensor.matmul(
                                    ps[:],
                                    lhsT=st_[:, k * P : (k + 1) * P],
                                    rhs=gbufs[c][:, bc + j, 0:40],
                                    start=(k == 0),
                                    stop=(k == tot - 1),
                                )
                                k += 1
                        evac(t, ps)

            # ---- 10 PLP iterations ----
            for it in range(10):
                if EXCHANGE == "pad":
                    tsrc = tbl_init if it == 0 else callg
                else:
                    tsrc = table
                cbuf = c1 if it == 0 else c2

                if it < 9:
                    def evac_iter(t, ps, cbuf=cbuf):
                        tmp = wp.tile([P, 40], F32, tag="ev")
                        nc.vector.tensor_tensor(
                            out=tmp[:], in0=ps[:], in1=cbuf[:, t, :],
                            op=mybir.AluOpType.add,
                        )
                        nc.vector.tensor_scalar(
                            out=compact[:, t, 0:40], in0=tmp[:],
                            scalar1=dsqn_sb[:, t : t + 1], scalar2=None,
                            op0=mybir.AluOpType.mult,
                        )
                else:
                    def evac_iter(t, ps, cbuf=cbuf):
                        tmp = wp.tile([P, 40], F32, tag="ev")
                        nc.vector.tensor_tensor(
                            out=tmp[:], in0=ps[:], in1=cbuf[:, t, :],
                            op=mybir.AluOpType.add,
                        )
                        t2 = wp.tile([P, 40], F32, tag="ev2")
                        nc.vector.tensor_scalar(
                            out=t2[:], in0=tmp[:],
                            scalar1=disa_sb[:, t : t + 1], scalar2=None,
                            op0=mybir.AluOpType.mult,
                        )
                        t3 = wp.tile([P, 40], F32, tag="ev3")
                        nc.vector.tensor_scalar(
                            out=t3[:], in0=ft_nt[:, t, :],
                            scalar1=oman_sb[:, t : t + 1], scalar2=None,
                            op0=mybir.AluOpType.mult,
                        )
                        t4 = wp.tile([P, 40], F32, tag="ev4")
                        nc.vector.tensor_tensor(
                            out=t4[:], in0=t2[:], in1=t3[:],
                            op=mybir.AluOpType.add,
                        )
                        nc.sync.dma_start(
                            out=out_nt[t * P : (t + 1) * P, :], in_=t4[:]
                        )

                spmm_pass(pm, tsrc, idxm_sb, s_nt, evac_iter)

                if it < 9:
                    nc.sync.dma_start(
                        out=cown.ap().rearrange("(t p) c -> p t c", p=P),
                        in_=compact[:],
                    )
                    if os.environ.get("KERNEL_NO_CC", "0") == "1":
                        # debug mode: skip the collective (wrong cross-core data)
                        nc.sync.dma_start(
                            out=callg[0 : s_pad, :], in_=cown[:, :]
                        )
                    else:
                        nc.gpsimd.collective_compute(
                            "AllGather",
                            mybir.AluOpType.bypass,
                            replica_groups=RG,
                            ins=[cown.ap().opt()],
                            outs=[callg.ap().opt()],
                        )
                    if EXCHANGE != "pad":
                        nc.sync.dma_start(out=table[:, 0:40], in_=callg[:, :])

            # ---- T-side final combine ----
            for t in range(tt):
                hsb = wp.tile([P, 40], F32, tag="hsb")
                nc.sync.dma_start(out=hsb[:], in_=hard_t[t * P : (t + 1) * P, :])
                t1_ = wp.tile([P, 40], F32, tag="tc1")
                nc.vector.tensor_scalar(
                    out=t1_[:], in0=hsb[:], scalar1=sigt_sb[:, t : t + 1],
                    scalar2=None, op0=mybir.AluOpType.mult,
                )
                t2_ = wp.tile([P, 40], F32, tag="tc2")
                nc.vector.tensor_scalar(
                    out=t2_[:], in0=ft_t[:, t, :], scalar1=omat_sb[:, t : t + 1],
                    scalar2=None, op0=mybir.AluOpType.mult,
                )
                t3_ = wp.tile([P, 40], F32, tag="tc3")
                nc.vector.tensor_tensor(
                    out=t3_[:], in0=t1_[:], in1=t2_[:],
                    op=mybir.AluOpType.add,
                )
                nc.sync.dma_start(out=out_t[t * P : (t + 1) * P, :], in_=t3_[:])

    nc.compile()
    return nc


def kernel(**inputs):
    x = np.asarray(inputs["x"], dtype=np.float32)
    edge_index = np.asarray(inputs["edge_index"])
    label_init = np.asarray(inputs["label_init"], dtype=np.float32)
    train_mask = np.asarray(inputs["train_mask"]).astype(bool)
    hard = np.asarray(inputs["hard_one_hot"], dtype=np.float32)
    fc1_w = np.asarray(inputs["fc1_w"], dtype=np.float32)
    fc1_b = np.asarray(inputs["fc1_b"], dtype=np.float32)
    fc2_w = np.asarray(inputs["fc2_w"], dtype=np.float32)
    fc2_b = np.asarray(inputs["fc2_b"], dtype=np.float32)
    alpha = np.asarray(inputs["alpha"], dtype=np.float32)

    n = x.shape[0]
    row = edge_index[0].astype(np.int64)
    col = edge_index[1].astype(np.int64)

    deg = np.bincount(row, minlength=n).astype(np.float64) + 1.0
    dis = (1.0 / np.sqrt(deg)).astype(np.float32)

    nt_ids = np.nonzero(~train_mask)[0]
    t_ids = np.nonzero(train_mask)[0]
    n_nt, n_t = len(nt_ids), len(t_ids)

    s_real = _ceil(n_nt, NCORES)
    tn = _ceil(s_real, P)
    s_pad = tn * P
    nt_pad = NCORES * s_pad
    st_real = _ceil(n_t, NCORES)
    tt = _ceil(st_real, P)
    st_pad = tt * P

    # padded NT id / compact T id for each original node
    pid = np.full(n, -1, dtype=np.int64)
    j = np.arange(n_nt)
    stripe = j // s_real
    pid[nt_ids] = stripe * s_pad + (j - stripe * s_real)

    # edges into NT dsts
    sel = ~train_mask[col]
    es, ed = row[sel], col[sel]
    src_nt = ~train_mask[es]
    # main: NT->NT plus self-loops on NT
    m_src = np.concatenate([pid[es[src_nt]], pid[nt_ids]])
    m_dst = np.concatenate([pid[ed[src_nt]], pid[nt_ids]])
    pm = EdgePlan(m_src, m_dst, nt_pad, s_pad, tn)

    # ---- host-precomputed T-source constants c1/c2 ----
    scaled_li = dis[:, None] * label_init  # [n, 40]
    scaled_hd = dis[:, None] * hard
    es_t, ed_t = es[~src_nt], ed[~src_nt]
    d_pid_t = pid[ed_t]
    order_t = np.argsort(d_pid_t, kind="stable")
    d_sorted = d_pid_t[order_t]
    uniq_d, start_i = np.unique(d_sorted, return_index=True)
    c1_full = np.zeros((nt_pad, 40), dtype=np.float32)
    c2_full = np.zeros((nt_pad, 40), dtype=np.float32)
    if len(uniq_d):
        c1_full[uniq_d] = np.add.reduceat(
            scaled_li[es_t[order_t]].astype(np.float64), start_i, axis=0
        )
        c2_full[uniq_d] = np.add.reduceat(
            scaled_hd[es_t[order_t]].astype(np.float64), start_i, axis=0
        )

    # ---- tables ----
    tbl_init_g = np.zeros((nt_pad, TPAD), dtype=np.float16)
    tbl_init_g[pid[nt_ids], :40] = scaled_li[nt_ids].astype(np.float16)

    # ---- per-core MLP / combine inputs ----
    def stripe_rows(ids, srl, spad_, nstripes=NCORES):
        """Return [nstripes, spad_] original-id per padded slot (-1 pad)."""
        m = np.full((nstripes, spad_), -1, dtype=np.int64)
        for i in range(nstripes):
            lo = i * srl
            hi = min(len(ids), (i + 1) * srl)
            if hi > lo:
                m[i, : hi - lo] = ids[lo:hi]
        return m

    nt_map = stripe_rows(nt_ids, s_real, s_pad)
    t_map = stripe_rows(t_ids, st_real, st_pad)

    def take(arr, idmap, fill=0.0):
        out = np.full((idmap.shape[0], idmap.shape[1]) + arr.shape[1:], fill,
                      dtype=arr.dtype)
        valid = idmap >= 0
        out[valid] = arr[idmap[valid]]
        return out

    xnt_g = np.ascontiguousarray(
        take(x, nt_map).astype(np.float16).transpose(0, 2, 1)
    )
    xt_g = np.ascontiguousarray(take(x, t_map).astype(np.float16).transpose(0, 2, 1))
    al_nt_g = take(alpha, nt_map).astype(np.float32)
    al_t_g = take(alpha, t_map).astype(np.float32)
    dis_nt_g = take(dis[:, None], nt_map).astype(np.float32)
    dsq_nt_g = take((dis * dis)[:, None], nt_map).astype(np.float32)
    hard_t_g = take(hard, t_map).astype(np.float32)

    w1t_g = fc1_w.T.astype(np.float16).copy()  # [512, 256]
    b1_g = fc1_b.reshape(256, 1).astype(np.float32)
    w2t_g = fc2_w.T.astype(np.float16).copy()  # [256, 40]
    b2b_g = np.tile(fc2_b.reshape(1, 40), (P, 1)).astype(np.float32)

    nc = _build_program(pm, s_pad, st_pad, tn, tt)

    if os.environ.get("KERNEL_BUILD_ONLY", "0") == "1":
        e = BuildOnly()
        e.nc = nc
        raise e

    in_maps = []
    for i in range(NCORES):
        in_maps.append(
            dict(
                tbl_init=tbl_init_g,
                idx_nt=pm.wrapped_idx(i),
                s_nt=pm.s_blob(i),
                xnt=xnt_g[i],
                xt=xt_g[i],
                w1t=w1t_g,
                b1=b1_g,
                w2t=w2t_g,
                b2b=b2b_g,
                alpha_nt=al_nt_g[i],
                alpha_t=al_t_g[i],
                dis_nt=dis_nt_g[i],
                dissq_nt=dsq_nt_g[i],
                c1p=c1_full[i * s_pad : (i + 1) * s_pad],
                c2p=c2_full[i * s_pad : (i + 1) * s_pad],
                hard_t=hard_t_g[i],
            )
        )

    if os.environ.get("KERNEL_SIM", "0") == "1":
        from concourse import bass_interp

        sim = bass_interp.MultiCoreSim(nc, NCORES)
        for i in range(NCORES):
            for k, v in in_maps[i].items():
                sim.cores[i].tensor(k)[:] = v
        sim.simulate()
        results = [
            {k: np.array(sim.cores[i].mem_tensor(k)) for k in ("out_nt", "out_t")}
            for i in range(NCORES)
        ]
        res = None
    else:
        res = run_bass_kernel_spmd(
            nc, in_maps, core_ids=list(range(NCORES)),
            trace=bool(int(os.environ.get("KERNEL_TRACE", "0"))),
        )
        results = res.results
    kernel.last_results = res
    kernel.last_nc = nc
    kernel.last_in_maps = in_maps

    out = np.zeros((n, 40), dtype=np.float32)
    for i in range(NCORES):
        om = results[i]["out_nt"]
        ot = results[i]["out_t"]
        v = nt_map[i] >= 0
        out[nt_map[i][v]] = om[v]
        v = t_map[i] >= 0
        out[t_map[i][v]] = ot[v]
    return out


# revision 4
# speedup vs baseline: 1.2255x; 1.2255x over previous
"""CPFStudent (GNN label propagation + MLP mix) on 8 TRN2 NeuronCores.

Strategy (dst-sharded SpMM with selector matmuls), v2:
  - Reference: 10 PLP steps of plp <- where(mask, hard, A_hat @ plp), with
    A_hat = D^-1/2 (A+I) D^-1/2 built from out-degrees of edge_index[0];
    final logits = sigmoid(alpha)*plp + (1-sigmoid(alpha))*relu(x@W1^T+b1)@W2^T+b2.
  - Only non-train (NT) rows of plp evolve; train (T) rows are constant after
    step 1.  State kept as table = dis * plp (dis = deg^-1/2), fp16:
        plp_new[d] = dis[d] * ( sum_{e: src NT} table[src] + c )
    where c is a per-dst constant: c1 (dis*label_init over T srcs, step 1) or
    c2 (dis*hard over T srcs, steps 2..10).  c1/c2 are precomputed HOST-side
    (they are iteration-invariant) — no T-side SpMM passes on device.
  - Nodes permuted host-side: NT nodes first, padded per-core stripes.  Each
    core owns a contiguous stripe of NT dst rows; edges bucketed by (dst tile
    of 128, src chunk of <=32768) host-side, padded to uniform capacities
    across cores (SPMD), gathered per iteration with gpsimd.dma_gather (256B
    elements) from an HBM fp16 table, spread round-robin over 4 SWDGE queues.
  - Scatter/segment-sum on the TensorEngine: per 128-edge slot a
    host-precomputed fp8 selector S (S[e,d]=1 iff dst_local(e)==d) multiplies
    the gathered messages, accumulating in PSUM per dst tile.
  - Halo exchange: AllGather of each core's full-width (padded 256B) rows
    directly produces the next iteration's gather table — no post-collective
    re-strided table write.
"""

import math
import os
import sys

import numpy as np

sys.path.insert(0, "/opt/trn_rl_repo")

import ml_dtypes  # noqa: E402

import concourse.bass as bass  # noqa: E402
import concourse.mybir as mybir  # noqa: E402
import concourse.tile as tile  # noqa: E402
from concourse import bacc  # noqa: E402
from concourse.bass_utils import run_bass_kernel_spmd  # noqa: E402

P = 128
NCORES = 8
TPAD = 128  # fp16 elements per table row (256B, dma_gather elem granularity)
GROUP = 7  # dst tiles per dma_gather call group
MAX_CALL = int(os.environ.get("KERNEL_MAX_CALL", "1024"))
NQ = int(os.environ.get("KERNEL_NQ", "1"))
EXCHANGE = os.environ.get("KERNEL_EXCHANGE", "pad")

F16 = mybir.dt.float16
F32 = mybir.dt.float32
F8 = mybir.dt.float8e4
I16 = mybir.dt.int16
NP_F8 = ml_dtypes.float8_e4m3


def _ceil(a, b):
    return -(-a // b)


class BuildOnly(Exception):
    pass


class EdgePlan:
    """Host-side bucketed edge plan for one SpMM pass, uniform across cores.

    src_row: int array, row index into the pass's gather table
    dst_pid: int array, padded NT id of the destination
    """

    def __init__(self, src_row, dst_pid, n_rows, s_pad, n_tiles):
        self.n_chunks = max(1, _ceil(n_rows, 32768))
        self.chunk = _ceil(n_rows, self.n_chunks)
        self.n_tiles = n_tiles
        nch = self.n_chunks

        core = dst_pid // s_pad
        dloc = dst_pid - core * s_pad
        tl = dloc // P
        dstloc = dloc % P
        ch = src_row // self.chunk

        key = (core * n_tiles + tl) * nch + ch
        counts = np.bincount(key, minlength=NCORES * n_tiles * nch).reshape(
            NCORES, n_tiles, nch
        )
        caps = counts.max(axis=0)  # [n_tiles, nch]
        caps = ((caps + P - 1) // P) * P
        self.caps = caps
        self.slots_per_tile = caps.sum(axis=1) // P  # [n_tiles]
        self.s_off = np.concatenate([[0], np.cumsum(self.slots_per_tile)])
        self.total_slots = int(self.s_off[-1])

        # per (chunk, group) call: num idxs and per-tile column offsets
        self.n_groups = _ceil(n_tiles, GROUP)
        self.call_num = np.zeros((nch, self.n_groups), dtype=np.int64)
        self.buck_col = np.zeros((nch, n_tiles), dtype=np.int64)  # col in its call buf
        for c in range(nch):
            for g in range(self.n_groups):
                off = 0
                for t in range(g * GROUP, min((g + 1) * GROUP, n_tiles)):
                    self.buck_col[c, t] = off
                    off += caps[t, c] // P
                self.call_num[c, g] = off * P
        # col offset of each call inside the flat idx stream (per chunk then group)
        self.call_off = np.zeros((nch, self.n_groups), dtype=np.int64)
        off = 0
        for c in range(nch):
            for g in range(self.n_groups):
                self.call_off[c, g] = off
                off += self.call_num[c, g]
        self.total_idx = off

        # sub-calls of <= MAX_CALL idxs: per (c, g) a list of (idx_off, num, col0)
        self.subcalls = {}
        for c in range(nch):
            for g in range(self.n_groups):
                num = int(self.call_num[c, g])
                base = int(self.call_off[c, g])
                subs = []
                p0 = 0
                while p0 < num:
                    n_ = min(MAX_CALL, num - p0)
                    subs.append((base + p0, n_, p0 // P))
                    p0 += n_
                self.subcalls[(c, g)] = subs

        # order edges by (core, chunk, tile); build padded per-core streams
        order = np.argsort((core * nch + ch) * n_tiles + tl, kind="stable")
        src_o = src_row[order]
        core_o = core[order]
        ch_o = ch[order]
        tl_o = tl[order]
        dst_o = dstloc[order]

        # destination position of each edge in the padded stream
        # padded stream order: for chunk c, group g, tile t in g: cap[t,c] entries
        base_tc = np.zeros((nch, n_tiles), dtype=np.int64)
        for c in range(nch):
            for g in range(self.n_groups):
                for t in range(g * GROUP, min((g + 1) * GROUP, n_tiles)):
                    base_tc[c, t] = self.call_off[c, g] + self.buck_col[c, t] * P

        self.idx16 = np.zeros((NCORES, self.total_idx), dtype=np.int16)
        self.dstloc = np.full((NCORES, self.total_idx), -1, dtype=np.int16)
        # rank of each edge within its (core, chunk, tile) bucket
        grp_key = (core_o * nch + ch_o) * n_tiles + tl_o
        # stable sort keeps original order; compute rank via cumcount
        uniq, inv, cnt = np.unique(grp_key, return_inverse=True, return_counts=True)
        starts = np.concatenate([[0], np.cumsum(cnt)])[:-1]
        rank = np.arange(len(grp_key)) - starts[inv]
        pos = base_tc[ch_o, tl_o] + rank
        self.idx16[core_o, pos] = (src_o - ch_o * self.chunk).astype(np.int16)
        self.dstloc[core_o, pos] = dst_o.astype(np.int16)

    def wrapped_idx(self, core):
        """[128, total_idx//16] int16, wrapped-16 and replicated to 8 groups."""
        v = self.idx16[core].reshape(-1, 16).T  # [16, total/16]
        return np.tile(v, (8, 1)).copy()

    def s_blob(self, core):
        """[128, total_slots*128] fp8: per slot S[e,d] = (dstloc[e]==d).

        Slot order: tile-major (tile t: its chunk-0 slots then chunk-1 slots),
        matching the matmul loop.  Column range of tile t: s_off[t]*128.
        """
        nch = self.n_chunks
        out = np.zeros((P, self.total_slots * P), dtype=NP_F8)
        iota = np.arange(P, dtype=np.int16)
        for t in range(self.n_tiles):
            si = self.s_off[t]
            for c in range(nch):
                nsl = self.caps[t, c] // P
                if nsl == 0:
                    continue
                g = t // GROUP
                base = self.call_off[c, g] + self.buck_col[c, t] * P
                d = self.dstloc[core, base : base + nsl * P].reshape(nsl, P)
                # S [slot, e, d]
                s = (d[:, :, None] == iota[None, None, :]).astype(NP_F8)
                # [P(e), nsl, P(d)] -> columns
                out[:, si * P : (si + nsl) * P] = (
                    s.transpose(1, 0, 2).reshape(P, nsl * P)
                )
                si += nsl
        return out


def _build_program(pm, s_pad, st_pad, tn, tt):
    """pm: main-pass EdgePlan (NT->NT)."""
    nt_pad = NCORES * s_pad
    nc = bacc.Bacc(
        None, target_bir_lowering=False, num_devices=NCORES, num_swdge_queues=NQ
    )

    def param(name, shape, dt, out=False):
        return nc.declare_dram_parameter(name, list(shape), dt, isOutput=out)

    tbl_init = param("tbl_init", (nt_pad, TPAD), F16)
    idx_nt = param("idx_nt", (P, pm.total_idx // 16), I16)
    s_nt = param("s_nt", (P, pm.total_slots * P), F8)
    xnt = param("xnt", (512, s_pad), F16)  # pre-transposed on host
    xt = param("xt", (512, st_pad), F16)
    w1t = param("w1t", (512, 256), F16)
    b1 = param("b1", (256, 1), F32)
    w2t = param("w2t", (256, 40), F16)
    b2b = param("b2b", (P, 40), F32)
    alpha_nt = param("alpha_nt", (s_pad, 1), F32)
    alpha_t = param("alpha_t", (st_pad, 1), F32)
    dis_nt = param("dis_nt", (s_pad, 1), F32)
    dissq_nt = param("dissq_nt", (s_pad, 1), F32)
    own_init = param("own_init", (s_pad, 40), F16)
    c1p = param("c1p", (s_pad, 40), F32)
    c2p = param("c2p", (s_pad, 40), F32)
    hard_t = param("hard_t", (st_pad, 40), F32)
    out_nt = param("out_nt", (s_pad, 40), F32, out=True)
    out_t = param("out_t", (st_pad, 40), F32, out=True)

    if EXCHANGE == "pad":
        cown = nc.dram_tensor("cown", [s_pad, TPAD], F16)
        callg = nc.dram_tensor("callg", [nt_pad, TPAD], F16, addr_space="Shared")
        table = None
    else:
        cown = nc.dram_tensor("cown", [s_pad, 40], F16)
        callg = nc.dram_tensor("callg", [nt_pad, 40], F16, addr_space="Shared")
        table = nc.dram_tensor("table", [nt_pad, TPAD], F16)

    RG = [list(range(NCORES))]

    with tile.TileContext(nc) as tc:
        with (
            tc.tile_pool(name="persist", bufs=1) as pp,
            tc.tile_pool(name="work", bufs=4) as wp,
            tc.tile_pool(name="gpool", bufs=4) as gp,
            tc.tile_pool(name="spool", bufs=3) as sp,
            tc.tile_pool(name="mpsum", bufs=2, space="PSUM") as mp,
            tc.tile_pool(name="apsum", bufs=4, space="PSUM") as ap_,
        ):
            if table is not None:
                nc.sync.dma_start(out=table[:, :], in_=tbl_init[:, :])

            # ---- persistent SBUF ----
            idxm_sb = pp.tile([P, pm.total_idx // 16], I16, tag="idxm")
            nc.sync.dma_start(out=idxm_sb[:], in_=idx_nt[:, :])

            ft_nt = pp.tile([P, tn, 40], F32, tag="ftnt")
            ft_t = pp.tile([P, tt, 40], F32, tag="ftt")
            cwid = TPAD if EXCHANGE == "pad" else 40
            compact = pp.tile([P, tn, cwid], F16, tag="compact")
            if EXCHANGE == "pad":
                nc.vector.memset(compact[:], 0.0)
            nc.sync.dma_start(
                out=compact[:, :, 0:40],
                in_=own_init.ap().rearrange("(t p) c -> p t c", p=P),
            )

            w1_sb = pp.tile([P, 4, 256], F16, tag="w1")
            nc.sync.dma_start(
                out=w1_sb[:], in_=w1t.ap().rearrange("(k p) h -> p k h", p=P)
            )
            w2_sb = pp.tile([P, 2, 40], F16, tag="w2")
            nc.sync.dma_start(
                out=w2_sb[:], in_=w2t.ap().rearrange("(h p) c -> p h c", p=P)
            )
            b1_sb = pp.tile([P, 2], F32, tag="b1")
            nc.sync.dma_start(
                out=b1_sb[:], in_=b1.ap().rearrange("(h p) o -> p (h o)", p=P)
            )
            b2_sb = pp.tile([P, 40], F32, tag="b2")
            nc.sync.dma_start(out=b2_sb[:], in_=b2b[:, :])

            c1 = pp.tile([P, tn, 40], F32, tag="c1")
            nc.sync.dma_start(
                out=c1[:], in_=c1p.ap().rearrange("(t p) c -> p t c", p=P)
            )
            c2 = pp.tile([P, tn, 40], F32, tag="c2")
            nc.sync.dma_start(
                out=c2[:], in_=c2p.ap().rearrange("(t p) c -> p t c", p=P)
            )

            def cols_load(prm, n_tiles, tag):
                t_ = pp.tile([P, n_tiles], F32, tag=tag)
                nc.sync.dma_start(
                    out=t_[:], in_=prm.ap().rearrange("(t p) o -> p (t o)", p=P)
                )
                return t_

            disn_sb = cols_load(dis_nt, tn, "disn")
            dsqn_sb = cols_load(dissq_nt, tn, "dsqn")
            aln_sb = cols_load(alpha_nt, tn, "aln")
            alt_sb = cols_load(alpha_t, tt, "alt")

            # sigmoid(alpha); a*dis; 1-a
            sign_sb = pp.tile([P, tn], F32, tag="sign")
            nc.scalar.activation(
                sign_sb[:], aln_sb[:], mybir.ActivationFunctionType.Sigmoid
            )
            sigt_sb = pp.tile([P, tt], F32, tag="sigt")
            nc.scalar.activation(
                sigt_sb[:], alt_sb[:], mybir.ActivationFunctionType.Sigmoid
            )
            disa_sb = pp.tile([P, tn], F32, tag="disa")
            nc.vector.tensor_tensor(
                out=disa_sb[:], in0=sign_sb[:], in1=disn_sb[:],
                op=mybir.AluOpType.mult,
            )
            oman_sb = pp.tile([P, tn], F32, tag="oman")
            nc.vector.tensor_scalar(
                out=oman_sb[:], in0=sign_sb[:], scalar1=-1.0, scalar2=1.0,
                op0=mybir.AluOpType.mult, op1=mybir.AluOpType.add,
            )
            omat_sb = pp.tile([P, tt], F32, tag="omat")
            nc.vector.tensor_scalar(
                out=omat_sb[:], in0=sigt_sb[:], scalar1=-1.0, scalar2=1.0,
                op0=mybir.AluOpType.mult, op1=mybir.AluOpType.add,
            )

            # ---- MLP (FT branch) ----
            def mlp(xsrc, n_tiles, ft_dst):
                for n in range(n_tiles):
                    xTs = []
                    for k in range(4):
                        xT = wp.tile([P, P], F16, tag="xT")
                        nc.sync.dma_start(
                            out=xT[:],
                            in_=xsrc[k * P : (k + 1) * P, n * P : (n + 1) * P],
                        )
                        xTs.append(xT)
                    ps2 = mp.tile([P, 40], F32, tag="ps2")
                    for h in range(2):
                        ps1 = mp.tile([P, P], F32, tag="ps1")
                        for k in range(4):
                            nc.tensor.matmul(
                                ps1[:],
                                lhsT=w1_sb[:, k, h * P : (h + 1) * P],
                                rhs=xTs[k][:],
                                start=(k == 0),
                                stop=(k == 3),
                            )
                        hT = wp.tile([P, P], F16, tag="hT")
                        nc.scalar.activation(
                            hT[:], ps1[:], mybir.ActivationFunctionType.Relu,
                            bias=b1_sb[:, h : h + 1],
                        )
                        nc.tensor.matmul(
                            ps2[:], lhsT=hT[:], rhs=w2_sb[:, h, :],
                            start=(h == 0), stop=(h == 1),
                        )
                    nc.vector.tensor_tensor(
                        out=ft_dst[:, n, :], in0=ps2[:], in1=b2_sb[:],
                        op=mybir.AluOpType.add,
                    )

            mlp(xnt, tn, ft_nt)
            mlp(xt, tt, ft_t)

            # ---- generic SpMM pass ----
            _regs = {}

            def num_reg(v):
                if v not in _regs:
                    _regs[v] = nc.gpsimd.to_reg(v)
                return _regs[v]

            _q = [0]

            def spmm_pass(plan, tsrc, idx_sb, s_param, evac):
                """tsrc: DRAM table. evac(t, psum_ap) -> emits eviction."""
                nch = plan.n_chunks
                for g in range(plan.n_groups):
                    gbufs = []
                    for c in range(nch):
                        num = int(plan.call_num[c, g])
                        if num == 0:
                            gbufs.append(None)
                            continue
                        gb = gp.tile([P, num // P, TPAD], F16, tag="gb")
                        r0 = c * plan.chunk
                        nrow = plan.chunk
                        if os.environ.get("KERNEL_NO_GATHER", "0") == "1":
                            # debug: sequential read instead of gather
                            nc.sync.dma_start(
                                out=gb[:],
                                in_=tsrc[r0 : r0 + num, :].rearrange(
                                    "(n p) e -> p n e", p=P
                                ),
                            )
                        else:
                            for off, n_, col0 in plan.subcalls[(c, g)]:
                                nc.gpsimd.dma_gather(
                                    out_ap=gb[:, col0 : col0 + n_ // P, :],
                                    in_ap=tsrc[r0 : r0 + nrow, :],
                                    idxs_ap=idx_sb[:, off // 16 : (off + n_) // 16],
                                    num_idxs=n_,
                                    num_idxs_reg=num_reg(n_),
                                    elem_size=TPAD,
                                    queue_num=_q[0] % NQ,
                                )
                                _q[0] += 1
                        gbufs.append(gb)
                    for t in range(g * GROUP, min((g + 1) * GROUP, plan.n_tiles)):
                        tot = int(plan.slots_per_tile[t])
                        if tot == 0:
                            continue
                        si = int(plan.s_off[t])
                        st_ = sp.tile([P, tot * P], F8, tag="sstr")
                        nc.sync.dma_start(
                            out=st_[:], in_=s_param[:, si * P : (si + tot) * P]
                        )
                        ps = ap_.tile([P, 40], F32, tag="acc")
                        k = 0
                        for c in range(nch):
                            nsl = int(plan.caps[t, c]) // P
                            bc = int(plan.buck_col[c, t])
                            for j in range(nsl):
                                nc.tensor.matmul(
                                    ps[:],
                                    lhsT=st_[:, k * P : (k + 1) * P],
                                    rhs=gbufs[c][:, bc + j, 0:40],
                                    start=(k == 0),
                                    stop=(k == tot - 1),
                                )
                                k += 1
                        evac(t, ps)

            # ---- 10 PLP iterations ----
            for it in range(10):
                if EXCHANGE == "pad":
                    tsrc = tbl_init if it == 0 else callg
                else:
                    tsrc = table
                cbuf = c1 if it == 0 else c2

                if it < 9:
                    def evac_iter(t, ps, cbuf=cbuf):
                        tmp0 = wp.tile([P, 40], F32, tag="ev0")
                        nc.vector.tensor_tensor(
                            out=tmp0[:], in0=ps[:], in1=compact[:, t, 0:40],
                            op=mybir.AluOpType.add,
                        )
                        tmp = wp.tile([P, 40], F32, tag="ev")
                        nc.vector.tensor_tensor(
                            out=tmp[:], in0=tmp0[:], in1=cbuf[:, t, :],
                            op=mybir.AluOpType.add,
                        )
                        nc.vector.tensor_scalar(
                            out=compact[:, t, 0:40], in0=tmp[:],
                            scalar1=dsqn_sb[:, t : t + 1], scalar2=None,
                            op0=mybir.AluOpType.mult,
                        )
                else:
                    def evac_iter(t, ps, cbuf=cbuf):
                        tmp0 = wp.tile([P, 40], F32, tag="ev0")
                        nc.vector.tensor_tensor(
                            out=tmp0[:], in0=ps[:], in1=compact[:, t, 0:40],
                            op=mybir.AluOpType.add,
                        )
                        tmp = wp.tile([P, 40], F32, tag="ev")
                        nc.vector.tensor_tensor(
                            out=tmp[:], in0=tmp0[:], in1=cbuf[:, t, :],
                            op=mybir.AluOpType.add,
                        )
                        t2 = wp.tile([P, 40], F32, tag="ev2")
                        nc.vector.tensor_scalar(
                            out=t2[:], in0=tmp[:],
                            scalar1=disa_sb[:, t : t + 1], scalar2=None,
                            op0=mybir.AluOpType.mult,
                        )
                        t3 = wp.tile([P, 40], F32, tag="ev3")
                        nc.vector.tensor_scalar(
                            out=t3[:], in0=ft_nt[:, t, :],
                            scalar1=oman_sb[:, t : t + 1], scalar2=None,
                            op0=mybir.AluOpType.mult,
                        )
                        t4 = wp.tile([P, 40], F32, tag="ev4")
                        nc.vector.tensor_tensor(
                            out=t4[:], in0=t2[:], in1=t3[:],
                            op=mybir.AluOpType.add,
                        )
                        nc.sync.dma_start(
                            out=out_nt[t * P : (t + 1) * P, :], in_=t4[:]
                        )

                spmm_pass(pm, tsrc, idxm_sb, s_nt, evac_iter)

                if it < 9:
                    nc.sync.dma_start(
                        out=cown.ap().rearrange("(t p) c -> p t c", p=P),
                        in_=compact[:],
                    )
                    if os.environ.get("KERNEL_NO_CC", "0") == "1":
                        # debug mode: skip the collective (wrong cross-core data)
                        nc.sync.dma_start(
                            out=callg[0 : s_pad, :], in_=cown[:, :]
                        )
                    else:
                        nc.gpsimd.collective_compute(
                            "AllGather",
                            mybir.AluOpType.bypass,
                            replica_groups=RG,
                            ins=[cown.ap().opt()],
                            outs=[callg.ap().opt()],
                        )
                    if EXCHANGE != "pad":
                        nc.sync.dma_start(out=table[:, 0:40], in_=callg[:, :])

            # ---- T-side final combine ----
            for t in range(tt):
                hsb = wp.tile([P, 40], F32, tag="hsb")
                nc.sync.dma_start(out=hsb[:], in_=hard_t[t * P : (t + 1) * P, :])
                t1_ = wp.tile([P, 40], F32, tag="tc1")
                nc.vector.tensor_scalar(
                    out=t1_[:], in0=hsb[:], scalar1=sigt_sb[:, t : t + 1],
                    scalar2=None, op0=mybir.AluOpType.mult,
                )
                t2_ = wp.tile([P, 40], F32, tag="tc2")
                nc.vector.tensor_scalar(
                    out=t2_[:], in0=ft_t[:, t, :], scalar1=omat_sb[:, t : t + 1],
                    scalar2=None, op0=mybir.AluOpType.mult,
                )
                t3_ = wp.tile([P, 40], F32, tag="tc3")
                nc.vector.tensor_tensor(
                    out=t3_[:], in0=t1_[:], in1=t2_[:],
                    op=mybir.AluOpType.add,
                )
                nc.sync.dma_start(out=out_t[t * P : (t + 1) * P, :], in_=t3_[:])

    nc.compile()
    return nc


def kernel(**inputs):
    x = np.asarray(inputs["x"], dtype=np.float32)
    edge_index = np.asarray(inputs["edge_index"])
    label_init = np.asarray(inputs["label_init"], dtype=np.float32)
    train_mask = np.asarray(inputs["train_mask"]).astype(bool)
    hard = np.asarray(inputs["hard_one_hot"], dtype=np.float32)
    fc1_w = np.asarray(inputs["fc1_w"], dtype=np.float32)
    fc1_b = np.asarray(inputs["fc1_b"], dtype=np.float32)
    fc2_w = np.asarray(inputs["fc2_w"], dtype=np.float32)
    fc2_b = np.asarray(inputs["fc2_b"], dtype=np.float32)
    alpha = np.asarray(inputs["alpha"], dtype=np.float32)

    n = x.shape[0]
    row = edge_index[0].astype(np.int64)
    col = edge_index[1].astype(np.int64)

    deg = np.bincount(row, minlength=n).astype(np.float64) + 1.0
    dis = (1.0 / np.sqrt(deg)).astype(np.float32)

    nt_ids = np.nonzero(~train_mask)[0]
    t_ids = np.nonzero(train_mask)[0]
    n_nt, n_t = len(nt_ids), len(t_ids)

    s_real = _ceil(n_nt, NCORES)
    tn = _ceil(s_real, P)
    s_pad = tn * P
    nt_pad = NCORES * s_pad
    st_real = _ceil(n_t, NCORES)
    tt = _ceil(st_real, P)
    st_pad = tt * P

    # padded NT id / compact T id for each original node
    pid = np.full(n, -1, dtype=np.int64)
    j = np.arange(n_nt)
    stripe = j // s_real
    pid[nt_ids] = stripe * s_pad + (j - stripe * s_real)

    # edges into NT dsts
    sel = ~train_mask[col]
    es, ed = row[sel], col[sel]
    src_nt = ~train_mask[es]
    # main: NT->NT (self-loops handled in the evac via the compact tile)
    m_src = pid[es[src_nt]]
    m_dst = pid[ed[src_nt]]
    pm = EdgePlan(m_src, m_dst, nt_pad, s_pad, tn)

    # ---- host-precomputed T-source constants c1/c2 ----
    scaled_li = dis[:, None] * label_init  # [n, 40]
    scaled_hd = dis[:, None] * hard
    es_t, ed_t = es[~src_nt], ed[~src_nt]
    d_pid_t = pid[ed_t]
    order_t = np.argsort(d_pid_t, kind="stable")
    d_sorted = d_pid_t[order_t]
    uniq_d, start_i = np.unique(d_sorted, return_index=True)
    c1_full = np.zeros((nt_pad, 40), dtype=np.float32)
    c2_full = np.zeros((nt_pad, 40), dtype=np.float32)
    if len(uniq_d):
        c1_full[uniq_d] = np.add.reduceat(
            scaled_li[es_t[order_t]].astype(np.float64), start_i, axis=0
        )
        c2_full[uniq_d] = np.add.reduceat(
            scaled_hd[es_t[order_t]].astype(np.float64), start_i, axis=0
        )

    # ---- tables ----
    tbl_init_g = np.zeros((nt_pad, TPAD), dtype=np.float16)
    tbl_init_g[pid[nt_ids], :40] = scaled_li[nt_ids].astype(np.float16)
    own_init_g = tbl_init_g[:, :40].reshape(NCORES, s_pad, 40).copy()

    # ---- per-core MLP / combine inputs ----
    def stripe_rows(ids, srl, spad_, nstripes=NCORES):
        """Return [nstripes, spad_] original-id per padded slot (-1 pad)."""
        m = np.full((nstripes, spad_), -1, dtype=np.int64)
        for i in range(nstripes):
            lo = i * srl
            hi = min(len(ids), (i + 1) * srl)
            if hi > lo:
                m[i, : hi - lo] = ids[lo:hi]
        return m

    nt_map = stripe_rows(nt_ids, s_real, s_pad)
    t_map = stripe_rows(t_ids, st_real, st_pad)

    def take(arr, idmap, fill=0.0):
        out = np.full((idmap.shape[0], idmap.shape[1]) + arr.shape[1:], fill,
                      dtype=arr.dtype)
        valid = idmap >= 0
        out[valid] = arr[idmap[valid]]
        return out

    xnt_g = np.ascontiguousarray(
        take(x, nt_map).astype(np.float16).transpose(0, 2, 1)
    )
    xt_g = np.ascontiguousarray(take(x, t_map).astype(np.float16).transpose(0, 2, 1))
    al_nt_g = take(alpha, nt_map).astype(np.float32)
    al_t_g = take(alpha, t_map).astype(np.float32)
    dis_nt_g = take(dis[:, None], nt_map).astype(np.float32)
    dsq_nt_g = take((dis * dis)[:, None], nt_map).astype(np.float32)
    hard_t_g = take(hard, t_map).astype(np.float32)

    w1t_g = fc1_w.T.astype(np.float16).copy()  # [512, 256]
    b1_g = fc1_b.reshape(256, 1).astype(np.float32)
    w2t_g = fc2_w.T.astype(np.float16).copy()  # [256, 40]
    b2b_g = np.tile(fc2_b.reshape(1, 40), (P, 1)).astype(np.float32)

    nc = _build_program(pm, s_pad, st_pad, tn, tt)

    if os.environ.get("KERNEL_BUILD_ONLY", "0") == "1":
        e = BuildOnly()
        e.nc = nc
        raise e

    in_maps = []
    for i in range(NCORES):
        in_maps.append(
            dict(
                tbl_init=tbl_init_g,
                idx_nt=pm.wrapped_idx(i),
                s_nt=pm.s_blob(i),
                xnt=xnt_g[i],
                xt=xt_g[i],
                w1t=w1t_g,
                b1=b1_g,
                w2t=w2t_g,
                b2b=b2b_g,
                alpha_nt=al_nt_g[i],
                alpha_t=al_t_g[i],
                dis_nt=dis_nt_g[i],
                dissq_nt=dsq_nt_g[i],
                own_init=own_init_g[i],
                c1p=c1_full[i * s_pad : (i + 1) * s_pad],
                c2p=c2_full[i * s_pad : (i + 1) * s_pad],
                hard_t=hard_t_g[i],
            )
        )

    if os.environ.get("KERNEL_SIM", "0") == "1":
        from concourse import bass_interp

        sim = bass_interp.MultiCoreSim(nc, NCORES)
        for i in range(NCORES):
            for k, v in in_maps[i].items():
                sim.cores[i].tensor(k)[:] = v
        sim.simulate()
        results = [
            {k: np.array(sim.cores[i].mem_tensor(k)) for k in ("out_nt", "out_t")}
            for i in range(NCORES)
        ]
        res = None
    else:
        res = run_bass_kernel_spmd(
            nc, in_maps, core_ids=list(range(NCORES)),
            trace=bool(int(os.environ.get("KERNEL_TRACE", "0"))),
        )
        results = res.results
    kernel.last_results = res
    kernel.last_nc = nc
    kernel.last_in_maps = in_maps

    out = np.zeros((n, 40), dtype=np.float32)
    for i in range(NCORES):
        om = results[i]["out_nt"]
        ot = results[i]["out_t"]
        v = nt_map[i] >= 0
        out[nt_map[i][v]] = om[v]
        v = t_map[i] >= 0
        out[t_map[i][v]] = ot[v]
    return out


# revision 7
# speedup vs baseline: 1.2269x; 1.0012x over previous
"""CPFStudent (GNN label propagation + MLP mix) on 8 TRN2 NeuronCores.

Strategy (dst-sharded SpMM with selector matmuls), v2:
  - Reference: 10 PLP steps of plp <- where(mask, hard, A_hat @ plp), with
    A_hat = D^-1/2 (A+I) D^-1/2 built from out-degrees of edge_index[0];
    final logits = sigmoid(alpha)*plp + (1-sigmoid(alpha))*relu(x@W1^T+b1)@W2^T+b2.
  - Only non-train (NT) rows of plp evolve; train (T) rows are constant after
    step 1.  State kept as table = dis * plp (dis = deg^-1/2), fp16:
        plp_new[d] = dis[d] * ( sum_{e: src NT} table[src] + c )
    where c is a per-dst constant: c1 (dis*label_init over T srcs, step 1) or
    c2 (dis*hard over T srcs, steps 2..10).  c1/c2 are precomputed HOST-side
    (they are iteration-invariant) — no T-side SpMM passes on device.
  - Nodes permuted host-side: NT nodes first, padded per-core stripes.  Each
    core owns a contiguous stripe of NT dst rows; edges bucketed by (dst tile
    of 128, src chunk of <=32768) host-side, padded to uniform capacities
    across cores (SPMD), gathered per iteration with gpsimd.dma_gather (256B
    elements) from an HBM fp16 table, spread round-robin over 4 SWDGE queues.
  - Scatter/segment-sum on the TensorEngine: per 128-edge slot a
    host-precomputed fp8 selector S (S[e,d]=1 iff dst_local(e)==d) multiplies
    the gathered messages, accumulating in PSUM per dst tile.
  - Halo exchange: AllGather of each core's full-width (padded 256B) rows
    directly produces the next iteration's gather table — no post-collective
    re-strided table write.
"""

import math
import os
import sys

import numpy as np

sys.path.insert(0, "/opt/trn_rl_repo")

import ml_dtypes  # noqa: E402

import concourse.bass as bass  # noqa: E402
import concourse.mybir as mybir  # noqa: E402
import concourse.tile as tile  # noqa: E402
from concourse import bacc  # noqa: E402
from concourse.bass_utils import run_bass_kernel_spmd  # noqa: E402

P = 128
NCORES = 8
TPAD = 128  # fp16 elements per table row (256B, dma_gather elem granularity)
GROUP = 7  # dst tiles per dma_gather call group
MAX_CALL = int(os.environ.get("KERNEL_MAX_CALL", "1024"))
NQ = int(os.environ.get("KERNEL_NQ", "1"))
EXCHANGE = os.environ.get("KERNEL_EXCHANGE", "pad")

F16 = mybir.dt.float16
F32 = mybir.dt.float32
F8 = mybir.dt.float8e4
I16 = mybir.dt.int16
NP_F8 = ml_dtypes.float8_e4m3


def _ceil(a, b):
    return -(-a // b)


class BuildOnly(Exception):
    pass


class EdgePlan:
    """Host-side bucketed edge plan for one SpMM pass, uniform across cores.

    src_row: int array, row index into the pass's gather table
    dst_pid: int array, padded NT id of the destination
    """

    def __init__(self, src_row, dst_pid, n_rows, s_pad, n_tiles):
        self.n_chunks = max(1, _ceil(n_rows, 32768))
        self.chunk = _ceil(n_rows, self.n_chunks)
        self.n_tiles = n_tiles
        nch = self.n_chunks

        core = dst_pid // s_pad
        dloc = dst_pid - core * s_pad
        tl = dloc // P
        dstloc = dloc % P
        ch = src_row // self.chunk

        key = (core * n_tiles + tl) * nch + ch
        counts = np.bincount(key, minlength=NCORES * n_tiles * nch).reshape(
            NCORES, n_tiles, nch
        )
        caps = counts.max(axis=0)  # [n_tiles, nch]
        caps = ((caps + P - 1) // P) * P
        self.caps = caps
        self.slots_per_tile = caps.sum(axis=1) // P  # [n_tiles]
        self.s_off = np.concatenate([[0], np.cumsum(self.slots_per_tile)])
        self.total_slots = int(self.s_off[-1])

        # per (chunk, group) call: num idxs and per-tile column offsets
        self.n_groups = _ceil(n_tiles, GROUP)
        self.call_num = np.zeros((nch, self.n_groups), dtype=np.int64)
        self.buck_col = np.zeros((nch, n_tiles), dtype=np.int64)  # col in its call buf
        for c in range(nch):
            for g in range(self.n_groups):
                off = 0
                for t in range(g * GROUP, min((g + 1) * GROUP, n_tiles)):
                    self.buck_col[c, t] = off
                    off += caps[t, c] // P
                self.call_num[c, g] = off * P
        # col offset of each call inside the flat idx stream (per chunk then group)
        self.call_off = np.zeros((nch, self.n_groups), dtype=np.int64)
        off = 0
        for c in range(nch):
            for g in range(self.n_groups):
                self.call_off[c, g] = off
                off += self.call_num[c, g]
        self.total_idx = off

        # sub-calls of <= MAX_CALL idxs: per (c, g) a list of (idx_off, num, col0)
        self.subcalls = {}
        for c in range(nch):
            for g in range(self.n_groups):
                num = int(self.call_num[c, g])
                base = int(self.call_off[c, g])
                subs = []
                p0 = 0
                while p0 < num:
                    n_ = min(MAX_CALL, num - p0)
                    subs.append((base + p0, n_, p0 // P))
                    p0 += n_
                self.subcalls[(c, g)] = subs

        # order edges by (core, chunk, tile); build padded per-core streams
        order = np.argsort((core * nch + ch) * n_tiles + tl, kind="stable")
        src_o = src_row[order]
        core_o = core[order]
        ch_o = ch[order]
        tl_o = tl[order]
        dst_o = dstloc[order]

        # destination position of each edge in the padded stream
        # padded stream order: for chunk c, group g, tile t in g: cap[t,c] entries
        base_tc = np.zeros((nch, n_tiles), dtype=np.int64)
        for c in range(nch):
            for g in range(self.n_groups):
                for t in range(g * GROUP, min((g + 1) * GROUP, n_tiles)):
                    base_tc[c, t] = self.call_off[c, g] + self.buck_col[c, t] * P

        self.idx16 = np.zeros((NCORES, self.total_idx), dtype=np.int16)
        self.dstloc = np.full((NCORES, self.total_idx), -1, dtype=np.int16)
        # rank of each edge within its (core, chunk, tile) bucket
        grp_key = (core_o * nch + ch_o) * n_tiles + tl_o
        # stable sort keeps original order; compute rank via cumcount
        uniq, inv, cnt = np.unique(grp_key, return_inverse=True, return_counts=True)
        starts = np.concatenate([[0], np.cumsum(cnt)])[:-1]
        rank = np.arange(len(grp_key)) - starts[inv]
        pos = base_tc[ch_o, tl_o] + rank
        self.idx16[core_o, pos] = (src_o - ch_o * self.chunk).astype(np.int16)
        self.dstloc[core_o, pos] = dst_o.astype(np.int16)

    def wrapped_idx(self, core):
        """[128, total_idx//16] int16, wrapped-16 and replicated to 8 groups."""
        v = self.idx16[core].reshape(-1, 16).T  # [16, total/16]
        return np.tile(v, (8, 1)).copy()

    def s_blob(self, core):
        """[128, total_slots*128] fp8: per slot S[e,d] = (dstloc[e]==d).

        Slot order: tile-major (tile t: its chunk-0 slots then chunk-1 slots),
        matching the matmul loop.  Column range of tile t: s_off[t]*128.
        """
        nch = self.n_chunks
        out = np.zeros((P, self.total_slots * P), dtype=NP_F8)
        iota = np.arange(P, dtype=np.int16)
        for t in range(self.n_tiles):
            si = self.s_off[t]
            for c in range(nch):
                nsl = self.caps[t, c] // P
                if nsl == 0:
                    continue
                g = t // GROUP
                base = self.call_off[c, g] + self.buck_col[c, t] * P
                d = self.dstloc[core, base : base + nsl * P].reshape(nsl, P)
                # S [slot, e, d]
                s = (d[:, :, None] == iota[None, None, :]).astype(NP_F8)
                # [P(e), nsl, P(d)] -> columns
                out[:, si * P : (si + nsl) * P] = (
                    s.transpose(1, 0, 2).reshape(P, nsl * P)
                )
                si += nsl
        return out


def _build_program(pm, s_pad, st_pad, tn, tt):
    """pm: main-pass EdgePlan (NT->NT)."""
    nt_pad = NCORES * s_pad
    nc = bacc.Bacc(
        None, target_bir_lowering=False, num_devices=NCORES, num_swdge_queues=NQ
    )

    def param(name, shape, dt, out=False):
        return nc.declare_dram_parameter(name, list(shape), dt, isOutput=out)

    tbl_init = param("tbl_init", (nt_pad, TPAD), F16)
    idx_nt = param("idx_nt", (P, pm.total_idx // 16), I16)
    s_nt = param("s_nt", (P, pm.total_slots * P), F8)
    xnt = param("xnt", (512, s_pad), F16)  # pre-transposed on host
    xt = param("xt", (512, st_pad), F16)
    w1t = param("w1t", (512, 256), F16)
    b1 = param("b1", (256, 1), F32)
    w2t = param("w2t", (256, 40), F16)
    b2b = param("b2b", (P, 40), F32)
    alpha_nt = param("alpha_nt", (s_pad, 1), F32)
    alpha_t = param("alpha_t", (st_pad, 1), F32)
    dis_nt = param("dis_nt", (s_pad, 1), F32)
    dissq_nt = param("dissq_nt", (s_pad, 1), F32)
    own_init = param("own_init", (s_pad, 40), F16)
    c1p = param("c1p", (s_pad, 40), F32)
    c2p = param("c2p", (s_pad, 40), F32)
    hard_t = param("hard_t", (st_pad, 40), F32)
    out_nt = param("out_nt", (s_pad, 40), F32, out=True)
    out_t = param("out_t", (st_pad, 40), F32, out=True)

    if EXCHANGE == "pad":
        cown = nc.dram_tensor("cown", [s_pad, TPAD], F16)
        callg = nc.dram_tensor("callg", [nt_pad, TPAD], F16, addr_space="Shared")
        table = None
    else:
        cown = nc.dram_tensor("cown", [s_pad, 40], F16)
        callg = nc.dram_tensor("callg", [nt_pad, 40], F16, addr_space="Shared")
        table = nc.dram_tensor("table", [nt_pad, TPAD], F16)

    RG = [list(range(NCORES))]

    with tile.TileContext(nc) as tc:
        with (
            tc.tile_pool(name="persist", bufs=1) as pp,
            tc.tile_pool(name="work", bufs=4) as wp,
            tc.tile_pool(name="gpool", bufs=4) as gp,
            tc.tile_pool(name="spool", bufs=3) as sp,
            tc.tile_pool(name="mpsum", bufs=2, space="PSUM") as mp,
            tc.tile_pool(name="apsum", bufs=4, space="PSUM") as ap_,
        ):
            if table is not None:
                nc.sync.dma_start(out=table[:, :], in_=tbl_init[:, :])

            # ---- persistent SBUF ----
            idxm_sb = pp.tile([P, pm.total_idx // 16], I16, tag="idxm")
            nc.sync.dma_start(out=idxm_sb[:], in_=idx_nt[:, :])

            ft_nt = pp.tile([P, tn, 40], F32, tag="ftnt")
            ft_t = pp.tile([P, tt, 40], F32, tag="ftt")
            cwid = TPAD if EXCHANGE == "pad" else 40
            compact = pp.tile([P, tn, cwid], F16, tag="compact")
            if EXCHANGE == "pad":
                nc.vector.memset(compact[:], 0.0)
            nc.sync.dma_start(
                out=compact[:, :, 0:40],
                in_=own_init.ap().rearrange("(t p) c -> p t c", p=P),
            )

            w1_sb = pp.tile([P, 4, 256], F16, tag="w1")
            nc.sync.dma_start(
                out=w1_sb[:], in_=w1t.ap().rearrange("(k p) h -> p k h", p=P)
            )
            w2_sb = pp.tile([P, 2, 40], F16, tag="w2")
            nc.sync.dma_start(
                out=w2_sb[:], in_=w2t.ap().rearrange("(h p) c -> p h c", p=P)
            )
            b1_sb = pp.tile([P, 2], F32, tag="b1")
            nc.sync.dma_start(
                out=b1_sb[:], in_=b1.ap().rearrange("(h p) o -> p (h o)", p=P)
            )
            b2_sb = pp.tile([P, 40], F32, tag="b2")
            nc.sync.dma_start(out=b2_sb[:], in_=b2b[:, :])

            c1 = pp.tile([P, tn, 40], F32, tag="c1")
            nc.sync.dma_start(
                out=c1[:], in_=c1p.ap().rearrange("(t p) c -> p t c", p=P)
            )
            c2 = pp.tile([P, tn, 40], F32, tag="c2")
            nc.sync.dma_start(
                out=c2[:], in_=c2p.ap().rearrange("(t p) c -> p t c", p=P)
            )

            def cols_load(prm, n_tiles, tag):
                t_ = pp.tile([P, n_tiles], F32, tag=tag)
                nc.sync.dma_start(
                    out=t_[:], in_=prm.ap().rearrange("(t p) o -> p (t o)", p=P)
                )
                return t_

            disn_sb = cols_load(dis_nt, tn, "disn")
            dsqn_sb = cols_load(dissq_nt, tn, "dsqn")
            aln_sb = cols_load(alpha_nt, tn, "aln")
            alt_sb = cols_load(alpha_t, tt, "alt")

            # sigmoid(alpha); a*dis; 1-a
            sign_sb = pp.tile([P, tn], F32, tag="sign")
            nc.scalar.activation(
                sign_sb[:], aln_sb[:], mybir.ActivationFunctionType.Sigmoid
            )
            sigt_sb = pp.tile([P, tt], F32, tag="sigt")
            nc.scalar.activation(
                sigt_sb[:], alt_sb[:], mybir.ActivationFunctionType.Sigmoid
            )
            disa_sb = pp.tile([P, tn], F32, tag="disa")
            nc.vector.tensor_tensor(
                out=disa_sb[:], in0=sign_sb[:], in1=disn_sb[:],
                op=mybir.AluOpType.mult,
            )
            oman_sb = pp.tile([P, tn], F32, tag="oman")
            nc.vector.tensor_scalar(
                out=oman_sb[:], in0=sign_sb[:], scalar1=-1.0, scalar2=1.0,
                op0=mybir.AluOpType.mult, op1=mybir.AluOpType.add,
            )
            omat_sb = pp.tile([P, tt], F32, tag="omat")
            nc.vector.tensor_scalar(
                out=omat_sb[:], in0=sigt_sb[:], scalar1=-1.0, scalar2=1.0,
                op0=mybir.AluOpType.mult, op1=mybir.AluOpType.add,
            )

            # ---- MLP (FT branch) ----
            def mlp(xsrc, n_tiles, ft_dst):
                for n in range(n_tiles):
                    xTs = []
                    for k in range(4):
                        xT = wp.tile([P, P], F16, tag="xT")
                        nc.sync.dma_start(
                            out=xT[:],
                            in_=xsrc[k * P : (k + 1) * P, n * P : (n + 1) * P],
                        )
                        xTs.append(xT)
                    ps2 = mp.tile([P, 40], F32, tag="ps2")
                    for h in range(2):
                        ps1 = mp.tile([P, P], F32, tag="ps1")
                        for k in range(4):
                            nc.tensor.matmul(
                                ps1[:],
                                lhsT=w1_sb[:, k, h * P : (h + 1) * P],
                                rhs=xTs[k][:],
                                start=(k == 0),
                                stop=(k == 3),
                            )
                        hT = wp.tile([P, P], F16, tag="hT")
                        nc.scalar.activation(
                            hT[:], ps1[:], mybir.ActivationFunctionType.Relu,
                            bias=b1_sb[:, h : h + 1],
                        )
                        nc.tensor.matmul(
                            ps2[:], lhsT=hT[:], rhs=w2_sb[:, h, :],
                            start=(h == 0), stop=(h == 1),
                        )
                    nc.vector.tensor_tensor(
                        out=ft_dst[:, n, :], in0=ps2[:], in1=b2_sb[:],
                        op=mybir.AluOpType.add,
                    )

            mlp(xnt, tn, ft_nt)
            mlp(xt, tt, ft_t)

            # ---- generic SpMM pass ----
            _regs = {}

            def num_reg(v):
                if v not in _regs:
                    _regs[v] = nc.gpsimd.to_reg(v)
                return _regs[v]

            _q = [0]

            def spmm_pass(plan, tsrc, idx_sb, s_param, evac):
                """tsrc: DRAM table. evac(t, psum_ap) -> emits eviction."""
                nch = plan.n_chunks
                for g in range(plan.n_groups):
                    gbufs = []
                    for c in range(nch):
                        num = int(plan.call_num[c, g])
                        if num == 0:
                            gbufs.append(None)
                            continue
                        gb = gp.tile([P, num // P, TPAD], F16, tag="gb")
                        r0 = c * plan.chunk
                        nrow = plan.chunk
                        if os.environ.get("KERNEL_NO_GATHER", "0") == "1":
                            # debug: sequential read instead of gather
                            nc.sync.dma_start(
                                out=gb[:],
                                in_=tsrc[r0 : r0 + num, :].rearrange(
                                    "(n p) e -> p n e", p=P
                                ),
                            )
                        else:
                            for off, n_, col0 in plan.subcalls[(c, g)]:
                                nc.gpsimd.dma_gather(
                                    out_ap=gb[:, col0 : col0 + n_ // P, :],
                                    in_ap=tsrc[r0 : r0 + nrow, :],
                                    idxs_ap=idx_sb[:, off // 16 : (off + n_) // 16],
                                    num_idxs=n_,
                                    num_idxs_reg=num_reg(n_),
                                    elem_size=TPAD,
                                    queue_num=_q[0] % NQ,
                                )
                                _q[0] += 1
                        gbufs.append(gb)
                    for t in range(g * GROUP, min((g + 1) * GROUP, plan.n_tiles)):
                        tot = int(plan.slots_per_tile[t])
                        if tot == 0:
                            continue
                        si = int(plan.s_off[t])
                        st_ = sp.tile([P, tot * P], F8, tag="sstr")
                        nc.sync.dma_start(
                            out=st_[:], in_=s_param[:, si * P : (si + tot) * P]
                        )
                        ps = ap_.tile([P, 40], F32, tag="acc")
                        k = 0
                        for c in range(nch):
                            nsl = int(plan.caps[t, c]) // P
                            bc = int(plan.buck_col[c, t])
                            for j in range(nsl):
                                nc.tensor.matmul(
                                    ps[:],
                                    lhsT=st_[:, k * P : (k + 1) * P],
                                    rhs=gbufs[c][:, bc + j, 0:40],
                                    start=(k == 0),
                                    stop=(k == tot - 1),
                                )
                                k += 1
                        evac(t, ps)

            # ---- 10 PLP iterations ----
            for it in range(10):
                if EXCHANGE == "pad":
                    tsrc = tbl_init if it == 0 else callg
                else:
                    tsrc = table
                cbuf = c1 if it == 0 else c2

                if it < 9:
                    def evac_iter(t, ps, cbuf=cbuf):
                        tmp0 = wp.tile([P, 40], F32, tag="ev0")
                        nc.vector.tensor_tensor(
                            out=tmp0[:], in0=ps[:], in1=compact[:, t, 0:40],
                            op=mybir.AluOpType.add,
                        )
                        tmp = wp.tile([P, 40], F32, tag="ev")
                        nc.vector.tensor_tensor(
                            out=tmp[:], in0=tmp0[:], in1=cbuf[:, t, :],
                            op=mybir.AluOpType.add,
                        )
                        nc.vector.tensor_scalar(
                            out=compact[:, t, 0:40], in0=tmp[:],
                            scalar1=dsqn_sb[:, t : t + 1], scalar2=None,
                            op0=mybir.AluOpType.mult,
                        )
                else:
                    def evac_iter(t, ps, cbuf=cbuf):
                        tmp0 = wp.tile([P, 40], F32, tag="ev0")
                        nc.vector.tensor_tensor(
                            out=tmp0[:], in0=ps[:], in1=compact[:, t, 0:40],
                            op=mybir.AluOpType.add,
                        )
                        tmp = wp.tile([P, 40], F32, tag="ev")
                        nc.vector.tensor_tensor(
                            out=tmp[:], in0=tmp0[:], in1=cbuf[:, t, :],
                            op=mybir.AluOpType.add,
                        )
                        t2 = wp.tile([P, 40], F32, tag="ev2")
                        nc.vector.tensor_scalar(
                            out=t2[:], in0=tmp[:],
                            scalar1=disa_sb[:, t : t + 1], scalar2=None,
                            op0=mybir.AluOpType.mult,
                        )
                        t3 = wp.tile([P, 40], F32, tag="ev3")
                        nc.vector.tensor_scalar(
                            out=t3[:], in0=ft_nt[:, t, :],
                            scalar1=oman_sb[:, t : t + 1], scalar2=None,
                            op0=mybir.AluOpType.mult,
                        )
                        t4 = wp.tile([P, 40], F32, tag="ev4")
                        nc.vector.tensor_tensor(
                            out=t4[:], in0=t2[:], in1=t3[:],
                            op=mybir.AluOpType.add,
                        )
                        nc.sync.dma_start(
                            out=out_nt[t * P : (t + 1) * P, :], in_=t4[:]
                        )

                spmm_pass(pm, tsrc, idxm_sb, s_nt, evac_iter)

                if it < 9:
                    nc.sync.dma_start(
                        out=cown.ap().rearrange("(t p) c -> p t c", p=P),
                        in_=compact[:],
                    )
                    if os.environ.get("KERNEL_NO_CC", "0") == "1":
                        # debug mode: skip the collective (wrong cross-core data)
                        nc.sync.dma_start(
                            out=callg[0 : s_pad, :], in_=cown[:, :]
                        )
                    else:
                        nc.gpsimd.collective_compute(
                            "AllGather",
                            mybir.AluOpType.bypass,
                            replica_groups=RG,
                            ins=[cown.ap().opt()],
                            outs=[callg.ap().opt()],
                        )
                    if EXCHANGE != "pad":
                        nc.sync.dma_start(out=table[:, 0:40], in_=callg[:, :])

            # ---- T-side final combine ----
            for t in range(tt):
                hsb = wp.tile([P, 40], F32, tag="hsb")
                nc.sync.dma_start(out=hsb[:], in_=hard_t[t * P : (t + 1) * P, :])
                t1_ = wp.tile([P, 40], F32, tag="tc1")
                nc.vector.tensor_scalar(
                    out=t1_[:], in0=hsb[:], scalar1=sigt_sb[:, t : t + 1],
                    scalar2=None, op0=mybir.AluOpType.mult,
                )
                t2_ = wp.tile([P, 40], F32, tag="tc2")
                nc.vector.tensor_scalar(
                    out=t2_[:], in0=ft_t[:, t, :], scalar1=omat_sb[:, t : t + 1],
                    scalar2=None, op0=mybir.AluOpType.mult,
                )
                t3_ = wp.tile([P, 40], F32, tag="tc3")
                nc.vector.tensor_tensor(
                    out=t3_[:], in0=t1_[:], in1=t2_[:],
                    op=mybir.AluOpType.add,
                )
                nc.sync.dma_start(out=out_t[t * P : (t + 1) * P, :], in_=t3_[:])

    nc.compile()
    return nc


def kernel(**inputs):
    x = np.asarray(inputs["x"], dtype=np.float32)
    edge_index = np.asarray(inputs["edge_index"])
    label_init = np.asarray(inputs["label_init"], dtype=np.float32)
    train_mask = np.asarray(inputs["train_mask"]).astype(bool)
    hard = np.asarray(inputs["hard_one_hot"], dtype=np.float32)
    fc1_w = np.asarray(inputs["fc1_w"], dtype=np.float32)
    fc1_b = np.asarray(inputs["fc1_b"], dtype=np.float32)
    fc2_w = np.asarray(inputs["fc2_w"], dtype=np.float32)
    fc2_b = np.asarray(inputs["fc2_b"], dtype=np.float32)
    alpha = np.asarray(inputs["alpha"], dtype=np.float32)

    n = x.shape[0]
    row = edge_index[0].astype(np.int64)
    col = edge_index[1].astype(np.int64)

    deg = np.bincount(row, minlength=n).astype(np.float64) + 1.0
    dis = (1.0 / np.sqrt(deg)).astype(np.float32)

    nt_ids = np.nonzero(~train_mask)[0]
    t_ids = np.nonzero(train_mask)[0]
    n_nt, n_t = len(nt_ids), len(t_ids)

    s_real = _ceil(n_nt, NCORES)
    tn = _ceil(s_real, P)
    s_pad = tn * P
    nt_pad = NCORES * s_pad
    st_real = _ceil(n_t, NCORES)
    tt = _ceil(st_real, P)
    st_pad = tt * P

    # padded NT id / compact T id for each original node
    pid = np.full(n, -1, dtype=np.int64)
    j = np.arange(n_nt)
    stripe = j // s_real
    pid[nt_ids] = stripe * s_pad + (j - stripe * s_real)

    # edges into NT dsts
    sel = ~train_mask[col]
    es, ed = row[sel], col[sel]
    src_nt = ~train_mask[es]
    # main: NT->NT (self-loops handled in the evac via the compact tile)
    m_src = pid[es[src_nt]]
    m_dst = pid[ed[src_nt]]
    pm = EdgePlan(m_src, m_dst, nt_pad, s_pad, tn)

    # ---- host-precomputed T-source constants c1/c2 ----
    scaled_li = dis[:, None] * label_init  # [n, 40]
    scaled_hd = dis[:, None] * hard
    es_t, ed_t = es[~src_nt], ed[~src_nt]
    d_pid_t = pid[ed_t]
    order_t = np.argsort(d_pid_t, kind="stable")
    d_sorted = d_pid_t[order_t]
    uniq_d, start_i = np.unique(d_sorted, return_index=True)
    c1_full = np.zeros((nt_pad, 40), dtype=np.float32)
    c2_full = np.zeros((nt_pad, 40), dtype=np.float32)
    if len(uniq_d):
        c1_full[uniq_d] = np.add.reduceat(
            scaled_li[es_t[order_t]].astype(np.float64), start_i, axis=0
        )
        c2_full[uniq_d] = np.add.reduceat(
            scaled_hd[es_t[order_t]].astype(np.float64), start_i, axis=0
        )

    # ---- tables ----
    tbl_init_g = np.zeros((nt_pad, TPAD), dtype=np.float16)
    tbl_init_g[pid[nt_ids], :40] = scaled_li[nt_ids].astype(np.float16)
    own_init_g = tbl_init_g[:, :40].reshape(NCORES, s_pad, 40).copy()

    # ---- per-core MLP / combine inputs ----
    def stripe_rows(ids, srl, spad_, nstripes=NCORES):
        """Return [nstripes, spad_] original-id per padded slot (-1 pad)."""
        m = np.full((nstripes, spad_), -1, dtype=np.int64)
        for i in range(nstripes):
            lo = i * srl
            hi = min(len(ids), (i + 1) * srl)
            if hi > lo:
                m[i, : hi - lo] = ids[lo:hi]
        return m

    nt_map = stripe_rows(nt_ids, s_real, s_pad)
    t_map = stripe_rows(t_ids, st_real, st_pad)

    def take(arr, idmap, fill=0.0):
        out = np.full((idmap.shape[0], idmap.shape[1]) + arr.shape[1:], fill,
                      dtype=arr.dtype)
        valid = idmap >= 0
        out[valid] = arr[idmap[valid]]
        return out

    xnt_g = np.ascontiguousarray(
        take(x, nt_map).astype(np.float16).transpose(0, 2, 1)
    )
    xt_g = np.ascontiguousarray(take(x, t_map).astype(np.float16).transpose(0, 2, 1))
    al_nt_g = take(alpha, nt_map).astype(np.float32)
    al_t_g = take(alpha, t_map).astype(np.float32)
    dis_nt_g = take(dis[:, None], nt_map).astype(np.float32)
    dsq_nt_g = take((dis * dis)[:, None], nt_map).astype(np.float32)
    hard_t_g = take(hard, t_map).astype(np.float32)

    w1t_g = fc1_w.T.astype(np.float16).copy()  # [512, 256]
    b1_g = fc1_b.reshape(256, 1).astype(np.float32)
    w2t_g = fc2_w.T.astype(np.float16).copy()  # [256, 40]
    b2b_g = np.tile(fc2_b.reshape(1, 40), (P, 1)).astype(np.float32)

    nc = _build_program(pm, s_pad, st_pad, tn, tt)

    if os.environ.get("KERNEL_BUILD_ONLY", "0") == "1":
        e = BuildOnly()
        e.nc = nc
        raise e

    in_maps = []
    for i in range(NCORES):
        in_maps.append(
            dict(
                tbl_init=tbl_init_g,
                idx_nt=pm.wrapped_idx(i),
                s_nt=pm.s_blob(i),
                xnt=xnt_g[i],
                xt=xt_g[i],
                w1t=w1t_g,
                b1=b1_g,
                w2t=w2t_g,
                b2b=b2b_g,
                alpha_nt=al_nt_g[i],
                alpha_t=al_t_g[i],
                dis_nt=dis_nt_g[i],
                dissq_nt=dsq_nt_g[i],
                own_init=own_init_g[i],
                c1p=c1_full[i * s_pad : (i + 1) * s_pad],
                c2p=c2_full[i * s_pad : (i + 1) * s_pad],
                hard_t=hard_t_g[i],
            )
        )

    if os.environ.get("KERNEL_SIM", "0") == "1":
        from concourse import bass_interp

        sim = bass_interp.MultiCoreSim(nc, NCORES)
        for i in range(NCORES):
            for k, v in in_maps[i].items():
                sim.cores[i].tensor(k)[:] = v
        sim.simulate()
        results = [
            {k: np.array(sim.cores[i].mem_tensor(k)) for k in ("out_nt", "out_t")}
            for i in range(NCORES)
        ]
        res = None
    else:
        res = run_bass_kernel_spmd(
            nc, in_maps, core_ids=list(range(NCORES)),
            trace=bool(int(os.environ.get("KERNEL_TRACE", "0"))),
        )
        results = res.results
    kernel.last_results = res
    kernel.last_nc = nc
    kernel.last_in_maps = in_maps

    out = np.zeros((n, 40), dtype=np.float32)
    for i in range(NCORES):
        om = results[i]["out_nt"]
        ot = results[i]["out_t"]
        v = nt_map[i] >= 0
        out[nt_map[i][v]] = om[v]
        v = t_map[i] >= 0
        out[t_map[i][v]] = ot[v]
    return out


# revision 9
# speedup vs baseline: 1.2926x; 1.0535x over previous
"""CPFStudent (GNN label propagation + MLP mix) on 8 TRN2 NeuronCores.

Strategy (dst-sharded SpMM with selector matmuls), v2:
  - Reference: 10 PLP steps of plp <- where(mask, hard, A_hat @ plp), with
    A_hat = D^-1/2 (A+I) D^-1/2 built from out-degrees of edge_index[0];
    final logits = sigmoid(alpha)*plp + (1-sigmoid(alpha))*relu(x@W1^T+b1)@W2^T+b2.
  - Only non-train (NT) rows of plp evolve; train (T) rows are constant after
    step 1.  State kept as table = dis * plp (dis = deg^-1/2), fp16:
        plp_new[d] = dis[d] * ( sum_{e: src NT} table[src] + c )
    where c is a per-dst constant: c1 (dis*label_init over T srcs, step 1) or
    c2 (dis*hard over T srcs, steps 2..10).  c1/c2 are precomputed HOST-side
    (they are iteration-invariant) — no T-side SpMM passes on device.
  - Nodes permuted host-side: NT nodes first, padded per-core stripes.  Each
    core owns a contiguous stripe of NT dst rows; edges bucketed by (dst tile
    of 128, src chunk of <=32768) host-side, padded to uniform capacities
    across cores (SPMD), gathered per iteration with gpsimd.dma_gather (256B
    elements, <=1024 idxs per call) from an HBM fp16 table.
  - Scatter/segment-sum on the TensorEngine: per 128-edge slot a
    host-precomputed fp8 selector S (S[e,d]=1 iff dst_local(e)==d) multiplies
    the gathered messages, accumulating in PSUM per dst tile.
  - Halo exchange: AllGather of each core's full-width (padded 256B) rows
    directly produces the next iteration's gather table — no post-collective
    re-strided table write.
  - Self-loops are folded into the eviction: the previous iteration's own
    `compact` rows are exactly the self-loop messages, so they never hit the
    gather path.
"""

import math
import os
import sys

import numpy as np

sys.path.insert(0, "/opt/trn_rl_repo")

import ml_dtypes  # noqa: E402

import concourse.bass as bass  # noqa: E402
import concourse.mybir as mybir  # noqa: E402
import concourse.tile as tile  # noqa: E402
from concourse import bacc  # noqa: E402
from concourse.bass_utils import run_bass_kernel_spmd  # noqa: E402

P = 128
NCORES = 8
TPAD = 128  # fp16 elements per table row (256B, dma_gather elem granularity)
GROUP = 7  # dst tiles per dma_gather call group
MAX_CALL = int(os.environ.get("KERNEL_MAX_CALL", "1024"))
NQ = int(os.environ.get("KERNEL_NQ", "1"))
EXCHANGE = os.environ.get("KERNEL_EXCHANGE", "pad")

F16 = mybir.dt.float16
F32 = mybir.dt.float32
F8 = mybir.dt.float8e4
I16 = mybir.dt.int16
NP_F8 = ml_dtypes.float8_e4m3


def _ceil(a, b):
    return -(-a // b)


class BuildOnly(Exception):
    pass


class EdgePlan:
    """Host-side bucketed edge plan for one SpMM pass, uniform across cores.

    src_row: int array, row index into the pass's gather table
    dst_pid: int array, padded NT id of the destination
    """

    def __init__(self, src_row, dst_pid, n_rows, s_pad, n_tiles):
        self.n_chunks = max(1, _ceil(n_rows, 32768))
        self.chunk = _ceil(n_rows, self.n_chunks)
        self.n_tiles = n_tiles
        nch = self.n_chunks

        core = dst_pid // s_pad
        dloc = dst_pid - core * s_pad
        tl = dloc // P
        dstloc = dloc % P
        ch = src_row // self.chunk

        key = (core * n_tiles + tl) * nch + ch
        counts = np.bincount(key, minlength=NCORES * n_tiles * nch).reshape(
            NCORES, n_tiles, nch
        )
        caps = counts.max(axis=0)  # [n_tiles, nch]
        caps = ((caps + 15) // 16) * 16  # 16-granular buckets
        self.caps = caps

        # per (chunk, group) call: buckets packed back-to-back at 16-granule,
        # call padded to 128 so its gather buffer is slot-aligned
        self.n_groups = _ceil(n_tiles, GROUP)
        self.call_num = np.zeros((nch, self.n_groups), dtype=np.int64)
        self.buck_off = np.zeros((nch, n_tiles), dtype=np.int64)
        for c in range(nch):
            for g in range(self.n_groups):
                off = 0
                for t in range(g * GROUP, min((g + 1) * GROUP, n_tiles)):
                    self.buck_off[c, t] = off
                    off += caps[t, c]
                self.call_num[c, g] = _ceil(off, P) * P
        self.call_off = np.zeros((nch, self.n_groups), dtype=np.int64)
        off = 0
        for c in range(nch):
            for g in range(self.n_groups):
                self.call_off[c, g] = off
                off += self.call_num[c, g]
        self.total_idx = off

        # sub-calls of <= MAX_CALL idxs: per (c, g) a list of (idx_off, num, col0)
        self.subcalls = {}
        for c in range(nch):
            for g in range(self.n_groups):
                num = int(self.call_num[c, g])
                base = int(self.call_off[c, g])
                subs = []
                p0 = 0
                while p0 < num:
                    n_ = min(MAX_CALL, num - p0)
                    subs.append((base + p0, n_, p0 // P))
                    p0 += n_
                self.subcalls[(c, g)] = subs

        # per tile: list of (chunk, slot-in-call-buffer) S-blocks, in matmul order
        self.tile_blocks = []
        for t in range(n_tiles):
            blocks = []
            for c in range(nch):
                cap = int(caps[t, c])
                if cap == 0:
                    continue
                p0 = int(self.buck_off[c, t])
                for sl in range(p0 // P, (p0 + cap - 1) // P + 1):
                    blocks.append((c, sl))
            self.tile_blocks.append(blocks)
        self.slots_per_tile = np.array([len(b) for b in self.tile_blocks])
        self.s_off = np.concatenate([[0], np.cumsum(self.slots_per_tile)])
        self.total_slots = int(self.s_off[-1])

        # order edges by (core, chunk, tile); build padded per-core streams
        order = np.argsort((core * nch + ch) * n_tiles + tl, kind="stable")
        src_o = src_row[order]
        core_o = core[order]
        ch_o = ch[order]
        tl_o = tl[order]
        dst_o = dstloc[order]

        base_tc = np.zeros((nch, n_tiles), dtype=np.int64)
        for c in range(nch):
            for g in range(self.n_groups):
                for t in range(g * GROUP, min((g + 1) * GROUP, n_tiles)):
                    base_tc[c, t] = self.call_off[c, g] + self.buck_off[c, t]

        self.idx16 = np.zeros((NCORES, self.total_idx), dtype=np.int16)
        self.dstloc = np.full((NCORES, self.total_idx), -1, dtype=np.int16)
        grp_key = (core_o * nch + ch_o) * n_tiles + tl_o
        uniq, inv, cnt = np.unique(grp_key, return_inverse=True, return_counts=True)
        starts = np.concatenate([[0], np.cumsum(cnt)])[:-1]
        rank = np.arange(len(grp_key)) - starts[inv]
        pos = base_tc[ch_o, tl_o] + rank
        self.idx16[core_o, pos] = (src_o - ch_o * self.chunk).astype(np.int16)
        self.dstloc[core_o, pos] = dst_o.astype(np.int16)

    def wrapped_idx(self, core):
        """[128, total_idx//16] int16, wrapped-16 and replicated to 8 groups."""
        v = self.idx16[core].reshape(-1, 16).T  # [16, total/16]
        return np.tile(v, (8, 1)).copy()

    def s_blob(self, core):
        """[128, total_slots*128] fp8 selector blocks, tile-major.

        Block k of tile t (chunk c, call slot sl) has S[r, d] = 1 iff call
        position 128*sl + r belongs to tile t's (t, c) bucket and maps an
        edge with dst_local d.  Rows outside the bucket range (neighbouring
        tiles sharing the slot, or padding) are zero.
        """
        out = np.zeros((P, self.total_slots * P), dtype=NP_F8)
        iota = np.arange(P, dtype=np.int16)
        for t in range(self.n_tiles):
            si = int(self.s_off[t])
            for (c, sl) in self.tile_blocks[t]:
                g = t // GROUP
                p0 = int(self.buck_off[c, t])
                cap = int(self.caps[t, c])
                gp = 128 * sl + iota.astype(np.int64)  # call positions of rows
                inb = (gp >= p0) & (gp < p0 + cap)
                d = np.full(P, -1, dtype=np.int64)
                d[inb] = self.dstloc[core, int(self.call_off[c, g]) + gp[inb]]
                sblk = np.zeros((P, P), dtype=NP_F8)
                rows = np.nonzero(d >= 0)[0]
                sblk[rows, d[rows]] = 1.0
                out[:, si * P : (si + 1) * P] = sblk
                si += 1
        return out


def _build_program(pm, s_pad, st_pad, tn, tt):
    """pm: main-pass EdgePlan (NT->NT)."""
    nt_pad = NCORES * s_pad
    nc = bacc.Bacc(
        None, target_bir_lowering=False, num_devices=NCORES, num_swdge_queues=NQ
    )

    def param(name, shape, dt, out=False):
        return nc.declare_dram_parameter(name, list(shape), dt, isOutput=out)

    tbl_init = param("tbl_init", (nt_pad, TPAD), F16)
    idx_nt = param("idx_nt", (P, pm.total_idx // 16), I16)
    s_nt = param("s_nt", (P, pm.total_slots * P), F8)
    xnt = param("xnt", (512, s_pad), F16)  # pre-transposed on host
    xt = param("xt", (512, st_pad), F16)
    w1t = param("w1t", (512, 256), F16)
    b1 = param("b1", (256, 1), F32)
    w2t = param("w2t", (256, 40), F16)
    b2b = param("b2b", (P, 40), F32)
    alpha_nt = param("alpha_nt", (s_pad, 1), F32)
    alpha_t = param("alpha_t", (st_pad, 1), F32)
    dis_nt = param("dis_nt", (s_pad, 1), F32)
    dissq_nt = param("dissq_nt", (s_pad, 1), F32)
    own_init = param("own_init", (s_pad, 40), F16)
    c1p = param("c1p", (s_pad, 40), F32)
    c2p = param("c2p", (s_pad, 40), F32)
    hard_t = param("hard_t", (st_pad, 40), F32)
    out_nt = param("out_nt", (s_pad, 40), F32, out=True)
    out_t = param("out_t", (st_pad, 40), F32, out=True)

    if EXCHANGE == "pad":
        cown = nc.dram_tensor("cown", [s_pad, TPAD], F16)
        callg = nc.dram_tensor("callg", [nt_pad, TPAD], F16, addr_space="Shared")
        table = None
    else:
        cown = nc.dram_tensor("cown", [s_pad, 40], F16)
        callg = nc.dram_tensor("callg", [nt_pad, 40], F16, addr_space="Shared")
        table = nc.dram_tensor("table", [nt_pad, TPAD], F16)

    RG = [list(range(NCORES))]

    with tile.TileContext(nc) as tc:
        with (
            tc.tile_pool(name="persist", bufs=1) as pp,
            tc.tile_pool(name="work", bufs=4) as wp,
            tc.tile_pool(name="gpool", bufs=4) as gp,
            tc.tile_pool(name="spool", bufs=3) as sp,
            tc.tile_pool(name="mpsum", bufs=2, space="PSUM") as mp,
            tc.tile_pool(name="apsum", bufs=4, space="PSUM") as ap_,
        ):
            if table is not None:
                nc.sync.dma_start(out=table[:, :], in_=tbl_init[:, :])

            # ---- persistent SBUF ----
            idxm_sb = pp.tile([P, pm.total_idx // 16], I16, tag="idxm")
            nc.sync.dma_start(out=idxm_sb[:], in_=idx_nt[:, :])

            ft_nt = pp.tile([P, tn, 40], F32, tag="ftnt")
            ft_t = pp.tile([P, tt, 40], F32, tag="ftt")
            cwid = TPAD if EXCHANGE == "pad" else 40
            compact = pp.tile([P, tn, cwid], F16, tag="compact")
            if EXCHANGE == "pad":
                nc.vector.memset(compact[:], 0.0)
            nc.sync.dma_start(
                out=compact[:, :, 0:40],
                in_=own_init.ap().rearrange("(t p) c -> p t c", p=P),
            )

            w1_sb = pp.tile([P, 4, 256], F16, tag="w1")
            nc.sync.dma_start(
                out=w1_sb[:], in_=w1t.ap().rearrange("(k p) h -> p k h", p=P)
            )
            w2_sb = pp.tile([P, 2, 40], F16, tag="w2")
            nc.sync.dma_start(
                out=w2_sb[:], in_=w2t.ap().rearrange("(h p) c -> p h c", p=P)
            )
            b1_sb = pp.tile([P, 2], F32, tag="b1")
            nc.sync.dma_start(
                out=b1_sb[:], in_=b1.ap().rearrange("(h p) o -> p (h o)", p=P)
            )
            b2_sb = pp.tile([P, 40], F32, tag="b2")
            nc.sync.dma_start(out=b2_sb[:], in_=b2b[:, :])

            c1 = pp.tile([P, tn, 40], F32, tag="c1")
            nc.sync.dma_start(
                out=c1[:], in_=c1p.ap().rearrange("(t p) c -> p t c", p=P)
            )
            c2 = pp.tile([P, tn, 40], F32, tag="c2")
            nc.sync.dma_start(
                out=c2[:], in_=c2p.ap().rearrange("(t p) c -> p t c", p=P)
            )

            def cols_load(prm, n_tiles, tag):
                t_ = pp.tile([P, n_tiles], F32, tag=tag)
                nc.sync.dma_start(
                    out=t_[:], in_=prm.ap().rearrange("(t p) o -> p (t o)", p=P)
                )
                return t_

            disn_sb = cols_load(dis_nt, tn, "disn")
            dsqn_sb = cols_load(dissq_nt, tn, "dsqn")
            aln_sb = cols_load(alpha_nt, tn, "aln")
            alt_sb = cols_load(alpha_t, tt, "alt")

            # sigmoid(alpha); a*dis; 1-a
            sign_sb = pp.tile([P, tn], F32, tag="sign")
            nc.scalar.activation(
                sign_sb[:], aln_sb[:], mybir.ActivationFunctionType.Sigmoid
            )
            sigt_sb = pp.tile([P, tt], F32, tag="sigt")
            nc.scalar.activation(
                sigt_sb[:], alt_sb[:], mybir.ActivationFunctionType.Sigmoid
            )
            disa_sb = pp.tile([P, tn], F32, tag="disa")
            nc.vector.tensor_tensor(
                out=disa_sb[:], in0=sign_sb[:], in1=disn_sb[:],
                op=mybir.AluOpType.mult,
            )
            oman_sb = pp.tile([P, tn], F32, tag="oman")
            nc.vector.tensor_scalar(
                out=oman_sb[:], in0=sign_sb[:], scalar1=-1.0, scalar2=1.0,
                op0=mybir.AluOpType.mult, op1=mybir.AluOpType.add,
            )
            omat_sb = pp.tile([P, tt], F32, tag="omat")
            nc.vector.tensor_scalar(
                out=omat_sb[:], in0=sigt_sb[:], scalar1=-1.0, scalar2=1.0,
                op0=mybir.AluOpType.mult, op1=mybir.AluOpType.add,
            )

            # ---- MLP (FT branch) ----
            def mlp(xsrc, n_tiles, ft_dst):
                for n in range(n_tiles):
                    xTs = []
                    for k in range(4):
                        xT = wp.tile([P, P], F16, tag="xT")
                        nc.sync.dma_start(
                            out=xT[:],
                            in_=xsrc[k * P : (k + 1) * P, n * P : (n + 1) * P],
                        )
                        xTs.append(xT)
                    ps2 = mp.tile([P, 40], F32, tag="ps2")
                    for h in range(2):
                        ps1 = mp.tile([P, P], F32, tag="ps1")
                        for k in range(4):
                            nc.tensor.matmul(
                                ps1[:],
                                lhsT=w1_sb[:, k, h * P : (h + 1) * P],
                                rhs=xTs[k][:],
                                start=(k == 0),
                                stop=(k == 3),
                            )
                        hT = wp.tile([P, P], F16, tag="hT")
                        nc.scalar.activation(
                            hT[:], ps1[:], mybir.ActivationFunctionType.Relu,
                            bias=b1_sb[:, h : h + 1],
                        )
                        nc.tensor.matmul(
                            ps2[:], lhsT=hT[:], rhs=w2_sb[:, h, :],
                            start=(h == 0), stop=(h == 1),
                        )
                    nc.vector.tensor_tensor(
                        out=ft_dst[:, n, :], in0=ps2[:], in1=b2_sb[:],
                        op=mybir.AluOpType.add,
                    )

            mlp(xnt, tn, ft_nt)
            mlp(xt, tt, ft_t)

            # ---- generic SpMM pass ----
            _regs = {}

            def num_reg(v):
                if v not in _regs:
                    _regs[v] = nc.gpsimd.to_reg(v)
                return _regs[v]

            _q = [0]

            def spmm_pass(plan, tsrc, idx_sb, s_param, evac):
                """tsrc: DRAM table. evac(t, psum_ap) -> emits eviction."""
                nch = plan.n_chunks
                for g in range(plan.n_groups):
                    gbufs = []
                    for c in range(nch):
                        num = int(plan.call_num[c, g])
                        if num == 0:
                            gbufs.append(None)
                            continue
                        gb = gp.tile([P, num // P, TPAD], F16, tag="gb")
                        r0 = c * plan.chunk
                        nrow = plan.chunk
                        if os.environ.get("KERNEL_NO_GATHER", "0") == "1":
                            # debug: sequential read instead of gather
                            nc.sync.dma_start(
                                out=gb[:],
                                in_=tsrc[r0 : r0 + num, :].rearrange(
                                    "(n p) e -> p n e", p=P
                                ),
                            )
                        else:
                            for off, n_, col0 in plan.subcalls[(c, g)]:
                                nc.gpsimd.dma_gather(
                                    out_ap=gb[:, col0 : col0 + n_ // P, :],
                                    in_ap=tsrc[r0 : r0 + nrow, :],
                                    idxs_ap=idx_sb[:, off // 16 : (off + n_) // 16],
                                    num_idxs=n_,
                                    num_idxs_reg=num_reg(n_),
                                    elem_size=TPAD,
                                    queue_num=_q[0] % NQ,
                                )
                                _q[0] += 1
                        gbufs.append(gb)
                    for t in range(g * GROUP, min((g + 1) * GROUP, plan.n_tiles)):
                        tot = int(plan.slots_per_tile[t])
                        if tot == 0:
                            continue
                        si = int(plan.s_off[t])
                        st_ = sp.tile([P, tot * P], F8, tag="sstr")
                        nc.sync.dma_start(
                            out=st_[:], in_=s_param[:, si * P : (si + tot) * P]
                        )
                        ps = ap_.tile([P, 40], F32, tag="acc")
                        for k, (c, sl) in enumerate(plan.tile_blocks[t]):
                            nc.tensor.matmul(
                                ps[:],
                                lhsT=st_[:, k * P : (k + 1) * P],
                                rhs=gbufs[c][:, sl, 0:40],
                                start=(k == 0),
                                stop=(k == tot - 1),
                            )
                        evac(t, ps)

            # ---- 10 PLP iterations ----
            for it in range(10):
                if EXCHANGE == "pad":
                    tsrc = tbl_init if it == 0 else callg
                else:
                    tsrc = table
                cbuf = c1 if it == 0 else c2

                if it < 9:
                    def evac_iter(t, ps, cbuf=cbuf):
                        tmp0 = wp.tile([P, 40], F32, tag="ev0")
                        nc.vector.tensor_tensor(
                            out=tmp0[:], in0=ps[:], in1=compact[:, t, 0:40],
                            op=mybir.AluOpType.add,
                        )
                        tmp = wp.tile([P, 40], F32, tag="ev")
                        nc.vector.tensor_tensor(
                            out=tmp[:], in0=tmp0[:], in1=cbuf[:, t, :],
                            op=mybir.AluOpType.add,
                        )
                        nc.vector.tensor_scalar(
                            out=compact[:, t, 0:40], in0=tmp[:],
                            scalar1=dsqn_sb[:, t : t + 1], scalar2=None,
                            op0=mybir.AluOpType.mult,
                        )
                else:
                    def evac_iter(t, ps, cbuf=cbuf):
                        tmp0 = wp.tile([P, 40], F32, tag="ev0")
                        nc.vector.tensor_tensor(
                            out=tmp0[:], in0=ps[:], in1=compact[:, t, 0:40],
                            op=mybir.AluOpType.add,
                        )
                        tmp = wp.tile([P, 40], F32, tag="ev")
                        nc.vector.tensor_tensor(
                            out=tmp[:], in0=tmp0[:], in1=cbuf[:, t, :],
                            op=mybir.AluOpType.add,
                        )
                        t2 = wp.tile([P, 40], F32, tag="ev2")
                        nc.vector.tensor_scalar(
                            out=t2[:], in0=tmp[:],
                            scalar1=disa_sb[:, t : t + 1], scalar2=None,
                            op0=mybir.AluOpType.mult,
                        )
                        t3 = wp.tile([P, 40], F32, tag="ev3")
                        nc.vector.tensor_scalar(
                            out=t3[:], in0=ft_nt[:, t, :],
                            scalar1=oman_sb[:, t : t + 1], scalar2=None,
                            op0=mybir.AluOpType.mult,
                        )
                        t4 = wp.tile([P, 40], F32, tag="ev4")
                        nc.vector.tensor_tensor(
                            out=t4[:], in0=t2[:], in1=t3[:],
                            op=mybir.AluOpType.add,
                        )
                        nc.sync.dma_start(
                            out=out_nt[t * P : (t + 1) * P, :], in_=t4[:]
                        )

                spmm_pass(pm, tsrc, idxm_sb, s_nt, evac_iter)

                if it < 9:
                    nc.sync.dma_start(
                        out=cown.ap().rearrange("(t p) c -> p t c", p=P),
                        in_=compact[:],
                    )
                    if os.environ.get("KERNEL_NO_CC", "0") == "1":
                        # debug mode: skip the collective (wrong cross-core data)
                        nc.sync.dma_start(
                            out=callg[0 : s_pad, :], in_=cown[:, :]
                        )
                    else:
                        nc.gpsimd.collective_compute(
                            "AllGather",
                            mybir.AluOpType.bypass,
                            replica_groups=RG,
                            ins=[cown.ap().opt()],
                            outs=[callg.ap().opt()],
                        )
                    if EXCHANGE != "pad":
                        nc.sync.dma_start(out=table[:, 0:40], in_=callg[:, :])

            # ---- T-side final combine ----
            for t in range(tt):
                hsb = wp.tile([P, 40], F32, tag="hsb")
                nc.sync.dma_start(out=hsb[:], in_=hard_t[t * P : (t + 1) * P, :])
                t1_ = wp.tile([P, 40], F32, tag="tc1")
                nc.vector.tensor_scalar(
                    out=t1_[:], in0=hsb[:], scalar1=sigt_sb[:, t : t + 1],
                    scalar2=None, op0=mybir.AluOpType.mult,
                )
                t2_ = wp.tile([P, 40], F32, tag="tc2")
                nc.vector.tensor_scalar(
                    out=t2_[:], in0=ft_t[:, t, :], scalar1=omat_sb[:, t : t + 1],
                    scalar2=None, op0=mybir.AluOpType.mult,
                )
                t3_ = wp.tile([P, 40], F32, tag="tc3")
                nc.vector.tensor_tensor(
                    out=t3_[:], in0=t1_[:], in1=t2_[:],
                    op=mybir.AluOpType.add,
                )
                nc.sync.dma_start(out=out_t[t * P : (t + 1) * P, :], in_=t3_[:])

    nc.compile()
    return nc


def kernel(**inputs):
    x = np.asarray(inputs["x"], dtype=np.float32)
    edge_index = np.asarray(inputs["edge_index"])
    label_init = np.asarray(inputs["label_init"], dtype=np.float32)
    train_mask = np.asarray(inputs["train_mask"]).astype(bool)
    hard = np.asarray(inputs["hard_one_hot"], dtype=np.float32)
    fc1_w = np.asarray(inputs["fc1_w"], dtype=np.float32)
    fc1_b = np.asarray(inputs["fc1_b"], dtype=np.float32)
    fc2_w = np.asarray(inputs["fc2_w"], dtype=np.float32)
    fc2_b = np.asarray(inputs["fc2_b"], dtype=np.float32)
    alpha = np.asarray(inputs["alpha"], dtype=np.float32)

    n = x.shape[0]
    row = edge_index[0].astype(np.int64)
    col = edge_index[1].astype(np.int64)

    deg = np.bincount(row, minlength=n).astype(np.float64) + 1.0
    dis = (1.0 / np.sqrt(deg)).astype(np.float32)

    nt_ids = np.nonzero(~train_mask)[0]
    t_ids = np.nonzero(train_mask)[0]
    n_nt, n_t = len(nt_ids), len(t_ids)

    s_real = _ceil(n_nt, NCORES)
    tn = _ceil(s_real, P)
    s_pad = tn * P
    nt_pad = NCORES * s_pad
    st_real = _ceil(n_t, NCORES)
    tt = _ceil(st_real, P)
    st_pad = tt * P

    # padded NT id / compact T id for each original node
    pid = np.full(n, -1, dtype=np.int64)
    j = np.arange(n_nt)
    stripe = j // s_real
    pid[nt_ids] = stripe * s_pad + (j - stripe * s_real)

    # edges into NT dsts
    sel = ~train_mask[col]
    es, ed = row[sel], col[sel]
    src_nt = ~train_mask[es]
    # main: NT->NT (self-loops handled in the evac via the compact tile)
    m_src = pid[es[src_nt]]
    m_dst = pid[ed[src_nt]]
    pm = EdgePlan(m_src, m_dst, nt_pad, s_pad, tn)

    # ---- host-precomputed T-source constants c1/c2 ----
    scaled_li = dis[:, None] * label_init  # [n, 40]
    scaled_hd = dis[:, None] * hard
    es_t, ed_t = es[~src_nt], ed[~src_nt]
    d_pid_t = pid[ed_t]
    order_t = np.argsort(d_pid_t, kind="stable")
    d_sorted = d_pid_t[order_t]
    uniq_d, start_i = np.unique(d_sorted, return_index=True)
    c1_full = np.zeros((nt_pad, 40), dtype=np.float32)
    c2_full = np.zeros((nt_pad, 40), dtype=np.float32)
    if len(uniq_d):
        c1_full[uniq_d] = np.add.reduceat(
            scaled_li[es_t[order_t]].astype(np.float64), start_i, axis=0
        )
        c2_full[uniq_d] = np.add.reduceat(
            scaled_hd[es_t[order_t]].astype(np.float64), start_i, axis=0
        )

    # ---- tables ----
    tbl_init_g = np.zeros((nt_pad, TPAD), dtype=np.float16)
    tbl_init_g[pid[nt_ids], :40] = scaled_li[nt_ids].astype(np.float16)
    own_init_g = tbl_init_g[:, :40].reshape(NCORES, s_pad, 40).copy()

    # ---- per-core MLP / combine inputs ----
    def stripe_rows(ids, srl, spad_, nstripes=NCORES):
        """Return [nstripes, spad_] original-id per padded slot (-1 pad)."""
        m = np.full((nstripes, spad_), -1, dtype=np.int64)
        for i in range(nstripes):
            lo = i * srl
            hi = min(len(ids), (i + 1) * srl)
            if hi > lo:
                m[i, : hi - lo] = ids[lo:hi]
        return m

    nt_map = stripe_rows(nt_ids, s_real, s_pad)
    t_map = stripe_rows(t_ids, st_real, st_pad)

    def take(arr, idmap, fill=0.0):
        out = np.full((idmap.shape[0], idmap.shape[1]) + arr.shape[1:], fill,
                      dtype=arr.dtype)
        valid = idmap >= 0
        out[valid] = arr[idmap[valid]]
        return out

    xnt_g = np.ascontiguousarray(
        take(x, nt_map).astype(np.float16).transpose(0, 2, 1)
    )
    xt_g = np.ascontiguousarray(take(x, t_map).astype(np.float16).transpose(0, 2, 1))
    al_nt_g = take(alpha, nt_map).astype(np.float32)
    al_t_g = take(alpha, t_map).astype(np.float32)
    dis_nt_g = take(dis[:, None], nt_map).astype(np.float32)
    dsq_nt_g = take((dis * dis)[:, None], nt_map).astype(np.float32)
    hard_t_g = take(hard, t_map).astype(np.float32)

    w1t_g = fc1_w.T.astype(np.float16).copy()  # [512, 256]
    b1_g = fc1_b.reshape(256, 1).astype(np.float32)
    w2t_g = fc2_w.T.astype(np.float16).copy()  # [256, 40]
    b2b_g = np.tile(fc2_b.reshape(1, 40), (P, 1)).astype(np.float32)

    nc = _build_program(pm, s_pad, st_pad, tn, tt)

    if os.environ.get("KERNEL_BUILD_ONLY", "0") == "1":
        e = BuildOnly()
        e.nc = nc
        raise e

    in_maps = []
    for i in range(NCORES):
        in_maps.append(
            dict(
                tbl_init=tbl_init_g,
                idx_nt=pm.wrapped_idx(i),
                s_nt=pm.s_blob(i),
                xnt=xnt_g[i],
                xt=xt_g[i],
                w1t=w1t_g,
                b1=b1_g,
                w2t=w2t_g,
                b2b=b2b_g,
                alpha_nt=al_nt_g[i],
                alpha_t=al_t_g[i],
                dis_nt=dis_nt_g[i],
                dissq_nt=dsq_nt_g[i],
                own_init=own_init_g[i],
                c1p=c1_full[i * s_pad : (i + 1) * s_pad],
                c2p=c2_full[i * s_pad : (i + 1) * s_pad],
                hard_t=hard_t_g[i],
            )
        )

    if os.environ.get("KERNEL_SIM", "0") == "1":
        from concourse import bass_interp

        sim = bass_interp.MultiCoreSim(nc, NCORES)
        for i in range(NCORES):
            for k, v in in_maps[i].items():
                sim.cores[i].tensor(k)[:] = v
        sim.simulate()
        results = [
            {k: np.array(sim.cores[i].mem_tensor(k)) for k in ("out_nt", "out_t")}
            for i in range(NCORES)
        ]
        res = None
    else:
        res = run_bass_kernel_spmd(
            nc, in_maps, core_ids=list(range(NCORES)),
            trace=bool(int(os.environ.get("KERNEL_TRACE", "0"))),
        )
        results = res.results
    kernel.last_results = res
    kernel.last_nc = nc
    kernel.last_in_maps = in_maps

    out = np.zeros((n, 40), dtype=np.float32)
    for i in range(NCORES):
        om = results[i]["out_nt"]
        ot = results[i]["out_t"]
        v = nt_map[i] >= 0
        out[nt_map[i][v]] = om[v]
        v = t_map[i] >= 0
        out[t_map[i][v]] = ot[v]
    return out


# revision 10
# speedup vs baseline: 2.1735x; 1.6815x over previous
"""CPFStudent (GNN label propagation + MLP mix) on 8 TRN2 NeuronCores.

Strategy (dst-sharded SpMM with selector matmuls), v2:
  - Reference: 10 PLP steps of plp <- where(mask, hard, A_hat @ plp), with
    A_hat = D^-1/2 (A+I) D^-1/2 built from out-degrees of edge_index[0];
    final logits = sigmoid(alpha)*plp + (1-sigmoid(alpha))*relu(x@W1^T+b1)@W2^T+b2.
  - Only non-train (NT) rows of plp evolve; train (T) rows are constant after
    step 1.  State kept as table = dis * plp (dis = deg^-1/2), fp16:
        plp_new[d] = dis[d] * ( sum_{e: src NT} table[src] + c )
    where c is a per-dst constant: c1 (dis*label_init over T srcs, step 1) or
    c2 (dis*hard over T srcs, steps 2..10).  c1/c2 are precomputed HOST-side
    (they are iteration-invariant) — no T-side SpMM passes on device.
  - Nodes permuted host-side: NT nodes first, padded per-core stripes.  Each
    core owns a contiguous stripe of NT dst rows; edges bucketed by (dst tile
    of 128, src chunk of <=32768) host-side at 16-edge granularity, padded to
    uniform capacities across cores (SPMD), gathered per iteration with
    gpsimd.dma_gather (256B elements, <=1024 idxs per call) from an HBM fp16
    table.  Buckets pack back-to-back inside each call, so a 128-edge matmul
    slot may span two buckets; each overlapped tile gets its own range-masked
    S block (zeros outside its bucket), keeping descriptors ~6% leaner than
    slot-aligned packing.
  - Scatter/segment-sum on the TensorEngine: per 128-edge slot a
    host-precomputed fp8 selector S (S[e,d]=1 iff dst_local(e)==d) multiplies
    the gathered messages, accumulating in PSUM per dst tile.
  - Halo exchange: AllGather of each core's full-width (padded 256B) rows
    directly produces the next iteration's gather table — no post-collective
    re-strided table write.
  - Self-loops are folded into the eviction: the previous iteration's own
    `compact` rows are exactly the self-loop messages, so they never hit the
    gather path.
"""

import math
import os
import sys

import numpy as np

sys.path.insert(0, "/opt/trn_rl_repo")

import ml_dtypes  # noqa: E402

import concourse.bass as bass  # noqa: E402
import concourse.mybir as mybir  # noqa: E402
import concourse.tile as tile  # noqa: E402
from concourse import bacc  # noqa: E402
from concourse.bass_utils import run_bass_kernel_spmd  # noqa: E402

P = 128
NCORES = 8
TPAD = 128  # fp16 elements per table row (256B, dma_gather elem granularity)
GROUP = 7  # dst tiles per dma_gather call group
MAX_CALL = int(os.environ.get("KERNEL_MAX_CALL", "1024"))
NQ = int(os.environ.get("KERNEL_NQ", "1"))
EXCHANGE = os.environ.get("KERNEL_EXCHANGE", "pad")

F16 = mybir.dt.float16
F32 = mybir.dt.float32
F8 = mybir.dt.float8e4
I16 = mybir.dt.int16
NP_F8 = ml_dtypes.float8_e4m3


def _ceil(a, b):
    return -(-a // b)


class BuildOnly(Exception):
    pass


class EdgePlan:
    """Host-side bucketed edge plan for one SpMM pass, uniform across cores.

    src_row: int array, row index into the pass's gather table
    dst_pid: int array, padded NT id of the destination
    """

    def __init__(self, src_row, dst_pid, n_rows, s_pad, n_tiles):
        self.n_chunks = max(1, _ceil(n_rows, 32768))
        self.chunk = _ceil(n_rows, self.n_chunks)
        self.n_tiles = n_tiles
        nch = self.n_chunks

        core = dst_pid // s_pad
        dloc = dst_pid - core * s_pad
        tl = dloc // P
        dstloc = dloc % P
        ch = src_row // self.chunk

        key = (core * n_tiles + tl) * nch + ch
        counts = np.bincount(key, minlength=NCORES * n_tiles * nch).reshape(
            NCORES, n_tiles, nch
        )
        caps = counts.max(axis=0)  # [n_tiles, nch]
        caps = ((caps + 15) // 16) * 16  # 16-granular buckets
        self.caps = caps

        # per (chunk, group) call: buckets packed back-to-back at 16-granule,
        # call padded to 128 so its gather buffer is slot-aligned
        self.n_groups = _ceil(n_tiles, GROUP)
        self.call_num = np.zeros((nch, self.n_groups), dtype=np.int64)
        self.buck_off = np.zeros((nch, n_tiles), dtype=np.int64)
        for c in range(nch):
            for g in range(self.n_groups):
                off = 0
                for t in range(g * GROUP, min((g + 1) * GROUP, n_tiles)):
                    self.buck_off[c, t] = off
                    off += caps[t, c]
                self.call_num[c, g] = _ceil(off, P) * P
        self.call_off = np.zeros((nch, self.n_groups), dtype=np.int64)
        off = 0
        for c in range(nch):
            for g in range(self.n_groups):
                self.call_off[c, g] = off
                off += self.call_num[c, g]
        self.total_idx = off

        # sub-calls of <= MAX_CALL idxs: per (c, g) a list of (idx_off, num, col0)
        self.subcalls = {}
        for c in range(nch):
            for g in range(self.n_groups):
                num = int(self.call_num[c, g])
                base = int(self.call_off[c, g])
                subs = []
                p0 = 0
                while p0 < num:
                    n_ = min(MAX_CALL, num - p0)
                    subs.append((base + p0, n_, p0 // P))
                    p0 += n_
                self.subcalls[(c, g)] = subs

        # per tile: list of (chunk, slot-in-call-buffer) S-blocks, in matmul order
        self.tile_blocks = []
        for t in range(n_tiles):
            blocks = []
            for c in range(nch):
                cap = int(caps[t, c])
                if cap == 0:
                    continue
                p0 = int(self.buck_off[c, t])
                for sl in range(p0 // P, (p0 + cap - 1) // P + 1):
                    blocks.append((c, sl))
            self.tile_blocks.append(blocks)
        self.slots_per_tile = np.array([len(b) for b in self.tile_blocks])
        self.s_off = np.concatenate([[0], np.cumsum(self.slots_per_tile)])
        self.total_slots = int(self.s_off[-1])

        # order edges by (core, chunk, tile); build padded per-core streams
        order = np.argsort((core * nch + ch) * n_tiles + tl, kind="stable")
        src_o = src_row[order]
        core_o = core[order]
        ch_o = ch[order]
        tl_o = tl[order]
        dst_o = dstloc[order]

        base_tc = np.zeros((nch, n_tiles), dtype=np.int64)
        for c in range(nch):
            for g in range(self.n_groups):
                for t in range(g * GROUP, min((g + 1) * GROUP, n_tiles)):
                    base_tc[c, t] = self.call_off[c, g] + self.buck_off[c, t]

        self.idx16 = np.zeros((NCORES, self.total_idx), dtype=np.int16)
        self.dstloc = np.full((NCORES, self.total_idx), -1, dtype=np.int16)
        grp_key = (core_o * nch + ch_o) * n_tiles + tl_o
        uniq, inv, cnt = np.unique(grp_key, return_inverse=True, return_counts=True)
        starts = np.concatenate([[0], np.cumsum(cnt)])[:-1]
        rank = np.arange(len(grp_key)) - starts[inv]
        pos = base_tc[ch_o, tl_o] + rank
        self.idx16[core_o, pos] = (src_o - ch_o * self.chunk).astype(np.int16)
        self.dstloc[core_o, pos] = dst_o.astype(np.int16)

    def wrapped_idx(self, core):
        """[128, total_idx//16] int16, wrapped-16 and replicated to 8 groups."""
        v = self.idx16[core].reshape(-1, 16).T  # [16, total/16]
        return np.tile(v, (8, 1)).copy()

    def s_blob(self, core):
        """[128, total_slots*128] fp8 selector blocks, tile-major.

        Block k of tile t (chunk c, call slot sl) has S[r, d] = 1 iff call
        position 128*sl + r belongs to tile t's (t, c) bucket and maps an
        edge with dst_local d.  Rows outside the bucket range (neighbouring
        tiles sharing the slot, or padding) are zero.
        """
        out = np.zeros((P, self.total_slots * P), dtype=NP_F8)
        iota = np.arange(P, dtype=np.int16)
        for t in range(self.n_tiles):
            si = int(self.s_off[t])
            for (c, sl) in self.tile_blocks[t]:
                g = t // GROUP
                p0 = int(self.buck_off[c, t])
                cap = int(self.caps[t, c])
                gp = 128 * sl + iota.astype(np.int64)  # call positions of rows
                inb = (gp >= p0) & (gp < p0 + cap)
                d = np.full(P, -1, dtype=np.int64)
                d[inb] = self.dstloc[core, int(self.call_off[c, g]) + gp[inb]]
                sblk = np.zeros((P, P), dtype=NP_F8)
                rows = np.nonzero(d >= 0)[0]
                sblk[rows, d[rows]] = 1.0
                out[:, si * P : (si + 1) * P] = sblk
                si += 1
        return out


def _build_program(pm, s_pad, st_pad, tn, tt):
    """pm: main-pass EdgePlan (NT->NT)."""
    nt_pad = NCORES * s_pad
    nc = bacc.Bacc(
        None, target_bir_lowering=False, num_devices=NCORES, num_swdge_queues=NQ
    )

    def param(name, shape, dt, out=False):
        return nc.declare_dram_parameter(name, list(shape), dt, isOutput=out)

    tbl_init = param("tbl_init", (nt_pad, TPAD), F16)
    idx_nt = param("idx_nt", (P, pm.total_idx // 16), I16)
    s_nt = param("s_nt", (P, pm.total_slots * P), F8)
    xnt = param("xnt", (512, s_pad), F16)  # pre-transposed on host
    xt = param("xt", (512, st_pad), F16)
    w1t = param("w1t", (512, 256), F16)
    b1 = param("b1", (256, 1), F32)
    w2t = param("w2t", (256, 40), F16)
    b2b = param("b2b", (P, 40), F32)
    alpha_nt = param("alpha_nt", (s_pad, 1), F32)
    alpha_t = param("alpha_t", (st_pad, 1), F32)
    dis_nt = param("dis_nt", (s_pad, 1), F32)
    dissq_nt = param("dissq_nt", (s_pad, 1), F32)
    own_init = param("own_init", (s_pad, 40), F16)
    c1p = param("c1p", (s_pad, 40), F32)
    c2p = param("c2p", (s_pad, 40), F32)
    hard_t = param("hard_t", (st_pad, 40), F32)
    out_nt = param("out_nt", (s_pad, 40), F32, out=True)
    out_t = param("out_t", (st_pad, 40), F32, out=True)

    if EXCHANGE == "pad":
        cown = nc.dram_tensor("cown", [s_pad, TPAD], F16)
        callg = nc.dram_tensor("callg", [nt_pad, TPAD], F16, addr_space="Shared")
        table = None
    else:
        cown = nc.dram_tensor("cown", [s_pad, 40], F16)
        callg = nc.dram_tensor("callg", [nt_pad, 40], F16, addr_space="Shared")
        table = nc.dram_tensor("table", [nt_pad, TPAD], F16)

    RG = [list(range(NCORES))]

    with tile.TileContext(nc) as tc:
        with (
            tc.tile_pool(name="persist", bufs=1) as pp,
            tc.tile_pool(name="work", bufs=4) as wp,
            tc.tile_pool(name="gpool", bufs=4) as gp,
            tc.tile_pool(name="spool", bufs=3) as sp,
            tc.tile_pool(name="mpsum", bufs=2, space="PSUM") as mp,
            tc.tile_pool(name="apsum", bufs=4, space="PSUM") as ap_,
        ):
            if table is not None:
                nc.sync.dma_start(out=table[:, :], in_=tbl_init[:, :])

            # ---- persistent SBUF ----
            idxm_sb = pp.tile([P, pm.total_idx // 16], I16, tag="idxm")
            nc.sync.dma_start(out=idxm_sb[:], in_=idx_nt[:, :])

            ft_nt = pp.tile([P, tn, 40], F32, tag="ftnt")
            ft_t = pp.tile([P, tt, 40], F32, tag="ftt")
            cwid = TPAD if EXCHANGE == "pad" else 40
            compact = pp.tile([P, tn, cwid], F16, tag="compact")
            if EXCHANGE == "pad":
                nc.vector.memset(compact[:], 0.0)
            nc.sync.dma_start(
                out=compact[:, :, 0:40],
                in_=own_init.ap().rearrange("(t p) c -> p t c", p=P),
            )

            w1_sb = pp.tile([P, 4, 256], F16, tag="w1")
            nc.sync.dma_start(
                out=w1_sb[:], in_=w1t.ap().rearrange("(k p) h -> p k h", p=P)
            )
            w2_sb = pp.tile([P, 2, 40], F16, tag="w2")
            nc.sync.dma_start(
                out=w2_sb[:], in_=w2t.ap().rearrange("(h p) c -> p h c", p=P)
            )
            b1_sb = pp.tile([P, 2], F32, tag="b1")
            nc.sync.dma_start(
                out=b1_sb[:], in_=b1.ap().rearrange("(h p) o -> p (h o)", p=P)
            )
            b2_sb = pp.tile([P, 40], F32, tag="b2")
            nc.sync.dma_start(out=b2_sb[:], in_=b2b[:, :])

            c1 = pp.tile([P, tn, 40], F32, tag="c1")
            nc.sync.dma_start(
                out=c1[:], in_=c1p.ap().rearrange("(t p) c -> p t c", p=P)
            )
            c2 = pp.tile([P, tn, 40], F32, tag="c2")
            nc.sync.dma_start(
                out=c2[:], in_=c2p.ap().rearrange("(t p) c -> p t c", p=P)
            )

            def cols_load(prm, n_tiles, tag):
                t_ = pp.tile([P, n_tiles], F32, tag=tag)
                nc.sync.dma_start(
                    out=t_[:], in_=prm.ap().rearrange("(t p) o -> p (t o)", p=P)
                )
                return t_

            disn_sb = cols_load(dis_nt, tn, "disn")
            dsqn_sb = cols_load(dissq_nt, tn, "dsqn")
            aln_sb = cols_load(alpha_nt, tn, "aln")
            alt_sb = cols_load(alpha_t, tt, "alt")

            # sigmoid(alpha); a*dis; 1-a
            sign_sb = pp.tile([P, tn], F32, tag="sign")
            nc.scalar.activation(
                sign_sb[:], aln_sb[:], mybir.ActivationFunctionType.Sigmoid
            )
            sigt_sb = pp.tile([P, tt], F32, tag="sigt")
            nc.scalar.activation(
                sigt_sb[:], alt_sb[:], mybir.ActivationFunctionType.Sigmoid
            )
            disa_sb = pp.tile([P, tn], F32, tag="disa")
            nc.vector.tensor_tensor(
                out=disa_sb[:], in0=sign_sb[:], in1=disn_sb[:],
                op=mybir.AluOpType.mult,
            )
            oman_sb = pp.tile([P, tn], F32, tag="oman")
            nc.vector.tensor_scalar(
                out=oman_sb[:], in0=sign_sb[:], scalar1=-1.0, scalar2=1.0,
                op0=mybir.AluOpType.mult, op1=mybir.AluOpType.add,
            )
            omat_sb = pp.tile([P, tt], F32, tag="omat")
            nc.vector.tensor_scalar(
                out=omat_sb[:], in0=sigt_sb[:], scalar1=-1.0, scalar2=1.0,
                op0=mybir.AluOpType.mult, op1=mybir.AluOpType.add,
            )

            # ---- MLP (FT branch) ----
            def mlp(xsrc, n_tiles, ft_dst):
                for n in range(n_tiles):
                    xTs = []
                    for k in range(4):
                        xT = wp.tile([P, P], F16, tag="xT")
                        nc.sync.dma_start(
                            out=xT[:],
                            in_=xsrc[k * P : (k + 1) * P, n * P : (n + 1) * P],
                        )
                        xTs.append(xT)
                    ps2 = mp.tile([P, 40], F32, tag="ps2")
                    for h in range(2):
                        ps1 = mp.tile([P, P], F32, tag="ps1")
                        for k in range(4):
                            nc.tensor.matmul(
                                ps1[:],
                                lhsT=w1_sb[:, k, h * P : (h + 1) * P],
                                rhs=xTs[k][:],
                                start=(k == 0),
                                stop=(k == 3),
                            )
                        hT = wp.tile([P, P], F16, tag="hT")
                        nc.scalar.activation(
                            hT[:], ps1[:], mybir.ActivationFunctionType.Relu,
                            bias=b1_sb[:, h : h + 1],
                        )
                        nc.tensor.matmul(
                            ps2[:], lhsT=hT[:], rhs=w2_sb[:, h, :],
                            start=(h == 0), stop=(h == 1),
                        )
                    nc.vector.tensor_tensor(
                        out=ft_dst[:, n, :], in0=ps2[:], in1=b2_sb[:],
                        op=mybir.AluOpType.add,
                    )

            mlp(xnt, tn, ft_nt)
            mlp(xt, tt, ft_t)

            # ---- generic SpMM pass ----
            _regs = {}

            def num_reg(v):
                if v not in _regs:
                    _regs[v] = nc.gpsimd.to_reg(v)
                return _regs[v]

            _q = [0]

            def spmm_pass(plan, tsrc, idx_sb, s_param, evac):
                """tsrc: DRAM table. evac(t, psum_ap) -> emits eviction."""
                nch = plan.n_chunks
                for g in range(plan.n_groups):
                    gbufs = []
                    for c in range(nch):
                        num = int(plan.call_num[c, g])
                        if num == 0:
                            gbufs.append(None)
                            continue
                        gb = gp.tile([P, num // P, TPAD], F16, tag="gb")
                        r0 = c * plan.chunk
                        nrow = plan.chunk
                        if os.environ.get("KERNEL_NO_GATHER", "0") == "1":
                            # debug: sequential read instead of gather
                            nc.sync.dma_start(
                                out=gb[:],
                                in_=tsrc[r0 : r0 + num, :].rearrange(
                                    "(n p) e -> p n e", p=P
                                ),
                            )
                        else:
                            for off, n_, col0 in plan.subcalls[(c, g)]:
                                nc.gpsimd.dma_gather(
                                    out_ap=gb[:, col0 : col0 + n_ // P, :],
                                    in_ap=tsrc[r0 : r0 + nrow, :],
                                    idxs_ap=idx_sb[:, off // 16 : (off + n_) // 16],
                                    num_idxs=n_,
                                    num_idxs_reg=num_reg(n_),
                                    elem_size=TPAD,
                                    queue_num=_q[0] % NQ,
                                )
                                _q[0] += 1
                        gbufs.append(gb)
                    for t in range(g * GROUP, min((g + 1) * GROUP, plan.n_tiles)):
                        tot = int(plan.slots_per_tile[t])
                        if tot == 0:
                            continue
                        si = int(plan.s_off[t])
                        st_ = sp.tile([P, tot * P], F8, tag="sstr")
                        nc.sync.dma_start(
                            out=st_[:], in_=s_param[:, si * P : (si + tot) * P]
                        )
                        ps = ap_.tile([P, 40], F32, tag="acc")
                        for k, (c, sl) in enumerate(plan.tile_blocks[t]):
                            nc.tensor.matmul(
                                ps[:],
                                lhsT=st_[:, k * P : (k + 1) * P],
                                rhs=gbufs[c][:, sl, 0:40],
                                start=(k == 0),
                                stop=(k == tot - 1),
                            )
                        evac(t, ps)

            # ---- 10 PLP iterations ----
            for it in range(10):
                if EXCHANGE == "pad":
                    tsrc = tbl_init if it == 0 else callg
                else:
                    tsrc = table
                cbuf = c1 if it == 0 else c2

                if it < 9:
                    def evac_iter(t, ps, cbuf=cbuf):
                        tmp0 = wp.tile([P, 40], F32, tag="ev0")
                        nc.vector.tensor_tensor(
                            out=tmp0[:], in0=ps[:], in1=compact[:, t, 0:40],
                            op=mybir.AluOpType.add,
                        )
                        tmp = wp.tile([P, 40], F32, tag="ev")
                        nc.vector.tensor_tensor(
                            out=tmp[:], in0=tmp0[:], in1=cbuf[:, t, :],
                            op=mybir.AluOpType.add,
                        )
                        nc.vector.tensor_scalar(
                            out=compact[:, t, 0:40], in0=tmp[:],
                            scalar1=dsqn_sb[:, t : t + 1], scalar2=None,
                            op0=mybir.AluOpType.mult,
                        )
                else:
                    def evac_iter(t, ps, cbuf=cbuf):
                        tmp0 = wp.tile([P, 40], F32, tag="ev0")
                        nc.vector.tensor_tensor(
                            out=tmp0[:], in0=ps[:], in1=compact[:, t, 0:40],
                            op=mybir.AluOpType.add,
                        )
                        tmp = wp.tile([P, 40], F32, tag="ev")
                        nc.vector.tensor_tensor(
                            out=tmp[:], in0=tmp0[:], in1=cbuf[:, t, :],
                            op=mybir.AluOpType.add,
                        )
                        t2 = wp.tile([P, 40], F32, tag="ev2")
                        nc.vector.tensor_scalar(
                            out=t2[:], in0=tmp[:],
                            scalar1=disa_sb[:, t : t + 1], scalar2=None,
                            op0=mybir.AluOpType.mult,
                        )
                        t3 = wp.tile([P, 40], F32, tag="ev3")
                        nc.vector.tensor_scalar(
                            out=t3[:], in0=ft_nt[:, t, :],
                            scalar1=oman_sb[:, t : t + 1], scalar2=None,
                            op0=mybir.AluOpType.mult,
                        )
                        t4 = wp.tile([P, 40], F32, tag="ev4")
                        nc.vector.tensor_tensor(
                            out=t4[:], in0=t2[:], in1=t3[:],
                            op=mybir.AluOpType.add,
                        )
                        nc.sync.dma_start(
                            out=out_nt[t * P : (t + 1) * P, :], in_=t4[:]
                        )

                spmm_pass(pm, tsrc, idxm_sb, s_nt, evac_iter)

                if it < 9:
                    nc.sync.dma_start(
                        out=cown.ap().rearrange("(t p) c -> p t c", p=P),
                        in_=compact[:],
                    )
                    if os.environ.get("KERNEL_NO_CC", "0") == "1":
                        # debug mode: skip the collective (wrong cross-core data)
                        nc.sync.dma_start(
                            out=callg[0 : s_pad, :], in_=cown[:, :]
                        )
                    else:
                        nc.gpsimd.collective_compute(
                            "AllGather",
                            mybir.AluOpType.bypass,
                            replica_groups=RG,
                            ins=[cown.ap().opt()],
                            outs=[callg.ap().opt()],
                        )
                    if EXCHANGE != "pad":
                        nc.sync.dma_start(out=table[:, 0:40], in_=callg[:, :])

            # ---- T-side final combine ----
            for t in range(tt):
                hsb = wp.tile([P, 40], F32, tag="hsb")
                nc.sync.dma_start(out=hsb[:], in_=hard_t[t * P : (t + 1) * P, :])
                t1_ = wp.tile([P, 40], F32, tag="tc1")
                nc.vector.tensor_scalar(
                    out=t1_[:], in0=hsb[:], scalar1=sigt_sb[:, t : t + 1],
                    scalar2=None, op0=mybir.AluOpType.mult,
                )
                t2_ = wp.tile([P, 40], F32, tag="tc2")
                nc.vector.tensor_scalar(
                    out=t2_[:], in0=ft_t[:, t, :], scalar1=omat_sb[:, t : t + 1],
                    scalar2=None, op0=mybir.AluOpType.mult,
                )
                t3_ = wp.tile([P, 40], F32, tag="tc3")
                nc.vector.tensor_tensor(
                    out=t3_[:], in0=t1_[:], in1=t2_[:],
                    op=mybir.AluOpType.add,
                )
                nc.sync.dma_start(out=out_t[t * P : (t + 1) * P, :], in_=t3_[:])

    nc.compile()
    return nc


def kernel(**inputs):
    x = np.asarray(inputs["x"], dtype=np.float32)
    edge_index = np.asarray(inputs["edge_index"])
    label_init = np.asarray(inputs["label_init"], dtype=np.float32)
    train_mask = np.asarray(inputs["train_mask"]).astype(bool)
    hard = np.asarray(inputs["hard_one_hot"], dtype=np.float32)
    fc1_w = np.asarray(inputs["fc1_w"], dtype=np.float32)
    fc1_b = np.asarray(inputs["fc1_b"], dtype=np.float32)
    fc2_w = np.asarray(inputs["fc2_w"], dtype=np.float32)
    fc2_b = np.asarray(inputs["fc2_b"], dtype=np.float32)
    alpha = np.asarray(inputs["alpha"], dtype=np.float32)

    n = x.shape[0]
    row = edge_index[0].astype(np.int64)
    col = edge_index[1].astype(np.int64)

    deg = np.bincount(row, minlength=n).astype(np.float64) + 1.0
    dis = (1.0 / np.sqrt(deg)).astype(np.float32)

    nt_ids = np.nonzero(~train_mask)[0]
    t_ids = np.nonzero(train_mask)[0]
    n_nt, n_t = len(nt_ids), len(t_ids)

    s_real = _ceil(n_nt, NCORES)
    tn = _ceil(s_real, P)
    s_pad = tn * P
    nt_pad = NCORES * s_pad
    st_real = _ceil(n_t, NCORES)
    tt = _ceil(st_real, P)
    st_pad = tt * P

    # padded NT id / compact T id for each original node
    pid = np.full(n, -1, dtype=np.int64)
    j = np.arange(n_nt)
    stripe = j // s_real
    pid[nt_ids] = stripe * s_pad + (j - stripe * s_real)

    # edges into NT dsts
    sel = ~train_mask[col]
    es, ed = row[sel], col[sel]
    src_nt = ~train_mask[es]
    # main: NT->NT (self-loops handled in the evac via the compact tile)
    m_src = pid[es[src_nt]]
    m_dst = pid[ed[src_nt]]
    pm = EdgePlan(m_src, m_dst, nt_pad, s_pad, tn)

    # ---- host-precomputed T-source constants c1/c2 ----
    scaled_li = dis[:, None] * label_init  # [n, 40]
    scaled_hd = dis[:, None] * hard
    es_t, ed_t = es[~src_nt], ed[~src_nt]
    d_pid_t = pid[ed_t]
    order_t = np.argsort(d_pid_t, kind="stable")
    d_sorted = d_pid_t[order_t]
    uniq_d, start_i = np.unique(d_sorted, return_index=True)
    c1_full = np.zeros((nt_pad, 40), dtype=np.float32)
    c2_full = np.zeros((nt_pad, 40), dtype=np.float32)
    if len(uniq_d):
        c1_full[uniq_d] = np.add.reduceat(
            scaled_li[es_t[order_t]].astype(np.float64), start_i, axis=0
        )
        c2_full[uniq_d] = np.add.reduceat(
            scaled_hd[es_t[order_t]].astype(np.float64), start_i, axis=0
        )

    # ---- tables ----
    tbl_init_g = np.zeros((nt_pad, TPAD), dtype=np.float16)
    tbl_init_g[pid[nt_ids], :40] = scaled_li[nt_ids].astype(np.float16)
    own_init_g = tbl_init_g[:, :40].reshape(NCORES, s_pad, 40).copy()

    # ---- per-core MLP / combine inputs ----
    def stripe_rows(ids, srl, spad_, nstripes=NCORES):
        """Return [nstripes, spad_] original-id per padded slot (-1 pad)."""
        m = np.full((nstripes, spad_), -1, dtype=np.int64)
        for i in range(nstripes):
            lo = i * srl
            hi = min(len(ids), (i + 1) * srl)
            if hi > lo:
                m[i, : hi - lo] = ids[lo:hi]
        return m

    nt_map = stripe_rows(nt_ids, s_real, s_pad)
    t_map = stripe_rows(t_ids, st_real, st_pad)

    def take(arr, idmap, fill=0.0):
        out = np.full((idmap.shape[0], idmap.shape[1]) + arr.shape[1:], fill,
                      dtype=arr.dtype)
        valid = idmap >= 0
        out[valid] = arr[idmap[valid]]
        return out

    xnt_g = np.ascontiguousarray(
        take(x, nt_map).astype(np.float16).transpose(0, 2, 1)
    )
    xt_g = np.ascontiguousarray(take(x, t_map).astype(np.float16).transpose(0, 2, 1))
    al_nt_g = take(alpha, nt_map).astype(np.float32)
    al_t_g = take(alpha, t_map).astype(np.float32)
    dis_nt_g = take(dis[:, None], nt_map).astype(np.float32)
    dsq_nt_g = take((dis * dis)[:, None], nt_map).astype(np.float32)
    hard_t_g = take(hard, t_map).astype(np.float32)

    w1t_g = fc1_w.T.astype(np.float16).copy()  # [512, 256]
    b1_g = fc1_b.reshape(256, 1).astype(np.float32)
    w2t_g = fc2_w.T.astype(np.float16).copy()  # [256, 40]
    b2b_g = np.tile(fc2_b.reshape(1, 40), (P, 1)).astype(np.float32)

    nc = _build_program(pm, s_pad, st_pad, tn, tt)

    if os.environ.get("KERNEL_BUILD_ONLY", "0") == "1":
        e = BuildOnly()
        e.nc = nc
        raise e

    in_maps = []
    for i in range(NCORES):
        in_maps.append(
            dict(
                tbl_init=tbl_init_g,
                idx_nt=pm.wrapped_idx(i),
                s_nt=pm.s_blob(i),
                xnt=xnt_g[i],
                xt=xt_g[i],
                w1t=w1t_g,
                b1=b1_g,
                w2t=w2t_g,
                b2b=b2b_g,
                alpha_nt=al_nt_g[i],
                alpha_t=al_t_g[i],
                dis_nt=dis_nt_g[i],
                dissq_nt=dsq_nt_g[i],
                own_init=own_init_g[i],
                c1p=c1_full[i * s_pad : (i + 1) * s_pad],
                c2p=c2_full[i * s_pad : (i + 1) * s_pad],
                hard_t=hard_t_g[i],
            )
        )

    if os.environ.get("KERNEL_SIM", "0") == "1":
        from concourse import bass_interp

        sim = bass_interp.MultiCoreSim(nc, NCORES)
        for i in range(NCORES):
            for k, v in in_maps[i].items():
                sim.cores[i].tensor(k)[:] = v
        sim.simulate()
        results = [
            {k: np.array(sim.cores[i].mem_tensor(k)) for k in ("out_nt", "out_t")}
            for i in range(NCORES)
        ]
        res = None
    else:
        res = run_bass_kernel_spmd(
            nc, in_maps, core_ids=list(range(NCORES)),
            trace=bool(int(os.environ.get("KERNEL_TRACE", "0"))),
        )
        results = res.results
    kernel.last_results = res
    kernel.last_nc = nc
    kernel.last_in_maps = in_maps

    out = np.zeros((n, 40), dtype=np.float32)
    for i in range(NCORES):
        om = results[i]["out_nt"]
        ot = results[i]["out_t"]
        v = nt_map[i] >= 0
        out[nt_map[i][v]] = om[v]
        v = t_map[i] >= 0
        out[t_map[i][v]] = ot[v]
    return out


# revision 11
# speedup vs baseline: 2.2961x; 1.0564x over previous
"""CPFStudent (GNN label propagation + MLP mix) on 8 TRN2 NeuronCores.

Strategy (dst-sharded SpMM with selector matmuls), v2:
  - Reference: 10 PLP steps of plp <- where(mask, hard, A_hat @ plp), with
    A_hat = D^-1/2 (A+I) D^-1/2 built from out-degrees of edge_index[0];
    final logits = sigmoid(alpha)*plp + (1-sigmoid(alpha))*relu(x@W1^T+b1)@W2^T+b2.
  - Only non-train (NT) rows of plp evolve; train (T) rows are constant after
    step 1.  State kept as table = dis * plp (dis = deg^-1/2), fp16:
        plp_new[d] = dis[d] * ( sum_{e: src NT} table[src] + c )
    where c is a per-dst constant: c1 (dis*label_init over T srcs, step 1) or
    c2 (dis*hard over T srcs, steps 2..10).  c1/c2 are precomputed HOST-side
    (they are iteration-invariant) — no T-side SpMM passes on device.
  - Nodes permuted host-side: NT nodes first, padded per-core stripes.  Each
    core owns a contiguous stripe of NT dst rows; edges bucketed by (dst tile
    of 128, src chunk of <=32768) host-side at 16-edge granularity, padded to
    uniform capacities across cores (SPMD), gathered per iteration with
    gpsimd.dma_gather (256B elements, <=1024 idxs per call) from an HBM fp16
    table.  Buckets pack back-to-back inside each call, so a 128-edge matmul
    slot may span two buckets; each overlapped tile gets its own range-masked
    S block (zeros outside its bucket), keeping descriptors ~6% leaner than
    slot-aligned packing.
  - Scatter/segment-sum on the TensorEngine: per 128-edge slot a
    host-precomputed fp8 selector S (S[e,d]=1 iff dst_local(e)==d) multiplies
    the gathered messages, accumulating in PSUM per dst tile.
  - Halo exchange: AllGather of each core's full-width (padded 256B) rows
    directly produces the next iteration's gather table — no post-collective
    re-strided table write.
  - Self-loops are folded into the eviction: the previous iteration's own
    `compact` rows are exactly the self-loop messages, so they never hit the
    gather path.
"""

import math
import os
import sys

import numpy as np

sys.path.insert(0, "/opt/trn_rl_repo")

import ml_dtypes  # noqa: E402

import concourse.bass as bass  # noqa: E402
import concourse.mybir as mybir  # noqa: E402
import concourse.tile as tile  # noqa: E402
from concourse import bacc  # noqa: E402
from concourse.bass_utils import run_bass_kernel_spmd  # noqa: E402

P = 128
NCORES = 8
TPAD = 128  # fp16 elements per table row (256B, dma_gather elem granularity)
GROUP = 7  # dst tiles per dma_gather call group
MAX_CALL = int(os.environ.get("KERNEL_MAX_CALL", "1024"))
NQ = int(os.environ.get("KERNEL_NQ", "2"))
EXCHANGE = os.environ.get("KERNEL_EXCHANGE", "pad")

F16 = mybir.dt.float16
F32 = mybir.dt.float32
F8 = mybir.dt.float8e4
I16 = mybir.dt.int16
NP_F8 = ml_dtypes.float8_e4m3


def _ceil(a, b):
    return -(-a // b)


class BuildOnly(Exception):
    pass


class EdgePlan:
    """Host-side bucketed edge plan for one SpMM pass, uniform across cores.

    src_row: int array, row index into the pass's gather table
    dst_pid: int array, padded NT id of the destination
    """

    def __init__(self, src_row, dst_pid, n_rows, s_pad, n_tiles):
        self.n_chunks = max(1, _ceil(n_rows, 32768))
        self.chunk = _ceil(n_rows, self.n_chunks)
        self.n_tiles = n_tiles
        nch = self.n_chunks

        core = dst_pid // s_pad
        dloc = dst_pid - core * s_pad
        tl = dloc // P
        dstloc = dloc % P
        ch = src_row // self.chunk

        key = (core * n_tiles + tl) * nch + ch
        counts = np.bincount(key, minlength=NCORES * n_tiles * nch).reshape(
            NCORES, n_tiles, nch
        )
        caps = counts.max(axis=0)  # [n_tiles, nch]
        caps = ((caps + 15) // 16) * 16  # 16-granular buckets
        self.caps = caps

        # per (chunk, group) call: buckets packed back-to-back at 16-granule,
        # call padded to 128 so its gather buffer is slot-aligned
        self.n_groups = _ceil(n_tiles, GROUP)
        self.call_num = np.zeros((nch, self.n_groups), dtype=np.int64)
        self.buck_off = np.zeros((nch, n_tiles), dtype=np.int64)
        for c in range(nch):
            for g in range(self.n_groups):
                off = 0
                for t in range(g * GROUP, min((g + 1) * GROUP, n_tiles)):
                    self.buck_off[c, t] = off
                    off += caps[t, c]
                self.call_num[c, g] = _ceil(off, P) * P
        self.call_off = np.zeros((nch, self.n_groups), dtype=np.int64)
        off = 0
        for c in range(nch):
            for g in range(self.n_groups):
                self.call_off[c, g] = off
                off += self.call_num[c, g]
        self.total_idx = off

        # sub-calls of <= MAX_CALL idxs: per (c, g) a list of (idx_off, num, col0)
        self.subcalls = {}
        for c in range(nch):
            for g in range(self.n_groups):
                num = int(self.call_num[c, g])
                base = int(self.call_off[c, g])
                subs = []
                p0 = 0
                while p0 < num:
                    n_ = min(MAX_CALL, num - p0)
                    subs.append((base + p0, n_, p0 // P))
                    p0 += n_
                self.subcalls[(c, g)] = subs

        # per tile: list of (chunk, slot-in-call-buffer) S-blocks, in matmul order
        self.tile_blocks = []
        for t in range(n_tiles):
            blocks = []
            for c in range(nch):
                cap = int(caps[t, c])
                if cap == 0:
                    continue
                p0 = int(self.buck_off[c, t])
                for sl in range(p0 // P, (p0 + cap - 1) // P + 1):
                    blocks.append((c, sl))
            self.tile_blocks.append(blocks)
        self.slots_per_tile = np.array([len(b) for b in self.tile_blocks])
        self.s_off = np.concatenate([[0], np.cumsum(self.slots_per_tile)])
        self.total_slots = int(self.s_off[-1])

        # order edges by (core, chunk, tile); build padded per-core streams
        order = np.argsort((core * nch + ch) * n_tiles + tl, kind="stable")
        src_o = src_row[order]
        core_o = core[order]
        ch_o = ch[order]
        tl_o = tl[order]
        dst_o = dstloc[order]

        base_tc = np.zeros((nch, n_tiles), dtype=np.int64)
        for c in range(nch):
            for g in range(self.n_groups):
                for t in range(g * GROUP, min((g + 1) * GROUP, n_tiles)):
                    base_tc[c, t] = self.call_off[c, g] + self.buck_off[c, t]

        self.idx16 = np.zeros((NCORES, self.total_idx), dtype=np.int16)
        self.dstloc = np.full((NCORES, self.total_idx), -1, dtype=np.int16)
        grp_key = (core_o * nch + ch_o) * n_tiles + tl_o
        uniq, inv, cnt = np.unique(grp_key, return_inverse=True, return_counts=True)
        starts = np.concatenate([[0], np.cumsum(cnt)])[:-1]
        rank = np.arange(len(grp_key)) - starts[inv]
        pos = base_tc[ch_o, tl_o] + rank
        self.idx16[core_o, pos] = (src_o - ch_o * self.chunk).astype(np.int16)
        self.dstloc[core_o, pos] = dst_o.astype(np.int16)

    def wrapped_idx(self, core):
        """[128, total_idx//16] int16, wrapped-16 and replicated to 8 groups."""
        v = self.idx16[core].reshape(-1, 16).T  # [16, total/16]
        return np.tile(v, (8, 1)).copy()

    def s_blob(self, core):
        """[128, total_slots*128] fp8 selector blocks, tile-major.

        Block k of tile t (chunk c, call slot sl) has S[r, d] = 1 iff call
        position 128*sl + r belongs to tile t's (t, c) bucket and maps an
        edge with dst_local d.  Rows outside the bucket range (neighbouring
        tiles sharing the slot, or padding) are zero.
        """
        out = np.zeros((P, self.total_slots * P), dtype=NP_F8)
        iota = np.arange(P, dtype=np.int16)
        for t in range(self.n_tiles):
            si = int(self.s_off[t])
            for (c, sl) in self.tile_blocks[t]:
                g = t // GROUP
                p0 = int(self.buck_off[c, t])
                cap = int(self.caps[t, c])
                gp = 128 * sl + iota.astype(np.int64)  # call positions of rows
                inb = (gp >= p0) & (gp < p0 + cap)
                d = np.full(P, -1, dtype=np.int64)
                d[inb] = self.dstloc[core, int(self.call_off[c, g]) + gp[inb]]
                sblk = np.zeros((P, P), dtype=NP_F8)
                rows = np.nonzero(d >= 0)[0]
                sblk[rows, d[rows]] = 1.0
                out[:, si * P : (si + 1) * P] = sblk
                si += 1
        return out


def _build_program(pm, s_pad, st_pad, tn, tt):
    """pm: main-pass EdgePlan (NT->NT)."""
    nt_pad = NCORES * s_pad
    nc = bacc.Bacc(
        None, target_bir_lowering=False, num_devices=NCORES, num_swdge_queues=NQ
    )

    def param(name, shape, dt, out=False):
        return nc.declare_dram_parameter(name, list(shape), dt, isOutput=out)

    tbl_init = param("tbl_init", (nt_pad, TPAD), F16)
    idx_nt = param("idx_nt", (P, pm.total_idx // 16), I16)
    s_nt = param("s_nt", (P, pm.total_slots * P), F8)
    xnt = param("xnt", (512, s_pad), F16)  # pre-transposed on host
    xt = param("xt", (512, st_pad), F16)
    w1t = param("w1t", (512, 256), F16)
    b1 = param("b1", (256, 1), F32)
    w2t = param("w2t", (256, 40), F16)
    b2b = param("b2b", (P, 40), F32)
    alpha_nt = param("alpha_nt", (s_pad, 1), F32)
    alpha_t = param("alpha_t", (st_pad, 1), F32)
    dis_nt = param("dis_nt", (s_pad, 1), F32)
    dissq_nt = param("dissq_nt", (s_pad, 1), F32)
    own_init = param("own_init", (s_pad, 40), F16)
    c1p = param("c1p", (s_pad, 40), F32)
    c2p = param("c2p", (s_pad, 40), F32)
    hard_t = param("hard_t", (st_pad, 40), F32)
    out_nt = param("out_nt", (s_pad, 40), F32, out=True)
    out_t = param("out_t", (st_pad, 40), F32, out=True)

    if EXCHANGE == "pad":
        cown = nc.dram_tensor("cown", [s_pad, TPAD], F16)
        callg = nc.dram_tensor("callg", [nt_pad, TPAD], F16, addr_space="Shared")
        table = None
    else:
        cown = nc.dram_tensor("cown", [s_pad, 40], F16)
        callg = nc.dram_tensor("callg", [nt_pad, 40], F16, addr_space="Shared")
        table = nc.dram_tensor("table", [nt_pad, TPAD], F16)

    RG = [list(range(NCORES))]

    with tile.TileContext(nc) as tc:
        with (
            tc.tile_pool(name="persist", bufs=1) as pp,
            tc.tile_pool(name="work", bufs=4) as wp,
            tc.tile_pool(name="gpool", bufs=4) as gp,
            tc.tile_pool(name="spool", bufs=3) as sp,
            tc.tile_pool(name="mpsum", bufs=2, space="PSUM") as mp,
            tc.tile_pool(name="apsum", bufs=4, space="PSUM") as ap_,
        ):
            if table is not None:
                nc.sync.dma_start(out=table[:, :], in_=tbl_init[:, :])

            # ---- persistent SBUF ----
            idxm_sb = pp.tile([P, pm.total_idx // 16], I16, tag="idxm")
            nc.sync.dma_start(out=idxm_sb[:], in_=idx_nt[:, :])

            ft_nt = pp.tile([P, tn, 40], F32, tag="ftnt")
            ft_t = pp.tile([P, tt, 40], F32, tag="ftt")
            cwid = TPAD if EXCHANGE == "pad" else 40
            compact = pp.tile([P, tn, cwid], F16, tag="compact")
            if EXCHANGE == "pad":
                nc.vector.memset(compact[:], 0.0)
            nc.sync.dma_start(
                out=compact[:, :, 0:40],
                in_=own_init.ap().rearrange("(t p) c -> p t c", p=P),
            )

            w1_sb = pp.tile([P, 4, 256], F16, tag="w1")
            nc.sync.dma_start(
                out=w1_sb[:], in_=w1t.ap().rearrange("(k p) h -> p k h", p=P)
            )
            w2_sb = pp.tile([P, 2, 40], F16, tag="w2")
            nc.sync.dma_start(
                out=w2_sb[:], in_=w2t.ap().rearrange("(h p) c -> p h c", p=P)
            )
            b1_sb = pp.tile([P, 2], F32, tag="b1")
            nc.sync.dma_start(
                out=b1_sb[:], in_=b1.ap().rearrange("(h p) o -> p (h o)", p=P)
            )
            b2_sb = pp.tile([P, 40], F32, tag="b2")
            nc.sync.dma_start(out=b2_sb[:], in_=b2b[:, :])

            c1 = pp.tile([P, tn, 40], F32, tag="c1")
            nc.sync.dma_start(
                out=c1[:], in_=c1p.ap().rearrange("(t p) c -> p t c", p=P)
            )
            c2 = pp.tile([P, tn, 40], F32, tag="c2")
            nc.sync.dma_start(
                out=c2[:], in_=c2p.ap().rearrange("(t p) c -> p t c", p=P)
            )

            def cols_load(prm, n_tiles, tag):
                t_ = pp.tile([P, n_tiles], F32, tag=tag)
                nc.sync.dma_start(
                    out=t_[:], in_=prm.ap().rearrange("(t p) o -> p (t o)", p=P)
                )
                return t_

            disn_sb = cols_load(dis_nt, tn, "disn")
            dsqn_sb = cols_load(dissq_nt, tn, "dsqn")
            aln_sb = cols_load(alpha_nt, tn, "aln")
            alt_sb = cols_load(alpha_t, tt, "alt")

            # sigmoid(alpha); a*dis; 1-a
            sign_sb = pp.tile([P, tn], F32, tag="sign")
            nc.scalar.activation(
                sign_sb[:], aln_sb[:], mybir.ActivationFunctionType.Sigmoid
            )
            sigt_sb = pp.tile([P, tt], F32, tag="sigt")
            nc.scalar.activation(
                sigt_sb[:], alt_sb[:], mybir.ActivationFunctionType.Sigmoid
            )
            disa_sb = pp.tile([P, tn], F32, tag="disa")
            nc.vector.tensor_tensor(
                out=disa_sb[:], in0=sign_sb[:], in1=disn_sb[:],
                op=mybir.AluOpType.mult,
            )
            oman_sb = pp.tile([P, tn], F32, tag="oman")
            nc.vector.tensor_scalar(
                out=oman_sb[:], in0=sign_sb[:], scalar1=-1.0, scalar2=1.0,
                op0=mybir.AluOpType.mult, op1=mybir.AluOpType.add,
            )
            omat_sb = pp.tile([P, tt], F32, tag="omat")
            nc.vector.tensor_scalar(
                out=omat_sb[:], in0=sigt_sb[:], scalar1=-1.0, scalar2=1.0,
                op0=mybir.AluOpType.mult, op1=mybir.AluOpType.add,
            )

            # ---- MLP (FT branch) ----
            def mlp(xsrc, n_tiles, ft_dst):
                for n in range(n_tiles):
                    xTs = []
                    for k in range(4):
                        xT = wp.tile([P, P], F16, tag="xT")
                        nc.sync.dma_start(
                            out=xT[:],
                            in_=xsrc[k * P : (k + 1) * P, n * P : (n + 1) * P],
                        )
                        xTs.append(xT)
                    ps2 = mp.tile([P, 40], F32, tag="ps2")
                    for h in range(2):
                        ps1 = mp.tile([P, P], F32, tag="ps1")
                        for k in range(4):
                            nc.tensor.matmul(
                                ps1[:],
                                lhsT=w1_sb[:, k, h * P : (h + 1) * P],
                                rhs=xTs[k][:],
                                start=(k == 0),
                                stop=(k == 3),
                            )
                        hT = wp.tile([P, P], F16, tag="hT")
                        nc.scalar.activation(
                            hT[:], ps1[:], mybir.ActivationFunctionType.Relu,
                            bias=b1_sb[:, h : h + 1],
                        )
                        nc.tensor.matmul(
                            ps2[:], lhsT=hT[:], rhs=w2_sb[:, h, :],
                            start=(h == 0), stop=(h == 1),
                        )
                    nc.vector.tensor_tensor(
                        out=ft_dst[:, n, :], in0=ps2[:], in1=b2_sb[:],
                        op=mybir.AluOpType.add,
                    )

            mlp(xnt, tn, ft_nt)
            mlp(xt, tt, ft_t)

            # ---- generic SpMM pass ----
            _regs = {}

            def num_reg(v):
                if v not in _regs:
                    _regs[v] = nc.gpsimd.to_reg(v)
                return _regs[v]

            _q = [0]

            def spmm_pass(plan, tsrc, idx_sb, s_param, evac):
                """tsrc: DRAM table. evac(t, psum_ap) -> emits eviction."""
                nch = plan.n_chunks
                for g in range(plan.n_groups):
                    gbufs = []
                    for c in range(nch):
                        num = int(plan.call_num[c, g])
                        if num == 0:
                            gbufs.append(None)
                            continue
                        gb = gp.tile([P, num // P, TPAD], F16, tag="gb")
                        r0 = c * plan.chunk
                        nrow = plan.chunk
                        if os.environ.get("KERNEL_NO_GATHER", "0") == "1":
                            # debug: sequential read instead of gather
                            nc.sync.dma_start(
                                out=gb[:],
                                in_=tsrc[r0 : r0 + num, :].rearrange(
                                    "(n p) e -> p n e", p=P
                                ),
                            )
                        else:
                            for off, n_, col0 in plan.subcalls[(c, g)]:
                                nc.gpsimd.dma_gather(
                                    out_ap=gb[:, col0 : col0 + n_ // P, :],
                                    in_ap=tsrc[r0 : r0 + nrow, :],
                                    idxs_ap=idx_sb[:, off // 16 : (off + n_) // 16],
                                    num_idxs=n_,
                                    num_idxs_reg=num_reg(n_),
                                    elem_size=TPAD,
                                    queue_num=_q[0] % NQ,
                                )
                                _q[0] += 1
                        gbufs.append(gb)
                    for t in range(g * GROUP, min((g + 1) * GROUP, plan.n_tiles)):
                        tot = int(plan.slots_per_tile[t])
                        if tot == 0:
                            continue
                        si = int(plan.s_off[t])
                        st_ = sp.tile([P, tot * P], F8, tag="sstr")
                        nc.sync.dma_start(
                            out=st_[:], in_=s_param[:, si * P : (si + tot) * P]
                        )
                        ps = ap_.tile([P, 40], F32, tag="acc")
                        for k, (c, sl) in enumerate(plan.tile_blocks[t]):
                            nc.tensor.matmul(
                                ps[:],
                                lhsT=st_[:, k * P : (k + 1) * P],
                                rhs=gbufs[c][:, sl, 0:40],
                                start=(k == 0),
                                stop=(k == tot - 1),
                            )
                        evac(t, ps)

            # ---- 10 PLP iterations ----
            for it in range(10):
                if EXCHANGE == "pad":
                    tsrc = tbl_init if it == 0 else callg
                else:
                    tsrc = table
                cbuf = c1 if it == 0 else c2

                if it < 9:
                    def evac_iter(t, ps, cbuf=cbuf):
                        tmp0 = wp.tile([P, 40], F32, tag="ev0")
                        nc.vector.tensor_tensor(
                            out=tmp0[:], in0=ps[:], in1=compact[:, t, 0:40],
                            op=mybir.AluOpType.add,
                        )
                        tmp = wp.tile([P, 40], F32, tag="ev")
                        nc.vector.tensor_tensor(
                            out=tmp[:], in0=tmp0[:], in1=cbuf[:, t, :],
                            op=mybir.AluOpType.add,
                        )
                        nc.vector.tensor_scalar(
                            out=compact[:, t, 0:40], in0=tmp[:],
                            scalar1=dsqn_sb[:, t : t + 1], scalar2=None,
                            op0=mybir.AluOpType.mult,
                        )
                else:
                    def evac_iter(t, ps, cbuf=cbuf):
                        tmp0 = wp.tile([P, 40], F32, tag="ev0")
                        nc.vector.tensor_tensor(
                            out=tmp0[:], in0=ps[:], in1=compact[:, t, 0:40],
                            op=mybir.AluOpType.add,
                        )
                        tmp = wp.tile([P, 40], F32, tag="ev")
                        nc.vector.tensor_tensor(
                            out=tmp[:], in0=tmp0[:], in1=cbuf[:, t, :],
                            op=mybir.AluOpType.add,
                        )
                        t2 = wp.tile([P, 40], F32, tag="ev2")
                        nc.vector.tensor_scalar(
                            out=t2[:], in0=tmp[:],
                            scalar1=disa_sb[:, t : t + 1], scalar2=None,
                            op0=mybir.AluOpType.mult,
                        )
                        t3 = wp.tile([P, 40], F32, tag="ev3")
                        nc.vector.tensor_scalar(
                            out=t3[:], in0=ft_nt[:, t, :],
                            scalar1=oman_sb[:, t : t + 1], scalar2=None,
                            op0=mybir.AluOpType.mult,
                        )
                        t4 = wp.tile([P, 40], F32, tag="ev4")
                        nc.vector.tensor_tensor(
                            out=t4[:], in0=t2[:], in1=t3[:],
                            op=mybir.AluOpType.add,
                        )
                        nc.sync.dma_start(
                            out=out_nt[t * P : (t + 1) * P, :], in_=t4[:]
                        )

                spmm_pass(pm, tsrc, idxm_sb, s_nt, evac_iter)

                if it < 9:
                    nc.sync.dma_start(
                        out=cown.ap().rearrange("(t p) c -> p t c", p=P),
                        in_=compact[:],
                    )
                    if os.environ.get("KERNEL_NO_CC", "0") == "1":
                        # debug mode: skip the collective (wrong cross-core data)
                        nc.sync.dma_start(
                            out=callg[0 : s_pad, :], in_=cown[:, :]
                        )
                    else:
                        nc.gpsimd.collective_compute(
                            "AllGather",
                            mybir.AluOpType.bypass,
                            replica_groups=RG,
                            ins=[cown.ap().opt()],
                            outs=[callg.ap().opt()],
                        )
                    if EXCHANGE != "pad":
                        nc.sync.dma_start(out=table[:, 0:40], in_=callg[:, :])

            # ---- T-side final combine ----
            for t in range(tt):
                hsb = wp.tile([P, 40], F32, tag="hsb")
                nc.sync.dma_start(out=hsb[:], in_=hard_t[t * P : (t + 1) * P, :])
                t1_ = wp.tile([P, 40], F32, tag="tc1")
                nc.vector.tensor_scalar(
                    out=t1_[:], in0=hsb[:], scalar1=sigt_sb[:, t : t + 1],
                    scalar2=None, op0=mybir.AluOpType.mult,
                )
                t2_ = wp.tile([P, 40], F32, tag="tc2")
                nc.vector.tensor_scalar(
                    out=t2_[:], in0=ft_t[:, t, :], scalar1=omat_sb[:, t : t + 1],
                    scalar2=None, op0=mybir.AluOpType.mult,
                )
                t3_ = wp.tile([P, 40], F32, tag="tc3")
                nc.vector.tensor_tensor(
                    out=t3_[:], in0=t1_[:], in1=t2_[:],
                    op=mybir.AluOpType.add,
                )
                nc.sync.dma_start(out=out_t[t * P : (t + 1) * P, :], in_=t3_[:])

    nc.compile()
    return nc


def kernel(**inputs):
    x = np.asarray(inputs["x"], dtype=np.float32)
    edge_index = np.asarray(inputs["edge_index"])
    label_init = np.asarray(inputs["label_init"], dtype=np.float32)
    train_mask = np.asarray(inputs["train_mask"]).astype(bool)
    hard = np.asarray(inputs["hard_one_hot"], dtype=np.float32)
    fc1_w = np.asarray(inputs["fc1_w"], dtype=np.float32)
    fc1_b = np.asarray(inputs["fc1_b"], dtype=np.float32)
    fc2_w = np.asarray(inputs["fc2_w"], dtype=np.float32)
    fc2_b = np.asarray(inputs["fc2_b"], dtype=np.float32)
    alpha = np.asarray(inputs["alpha"], dtype=np.float32)

    n = x.shape[0]
    row = edge_index[0].astype(np.int64)
    col = edge_index[1].astype(np.int64)

    deg = np.bincount(row, minlength=n).astype(np.float64) + 1.0
    dis = (1.0 / np.sqrt(deg)).astype(np.float32)

    nt_ids = np.nonzero(~train_mask)[0]
    t_ids = np.nonzero(train_mask)[0]
    n_nt, n_t = len(nt_ids), len(t_ids)

    s_real = _ceil(n_nt, NCORES)
    tn = _ceil(s_real, P)
    s_pad = tn * P
    nt_pad = NCORES * s_pad
    st_real = _ceil(n_t, NCORES)
    tt = _ceil(st_real, P)
    st_pad = tt * P

    # padded NT id / compact T id for each original node
    pid = np.full(n, -1, dtype=np.int64)
    j = np.arange(n_nt)
    stripe = j // s_real
    pid[nt_ids] = stripe * s_pad + (j - stripe * s_real)

    # edges into NT dsts
    sel = ~train_mask[col]
    es, ed = row[sel], col[sel]
    src_nt = ~train_mask[es]
    # main: NT->NT (self-loops handled in the evac via the compact tile)
    m_src = pid[es[src_nt]]
    m_dst = pid[ed[src_nt]]
    pm = EdgePlan(m_src, m_dst, nt_pad, s_pad, tn)

    # ---- host-precomputed T-source constants c1/c2 ----
    scaled_li = dis[:, None] * label_init  # [n, 40]
    scaled_hd = dis[:, None] * hard
    es_t, ed_t = es[~src_nt], ed[~src_nt]
    d_pid_t = pid[ed_t]
    order_t = np.argsort(d_pid_t, kind="stable")
    d_sorted = d_pid_t[order_t]
    uniq_d, start_i = np.unique(d_sorted, return_index=True)
    c1_full = np.zeros((nt_pad, 40), dtype=np.float32)
    c2_full = np.zeros((nt_pad, 40), dtype=np.float32)
    if len(uniq_d):
        c1_full[uniq_d] = np.add.reduceat(
            scaled_li[es_t[order_t]].astype(np.float64), start_i, axis=0
        )
        c2_full[uniq_d] = np.add.reduceat(
            scaled_hd[es_t[order_t]].astype(np.float64), start_i, axis=0
        )

    # ---- tables ----
    tbl_init_g = np.zeros((nt_pad, TPAD), dtype=np.float16)
    tbl_init_g[pid[nt_ids], :40] = scaled_li[nt_ids].astype(np.float16)
    own_init_g = tbl_init_g[:, :40].reshape(NCORES, s_pad, 40).copy()

    # ---- per-core MLP / combine inputs ----
    def stripe_rows(ids, srl, spad_, nstripes=NCORES):
        """Return [nstripes, spad_] original-id per padded slot (-1 pad)."""
        m = np.full((nstripes, spad_), -1, dtype=np.int64)
        for i in range(nstripes):
            lo = i * srl
            hi = min(len(ids), (i + 1) * srl)
            if hi > lo:
                m[i, : hi - lo] = ids[lo:hi]
        return m

    nt_map = stripe_rows(nt_ids, s_real, s_pad)
    t_map = stripe_rows(t_ids, st_real, st_pad)

    def take(arr, idmap, fill=0.0):
        out = np.full((idmap.shape[0], idmap.shape[1]) + arr.shape[1:], fill,
                      dtype=arr.dtype)
        valid = idmap >= 0
        out[valid] = arr[idmap[valid]]
        return out

    xnt_g = np.ascontiguousarray(
        take(x, nt_map).astype(np.float16).transpose(0, 2, 1)
    )
    xt_g = np.ascontiguousarray(take(x, t_map).astype(np.float16).transpose(0, 2, 1))
    al_nt_g = take(alpha, nt_map).astype(np.float32)
    al_t_g = take(alpha, t_map).astype(np.float32)
    dis_nt_g = take(dis[:, None], nt_map).astype(np.float32)
    dsq_nt_g = take((dis * dis)[:, None], nt_map).astype(np.float32)
    hard_t_g = take(hard, t_map).astype(np.float32)

    w1t_g = fc1_w.T.astype(np.float16).copy()  # [512, 256]
    b1_g = fc1_b.reshape(256, 1).astype(np.float32)
    w2t_g = fc2_w.T.astype(np.float16).copy()  # [256, 40]
    b2b_g = np.tile(fc2_b.reshape(1, 40), (P, 1)).astype(np.float32)

    nc = _build_program(pm, s_pad, st_pad, tn, tt)

    if os.environ.get("KERNEL_BUILD_ONLY", "0") == "1":
        e = BuildOnly()
        e.nc = nc
        raise e

    in_maps = []
    for i in range(NCORES):
        in_maps.append(
            dict(
                tbl_init=tbl_init_g,
                idx_nt=pm.wrapped_idx(i),
                s_nt=pm.s_blob(i),
                xnt=xnt_g[i],
                xt=xt_g[i],
                w1t=w1t_g,
                b1=b1_g,
                w2t=w2t_g,
                b2b=b2b_g,
                alpha_nt=al_nt_g[i],
                alpha_t=al_t_g[i],
                dis_nt=dis_nt_g[i],
                dissq_nt=dsq_nt_g[i],
                own_init=own_init_g[i],
                c1p=c1_full[i * s_pad : (i + 1) * s_pad],
                c2p=c2_full[i * s_pad : (i + 1) * s_pad],
                hard_t=hard_t_g[i],
            )
        )

    if os.environ.get("KERNEL_SIM", "0") == "1":
        from concourse import bass_interp

        sim = bass_interp.MultiCoreSim(nc, NCORES)
        for i in range(NCORES):
            for k, v in in_maps[i].items():
                sim.cores[i].tensor(k)[:] = v
        sim.simulate()
        results = [
            {k: np.array(sim.cores[i].mem_tensor(k)) for k in ("out_nt", "out_t")}
            for i in range(NCORES)
        ]
        res = None
    else:
        res = run_bass_kernel_spmd(
            nc, in_maps, core_ids=list(range(NCORES)),
            trace=bool(int(os.environ.get("KERNEL_TRACE", "0"))),
        )
        results = res.results
    kernel.last_results = res
    kernel.last_nc = nc
    kernel.last_in_maps = in_maps

    out = np.zeros((n, 40), dtype=np.float32)
    for i in range(NCORES):
        om = results[i]["out_nt"]
        ot = results[i]["out_t"]
        v = nt_map[i] >= 0
        out[nt_map[i][v]] = om[v]
        v = t_map[i] >= 0
        out[t_map[i][v]] = ot[v]
    return out


# revision 12
# speedup vs baseline: 2.4308x; 1.0587x over previous
"""CPFStudent (GNN label propagation + MLP mix) on 8 TRN2 NeuronCores.

Strategy (dst-sharded SpMM with selector matmuls), v2:
  - Reference: 10 PLP steps of plp <- where(mask, hard, A_hat @ plp), with
    A_hat = D^-1/2 (A+I) D^-1/2 built from out-degrees of edge_index[0];
    final logits = sigmoid(alpha)*plp + (1-sigmoid(alpha))*relu(x@W1^T+b1)@W2^T+b2.
  - Only non-train (NT) rows of plp evolve; train (T) rows are constant after
    step 1.  State kept as table = dis * plp (dis = deg^-1/2), fp16:
        plp_new[d] = dis[d] * ( sum_{e: src NT} table[src] + c )
    where c is a per-dst constant: c1 (dis*label_init over T srcs, step 1) or
    c2 (dis*hard over T srcs, steps 2..10).  c1/c2 are precomputed HOST-side
    (they are iteration-invariant) — no T-side SpMM passes on device.
  - Nodes permuted host-side: NT nodes first, padded per-core stripes.  Each
    core owns a contiguous stripe of NT dst rows; edges bucketed by (dst tile
    of 128, src chunk of <=32768) host-side at 16-edge granularity, padded to
    uniform capacities across cores (SPMD), gathered per iteration with
    gpsimd.dma_gather (256B elements, <=1024 idxs per call, calls
    round-robined over 2 SWDGE queues — 2x the single-queue rate) from an
    HBM fp16 table.  Buckets pack back-to-back inside each call, so a 128-edge matmul
    slot may span two buckets; each overlapped tile gets its own range-masked
    S block (zeros outside its bucket), keeping descriptors ~6% leaner than
    slot-aligned packing.
  - Scatter/segment-sum on the TensorEngine: per 128-edge slot a
    host-precomputed fp8 selector S (S[e,d]=1 iff dst_local(e)==d) multiplies
    the gathered messages, accumulating in PSUM per dst tile.
  - Halo exchange: AllGather of each core's full-width (padded 256B) rows
    directly produces the next iteration's gather table — no post-collective
    re-strided table write.
  - Self-loops are folded into the eviction: the previous iteration's own
    `compact` rows are exactly the self-loop messages, so they never hit the
    gather path.
"""

import math
import os
import sys

import numpy as np

sys.path.insert(0, "/opt/trn_rl_repo")

import ml_dtypes  # noqa: E402

import concourse.bass as bass  # noqa: E402
import concourse.mybir as mybir  # noqa: E402
import concourse.tile as tile  # noqa: E402
from concourse import bacc  # noqa: E402
from concourse.bass_utils import run_bass_kernel_spmd  # noqa: E402

P = 128
NCORES = 8
TPAD = 128  # fp16 elements per table row (256B, dma_gather elem granularity)
GROUP = 7  # dst tiles per dma_gather call group
MAX_CALL = int(os.environ.get("KERNEL_MAX_CALL", "1024"))
NQ = int(os.environ.get("KERNEL_NQ", "2"))
EXCHANGE = os.environ.get("KERNEL_EXCHANGE", "pad")

F16 = mybir.dt.float16
F32 = mybir.dt.float32
F8 = mybir.dt.float8e4
I16 = mybir.dt.int16
NP_F8 = ml_dtypes.float8_e4m3


def _ceil(a, b):
    return -(-a // b)


class BuildOnly(Exception):
    pass


class EdgePlan:
    """Host-side bucketed edge plan for one SpMM pass, uniform across cores.

    src_row: int array, row index into the pass's gather table
    dst_pid: int array, padded NT id of the destination
    """

    def __init__(self, src_row, dst_pid, n_rows, s_pad, n_tiles):
        self.n_chunks = max(1, _ceil(n_rows, 32768))
        self.chunk = _ceil(n_rows, self.n_chunks)
        self.n_tiles = n_tiles
        nch = self.n_chunks

        core = dst_pid // s_pad
        dloc = dst_pid - core * s_pad
        tl = dloc // P
        dstloc = dloc % P
        ch = src_row // self.chunk

        key = (core * n_tiles + tl) * nch + ch
        counts = np.bincount(key, minlength=NCORES * n_tiles * nch).reshape(
            NCORES, n_tiles, nch
        )
        caps = counts.max(axis=0)  # [n_tiles, nch]
        caps = ((caps + 15) // 16) * 16  # 16-granular buckets
        self.caps = caps

        # per (chunk, group) call: buckets packed back-to-back at 16-granule,
        # call padded to 128 so its gather buffer is slot-aligned
        self.n_groups = _ceil(n_tiles, GROUP)
        self.call_num = np.zeros((nch, self.n_groups), dtype=np.int64)
        self.buck_off = np.zeros((nch, n_tiles), dtype=np.int64)
        for c in range(nch):
            for g in range(self.n_groups):
                off = 0
                for t in range(g * GROUP, min((g + 1) * GROUP, n_tiles)):
                    self.buck_off[c, t] = off
                    off += caps[t, c]
                self.call_num[c, g] = _ceil(off, P) * P
        self.call_off = np.zeros((nch, self.n_groups), dtype=np.int64)
        off = 0
        for c in range(nch):
            for g in range(self.n_groups):
                self.call_off[c, g] = off
                off += self.call_num[c, g]
        self.total_idx = off

        # sub-calls of <= MAX_CALL idxs: per (c, g) a list of (idx_off, num, col0)
        self.subcalls = {}
        for c in range(nch):
            for g in range(self.n_groups):
                num = int(self.call_num[c, g])
                base = int(self.call_off[c, g])
                subs = []
                p0 = 0
                while p0 < num:
                    n_ = min(MAX_CALL, num - p0)
                    subs.append((base + p0, n_, p0 // P))
                    p0 += n_
                self.subcalls[(c, g)] = subs

        # per tile: list of (chunk, slot-in-call-buffer) S-blocks, in matmul order
        self.tile_blocks = []
        for t in range(n_tiles):
            blocks = []
            for c in range(nch):
                cap = int(caps[t, c])
                if cap == 0:
                    continue
                p0 = int(self.buck_off[c, t])
                for sl in range(p0 // P, (p0 + cap - 1) // P + 1):
                    blocks.append((c, sl))
            self.tile_blocks.append(blocks)
        self.slots_per_tile = np.array([len(b) for b in self.tile_blocks])
        self.s_off = np.concatenate([[0], np.cumsum(self.slots_per_tile)])
        self.total_slots = int(self.s_off[-1])

        # order edges by (core, chunk, tile); build padded per-core streams
        order = np.argsort((core * nch + ch) * n_tiles + tl, kind="stable")
        src_o = src_row[order]
        core_o = core[order]
        ch_o = ch[order]
        tl_o = tl[order]
        dst_o = dstloc[order]

        base_tc = np.zeros((nch, n_tiles), dtype=np.int64)
        for c in range(nch):
            for g in range(self.n_groups):
                for t in range(g * GROUP, min((g + 1) * GROUP, n_tiles)):
                    base_tc[c, t] = self.call_off[c, g] + self.buck_off[c, t]

        self.idx16 = np.zeros((NCORES, self.total_idx), dtype=np.int16)
        self.dstloc = np.full((NCORES, self.total_idx), -1, dtype=np.int16)
        grp_key = (core_o * nch + ch_o) * n_tiles + tl_o
        uniq, inv, cnt = np.unique(grp_key, return_inverse=True, return_counts=True)
        starts = np.concatenate([[0], np.cumsum(cnt)])[:-1]
        rank = np.arange(len(grp_key)) - starts[inv]
        pos = base_tc[ch_o, tl_o] + rank
        self.idx16[core_o, pos] = (src_o - ch_o * self.chunk).astype(np.int16)
        self.dstloc[core_o, pos] = dst_o.astype(np.int16)

    def wrapped_idx(self, core):
        """[128, total_idx//16] int16, wrapped-16 and replicated to 8 groups."""
        v = self.idx16[core].reshape(-1, 16).T  # [16, total/16]
        return np.tile(v, (8, 1)).copy()

    def s_blob(self, core):
        """[128, total_slots*128] fp8 selector blocks, tile-major.

        Block k of tile t (chunk c, call slot sl) has S[r, d] = 1 iff call
        position 128*sl + r belongs to tile t's (t, c) bucket and maps an
        edge with dst_local d.  Rows outside the bucket range (neighbouring
        tiles sharing the slot, or padding) are zero.
        """
        out = np.zeros((P, self.total_slots * P), dtype=NP_F8)
        iota = np.arange(P, dtype=np.int16)
        for t in range(self.n_tiles):
            si = int(self.s_off[t])
            for (c, sl) in self.tile_blocks[t]:
                g = t // GROUP
                p0 = int(self.buck_off[c, t])
                cap = int(self.caps[t, c])
                gp = 128 * sl + iota.astype(np.int64)  # call positions of rows
                inb = (gp >= p0) & (gp < p0 + cap)
                d = np.full(P, -1, dtype=np.int64)
                d[inb] = self.dstloc[core, int(self.call_off[c, g]) + gp[inb]]
                sblk = np.zeros((P, P), dtype=NP_F8)
                rows = np.nonzero(d >= 0)[0]
                sblk[rows, d[rows]] = 1.0
                out[:, si * P : (si + 1) * P] = sblk
                si += 1
        return out


def _build_program(pm, s_pad, st_pad, tn, tt):
    """pm: main-pass EdgePlan (NT->NT)."""
    nt_pad = NCORES * s_pad
    nc = bacc.Bacc(
        None, target_bir_lowering=False, num_devices=NCORES, num_swdge_queues=NQ
    )

    def param(name, shape, dt, out=False):
        return nc.declare_dram_parameter(name, list(shape), dt, isOutput=out)

    tbl_init = param("tbl_init", (nt_pad, TPAD), F16)
    idx_nt = param("idx_nt", (P, pm.total_idx // 16), I16)
    s_nt = param("s_nt", (P, pm.total_slots * P), F8)
    xnt = param("xnt", (512, s_pad), F16)  # pre-transposed on host
    xt = param("xt", (512, st_pad), F16)
    w1t = param("w1t", (512, 256), F16)
    b1 = param("b1", (256, 1), F32)
    w2t = param("w2t", (256, 40), F16)
    b2b = param("b2b", (P, 40), F32)
    alpha_nt = param("alpha_nt", (s_pad, 1), F32)
    alpha_t = param("alpha_t", (st_pad, 1), F32)
    dis_nt = param("dis_nt", (s_pad, 1), F32)
    dissq_nt = param("dissq_nt", (s_pad, 1), F32)
    own_init = param("own_init", (s_pad, 40), F16)
    c1p = param("c1p", (s_pad, 40), F32)
    c2p = param("c2p", (s_pad, 40), F32)
    hard_t = param("hard_t", (st_pad, 40), F32)
    out_nt = param("out_nt", (s_pad, 40), F32, out=True)
    out_t = param("out_t", (st_pad, 40), F32, out=True)

    if EXCHANGE == "pad":
        cown = nc.dram_tensor("cown", [s_pad, TPAD], F16)
        callg = nc.dram_tensor("callg", [nt_pad, TPAD], F16, addr_space="Shared")
        table = None
    else:
        cown = nc.dram_tensor("cown", [s_pad, 40], F16)
        callg = nc.dram_tensor("callg", [nt_pad, 40], F16, addr_space="Shared")
        table = nc.dram_tensor("table", [nt_pad, TPAD], F16)

    RG = [list(range(NCORES))]

    with tile.TileContext(nc) as tc:
        with (
            tc.tile_pool(name="persist", bufs=1) as pp,
            tc.tile_pool(name="work", bufs=4) as wp,
            tc.tile_pool(name="gpool", bufs=4) as gp,
            tc.tile_pool(name="spool", bufs=3) as sp,
            tc.tile_pool(name="mpsum", bufs=2, space="PSUM") as mp,
            tc.tile_pool(name="apsum", bufs=4, space="PSUM") as ap_,
        ):
            if table is not None:
                nc.sync.dma_start(out=table[:, :], in_=tbl_init[:, :])

            # ---- persistent SBUF ----
            idxm_sb = pp.tile([P, pm.total_idx // 16], I16, tag="idxm")
            nc.sync.dma_start(out=idxm_sb[:], in_=idx_nt[:, :])

            ft_nt = pp.tile([P, tn, 40], F32, tag="ftnt")
            ft_t = pp.tile([P, tt, 40], F32, tag="ftt")
            cwid = TPAD if EXCHANGE == "pad" else 40
            compact = pp.tile([P, tn, cwid], F16, tag="compact")
            if EXCHANGE == "pad":
                nc.vector.memset(compact[:], 0.0)
            nc.sync.dma_start(
                out=compact[:, :, 0:40],
                in_=own_init.ap().rearrange("(t p) c -> p t c", p=P),
            )

            w1_sb = pp.tile([P, 4, 256], F16, tag="w1")
            nc.sync.dma_start(
                out=w1_sb[:], in_=w1t.ap().rearrange("(k p) h -> p k h", p=P)
            )
            w2_sb = pp.tile([P, 2, 40], F16, tag="w2")
            nc.sync.dma_start(
                out=w2_sb[:], in_=w2t.ap().rearrange("(h p) c -> p h c", p=P)
            )
            b1_sb = pp.tile([P, 2], F32, tag="b1")
            nc.sync.dma_start(
                out=b1_sb[:], in_=b1.ap().rearrange("(h p) o -> p (h o)", p=P)
            )
            b2_sb = pp.tile([P, 40], F32, tag="b2")
            nc.sync.dma_start(out=b2_sb[:], in_=b2b[:, :])

            c1 = pp.tile([P, tn, 40], F32, tag="c1")
            nc.sync.dma_start(
                out=c1[:], in_=c1p.ap().rearrange("(t p) c -> p t c", p=P)
            )
            c2 = pp.tile([P, tn, 40], F32, tag="c2")
            nc.sync.dma_start(
                out=c2[:], in_=c2p.ap().rearrange("(t p) c -> p t c", p=P)
            )

            def cols_load(prm, n_tiles, tag):
                t_ = pp.tile([P, n_tiles], F32, tag=tag)
                nc.sync.dma_start(
                    out=t_[:], in_=prm.ap().rearrange("(t p) o -> p (t o)", p=P)
                )
                return t_

            disn_sb = cols_load(dis_nt, tn, "disn")
            dsqn_sb = cols_load(dissq_nt, tn, "dsqn")
            aln_sb = cols_load(alpha_nt, tn, "aln")
            alt_sb = cols_load(alpha_t, tt, "alt")

            # sigmoid(alpha); a*dis; 1-a
            sign_sb = pp.tile([P, tn], F32, tag="sign")
            nc.scalar.activation(
                sign_sb[:], aln_sb[:], mybir.ActivationFunctionType.Sigmoid
            )
            sigt_sb = pp.tile([P, tt], F32, tag="sigt")
            nc.scalar.activation(
                sigt_sb[:], alt_sb[:], mybir.ActivationFunctionType.Sigmoid
            )
            disa_sb = pp.tile([P, tn], F32, tag="disa")
            nc.vector.tensor_tensor(
                out=disa_sb[:], in0=sign_sb[:], in1=disn_sb[:],
                op=mybir.AluOpType.mult,
            )
            oman_sb = pp.tile([P, tn], F32, tag="oman")
            nc.vector.tensor_scalar(
                out=oman_sb[:], in0=sign_sb[:], scalar1=-1.0, scalar2=1.0,
                op0=mybir.AluOpType.mult, op1=mybir.AluOpType.add,
            )
            omat_sb = pp.tile([P, tt], F32, tag="omat")
            nc.vector.tensor_scalar(
                out=omat_sb[:], in0=sigt_sb[:], scalar1=-1.0, scalar2=1.0,
                op0=mybir.AluOpType.mult, op1=mybir.AluOpType.add,
            )

            # ---- MLP (FT branch) ----
            def mlp(xsrc, n_tiles, ft_dst):
                for n in range(n_tiles):
                    xTs = []
                    for k in range(4):
                        xT = wp.tile([P, P], F16, tag="xT")
                        nc.sync.dma_start(
                            out=xT[:],
                            in_=xsrc[k * P : (k + 1) * P, n * P : (n + 1) * P],
                        )
                        xTs.append(xT)
                    ps2 = mp.tile([P, 40], F32, tag="ps2")
                    for h in range(2):
                        ps1 = mp.tile([P, P], F32, tag="ps1")
                        for k in range(4):
                            nc.tensor.matmul(
                                ps1[:],
                                lhsT=w1_sb[:, k, h * P : (h + 1) * P],
                                rhs=xTs[k][:],
                                start=(k == 0),
                                stop=(k == 3),
                            )
                        hT = wp.tile([P, P], F16, tag="hT")
                        nc.scalar.activation(
                            hT[:], ps1[:], mybir.ActivationFunctionType.Relu,
                            bias=b1_sb[:, h : h + 1],
                        )
                        nc.tensor.matmul(
                            ps2[:], lhsT=hT[:], rhs=w2_sb[:, h, :],
                            start=(h == 0), stop=(h == 1),
                        )
                    nc.vector.tensor_tensor(
                        out=ft_dst[:, n, :], in0=ps2[:], in1=b2_sb[:],
                        op=mybir.AluOpType.add,
                    )

            mlp(xnt, tn, ft_nt)
            mlp(xt, tt, ft_t)

            # ---- generic SpMM pass ----
            _regs = {}

            def num_reg(v):
                if v not in _regs:
                    _regs[v] = nc.gpsimd.to_reg(v)
                return _regs[v]

            _q = [0]

            def spmm_pass(plan, tsrc, idx_sb, s_param, evac):
                """tsrc: DRAM table. evac(t, psum_ap) -> emits eviction."""
                nch = plan.n_chunks
                for g in range(plan.n_groups):
                    gbufs = []
                    for c in range(nch):
                        num = int(plan.call_num[c, g])
                        if num == 0:
                            gbufs.append(None)
                            continue
                        gb = gp.tile([P, num // P, TPAD], F16, tag="gb")
                        r0 = c * plan.chunk
                        nrow = plan.chunk
                        if os.environ.get("KERNEL_NO_GATHER", "0") == "1":
                            # debug: sequential read instead of gather
                            nc.sync.dma_start(
                                out=gb[:],
                                in_=tsrc[r0 : r0 + num, :].rearrange(
                                    "(n p) e -> p n e", p=P
                                ),
                            )
                        else:
                            for off, n_, col0 in plan.subcalls[(c, g)]:
                                nc.gpsimd.dma_gather(
                                    out_ap=gb[:, col0 : col0 + n_ // P, :],
                                    in_ap=tsrc[r0 : r0 + nrow, :],
                                    idxs_ap=idx_sb[:, off // 16 : (off + n_) // 16],
                                    num_idxs=n_,
                                    num_idxs_reg=num_reg(n_),
                                    elem_size=TPAD,
                                    queue_num=_q[0] % NQ,
                                )
                                _q[0] += 1
                        gbufs.append(gb)
                    for t in range(g * GROUP, min((g + 1) * GROUP, plan.n_tiles)):
                        tot = int(plan.slots_per_tile[t])
                        if tot == 0:
                            continue
                        si = int(plan.s_off[t])
                        st_ = sp.tile([P, tot * P], F8, tag="sstr")
                        nc.sync.dma_start(
                            out=st_[:], in_=s_param[:, si * P : (si + tot) * P]
                        )
                        ps = ap_.tile([P, 40], F32, tag="acc")
                        for k, (c, sl) in enumerate(plan.tile_blocks[t]):
                            nc.tensor.matmul(
                                ps[:],
                                lhsT=st_[:, k * P : (k + 1) * P],
                                rhs=gbufs[c][:, sl, 0:40],
                                start=(k == 0),
                                stop=(k == tot - 1),
                            )
                        evac(t, ps)

            # ---- 10 PLP iterations ----
            for it in range(10):
                if EXCHANGE == "pad":
                    tsrc = tbl_init if it == 0 else callg
                else:
                    tsrc = table
                cbuf = c1 if it == 0 else c2

                if it < 9:
                    def evac_iter(t, ps, cbuf=cbuf):
                        tmp0 = wp.tile([P, 40], F32, tag="ev0")
                        nc.vector.tensor_tensor(
                            out=tmp0[:], in0=ps[:], in1=compact[:, t, 0:40],
                            op=mybir.AluOpType.add,
                        )
                        tmp = wp.tile([P, 40], F32, tag="ev")
                        nc.vector.tensor_tensor(
                            out=tmp[:], in0=tmp0[:], in1=cbuf[:, t, :],
                            op=mybir.AluOpType.add,
                        )
                        nc.vector.tensor_scalar(
                            out=compact[:, t, 0:40], in0=tmp[:],
                            scalar1=dsqn_sb[:, t : t + 1], scalar2=None,
                            op0=mybir.AluOpType.mult,
                        )
                else:
                    def evac_iter(t, ps, cbuf=cbuf):
                        tmp0 = wp.tile([P, 40], F32, tag="ev0")
                        nc.vector.tensor_tensor(
                            out=tmp0[:], in0=ps[:], in1=compact[:, t, 0:40],
                            op=mybir.AluOpType.add,
                        )
                        tmp = wp.tile([P, 40], F32, tag="ev")
                        nc.vector.tensor_tensor(
                            out=tmp[:], in0=tmp0[:], in1=cbuf[:, t, :],
                            op=mybir.AluOpType.add,
                        )
                        t2 = wp.tile([P, 40], F32, tag="ev2")
                        nc.vector.tensor_scalar(
                            out=t2[:], in0=tmp[:],
                            scalar1=disa_sb[:, t : t + 1], scalar2=None,
                            op0=mybir.AluOpType.mult,
                        )
                        t3 = wp.tile([P, 40], F32, tag="ev3")
                        nc.vector.tensor_scalar(
                            out=t3[:], in0=ft_nt[:, t, :],
                            scalar1=oman_sb[:, t : t + 1], scalar2=None,
                            op0=mybir.AluOpType.mult,
                        )
                        t4 = wp.tile([P, 40], F32, tag="ev4")
                        nc.vector.tensor_tensor(
                            out=t4[:], in0=t2[:], in1=t3[:],
                            op=mybir.AluOpType.add,
                        )
                        nc.sync.dma_start(
                            out=out_nt[t * P : (t + 1) * P, :], in_=t4[:]
                        )

                spmm_pass(pm, tsrc, idxm_sb, s_nt, evac_iter)

                if it < 9:
                    nc.sync.dma_start(
                        out=cown.ap().rearrange("(t p) c -> p t c", p=P),
                        in_=compact[:],
                    )
                    if os.environ.get("KERNEL_NO_CC", "0") == "1":
                        # debug mode: skip the collective (wrong cross-core data)
                        nc.sync.dma_start(
                            out=callg[0 : s_pad, :], in_=cown[:, :]
                        )
                    else:
                        nc.gpsimd.collective_compute(
                            "AllGather",
                            mybir.AluOpType.bypass,
                            replica_groups=RG,
                            ins=[cown.ap().opt()],
                            outs=[callg.ap().opt()],
                        )
                    if EXCHANGE != "pad":
                        nc.sync.dma_start(out=table[:, 0:40], in_=callg[:, :])

            # ---- T-side final combine ----
            for t in range(tt):
                hsb = wp.tile([P, 40], F32, tag="hsb")
                nc.sync.dma_start(out=hsb[:], in_=hard_t[t * P : (t + 1) * P, :])
                t1_ = wp.tile([P, 40], F32, tag="tc1")
                nc.vector.tensor_scalar(
                    out=t1_[:], in0=hsb[:], scalar1=sigt_sb[:, t : t + 1],
                    scalar2=None, op0=mybir.AluOpType.mult,
                )
                t2_ = wp.tile([P, 40], F32, tag="tc2")
                nc.vector.tensor_scalar(
                    out=t2_[:], in0=ft_t[:, t, :], scalar1=omat_sb[:, t : t + 1],
                    scalar2=None, op0=mybir.AluOpType.mult,
                )
                t3_ = wp.tile([P, 40], F32, tag="tc3")
                nc.vector.tensor_tensor(
                    out=t3_[:], in0=t1_[:], in1=t2_[:],
                    op=mybir.AluOpType.add,
                )
                nc.sync.dma_start(out=out_t[t * P : (t + 1) * P, :], in_=t3_[:])

    nc.compile()
    return nc


def kernel(**inputs):
    x = np.asarray(inputs["x"], dtype=np.float32)
    edge_index = np.asarray(inputs["edge_index"])
    label_init = np.asarray(inputs["label_init"], dtype=np.float32)
    train_mask = np.asarray(inputs["train_mask"]).astype(bool)
    hard = np.asarray(inputs["hard_one_hot"], dtype=np.float32)
    fc1_w = np.asarray(inputs["fc1_w"], dtype=np.float32)
    fc1_b = np.asarray(inputs["fc1_b"], dtype=np.float32)
    fc2_w = np.asarray(inputs["fc2_w"], dtype=np.float32)
    fc2_b = np.asarray(inputs["fc2_b"], dtype=np.float32)
    alpha = np.asarray(inputs["alpha"], dtype=np.float32)

    n = x.shape[0]
    row = edge_index[0].astype(np.int64)
    col = edge_index[1].astype(np.int64)

    deg = np.bincount(row, minlength=n).astype(np.float64) + 1.0
    dis = (1.0 / np.sqrt(deg)).astype(np.float32)

    nt_ids = np.nonzero(~train_mask)[0]
    t_ids = np.nonzero(train_mask)[0]
    n_nt, n_t = len(nt_ids), len(t_ids)

    s_real = _ceil(n_nt, NCORES)
    tn = _ceil(s_real, P)
    s_pad = tn * P
    nt_pad = NCORES * s_pad
    st_real = _ceil(n_t, NCORES)
    tt = _ceil(st_real, P)
    st_pad = tt * P

    # padded NT id / compact T id for each original node
    pid = np.full(n, -1, dtype=np.int64)
    j = np.arange(n_nt)
    stripe = j // s_real
    pid[nt_ids] = stripe * s_pad + (j - stripe * s_real)

    # edges into NT dsts
    sel = ~train_mask[col]
    es, ed = row[sel], col[sel]
    src_nt = ~train_mask[es]
    # main: NT->NT (self-loops handled in the evac via the compact tile)
    m_src = pid[es[src_nt]]
    m_dst = pid[ed[src_nt]]
    pm = EdgePlan(m_src, m_dst, nt_pad, s_pad, tn)

    # ---- host-precomputed T-source constants c1/c2 ----
    scaled_li = dis[:, None] * label_init  # [n, 40]
    scaled_hd = dis[:, None] * hard
    es_t, ed_t = es[~src_nt], ed[~src_nt]
    d_pid_t = pid[ed_t]
    order_t = np.argsort(d_pid_t, kind="stable")
    d_sorted = d_pid_t[order_t]
    uniq_d, start_i = np.unique(d_sorted, return_index=True)
    c1_full = np.zeros((nt_pad, 40), dtype=np.float32)
    c2_full = np.zeros((nt_pad, 40), dtype=np.float32)
    if len(uniq_d):
        c1_full[uniq_d] = np.add.reduceat(
            scaled_li[es_t[order_t]].astype(np.float64), start_i, axis=0
        )
        c2_full[uniq_d] = np.add.reduceat(
            scaled_hd[es_t[order_t]].astype(np.float64), start_i, axis=0
        )

    # ---- tables ----
    tbl_init_g = np.zeros((nt_pad, TPAD), dtype=np.float16)
    tbl_init_g[pid[nt_ids], :40] = scaled_li[nt_ids].astype(np.float16)
    own_init_g = tbl_init_g[:, :40].reshape(NCORES, s_pad, 40).copy()

    # ---- per-core MLP / combine inputs ----
    def stripe_rows(ids, srl, spad_, nstripes=NCORES):
        """Return [nstripes, spad_] original-id per padded slot (-1 pad)."""
        m = np.full((nstripes, spad_), -1, dtype=np.int64)
        for i in range(nstripes):
            lo = i * srl
            hi = min(len(ids), (i + 1) * srl)
            if hi > lo:
                m[i, : hi - lo] = ids[lo:hi]
        return m

    nt_map = stripe_rows(nt_ids, s_real, s_pad)
    t_map = stripe_rows(t_ids, st_real, st_pad)

    def take(arr, idmap, fill=0.0):
        out = np.full((idmap.shape[0], idmap.shape[1]) + arr.shape[1:], fill,
                      dtype=arr.dtype)
        valid = idmap >= 0
        out[valid] = arr[idmap[valid]]
        return out

    xnt_g = np.ascontiguousarray(
        take(x, nt_map).astype(np.float16).transpose(0, 2, 1)
    )
    xt_g = np.ascontiguousarray(take(x, t_map).astype(np.float16).transpose(0, 2, 1))
    al_nt_g = take(alpha, nt_map).astype(np.float32)
    al_t_g = take(alpha, t_map).astype(np.float32)
    dis_nt_g = take(dis[:, None], nt_map).astype(np.float32)
    dsq_nt_g = take((dis * dis)[:, None], nt_map).astype(np.float32)
    hard_t_g = take(hard, t_map).astype(np.float32)

    w1t_g = fc1_w.T.astype(np.float16).copy()  # [512, 256]
    b1_g = fc1_b.reshape(256, 1).astype(np.float32)
    w2t_g = fc2_w.T.astype(np.float16).copy()  # [256, 40]
    b2b_g = np.tile(fc2_b.reshape(1, 40), (P, 1)).astype(np.float32)

    nc = _build_program(pm, s_pad, st_pad, tn, tt)

    if os.environ.get("KERNEL_BUILD_ONLY", "0") == "1":
        e = BuildOnly()
        e.nc = nc
        raise e

    in_maps = []
    for i in range(NCORES):
        in_maps.append(
            dict(
                tbl_init=tbl_init_g,
                idx_nt=pm.wrapped_idx(i),
                s_nt=pm.s_blob(i),
                xnt=xnt_g[i],
                xt=xt_g[i],
                w1t=w1t_g,
                b1=b1_g,
                w2t=w2t_g,
                b2b=b2b_g,
                alpha_nt=al_nt_g[i],
                alpha_t=al_t_g[i],
                dis_nt=dis_nt_g[i],
                dissq_nt=dsq_nt_g[i],
                own_init=own_init_g[i],
                c1p=c1_full[i * s_pad : (i + 1) * s_pad],
                c2p=c2_full[i * s_pad : (i + 1) * s_pad],
                hard_t=hard_t_g[i],
            )
        )

    if os.environ.get("KERNEL_SIM", "0") == "1":
        from concourse import bass_interp

        sim = bass_interp.MultiCoreSim(nc, NCORES)
        for i in range(NCORES):
            for k, v in in_maps[i].items():
                sim.cores[i].tensor(k)[:] = v
        sim.simulate()
        results = [
            {k: np.array(sim.cores[i].mem_tensor(k)) for k in ("out_nt", "out_t")}
            for i in range(NCORES)
        ]
        res = None
    else:
        res = run_bass_kernel_spmd(
            nc, in_maps, core_ids=list(range(NCORES)),
            trace=bool(int(os.environ.get("KERNEL_TRACE", "0"))),
        )
        results = res.results
    kernel.last_results = res
    kernel.last_nc = nc
    kernel.last_in_maps = in_maps

    out = np.zeros((n, 40), dtype=np.float32)
    for i in range(NCORES):
        om = results[i]["out_nt"]
        ot = results[i]["out_t"]
        v = nt_map[i] >= 0
        out[nt_map[i][v]] = om[v]
        v = t_map[i] >= 0
        out[t_map[i][v]] = ot[v]
    return out
